# revision 1
# baseline (speedup 1.0000x reference)
"""Trainium2 Bass kernel for a 16-head dense attention layer (v2, bf16).

Problem: x[1,4096,1024] @ w_qkv[1024,3072] -> 16-head attention (N=4096,
D=64) -> @ w_out[1024,1024].

Sharding: tensor-parallel over heads across 8 NeuronCores (2 heads/core).
Each core computes q/k/v for its 2 heads (weights column-sliced on host),
attention with a fused, max-free softmax (scores are bounded so exp never
overflows in fp32; denominator comes from an appended ones-column in V),
then an AllToAll converts the head-sharded attention output into a
sequence-sharded layout so every core applies the full output projection
to its own 512 rows. Host concatenates the 8 row slices.

vs the f32r baseline: all matmul operands bf16 (PSUM accumulation stays
fp32; rel-err budget 2e-2), V projected directly in [keys, dims] layout
(no PE transposes), merged x DMAs prefetched on the Pool queue, lag-1
attention interleave in phase 1, one AllToAll per stripe, and attention
software-pipelined with AV matmuls lagging LAG=9 units behind their
scores+exp: the ACT engine's exp backlog rides through the consolidated
stripe-boundary blocks (older stripes' output projection plus the
next-next stripe's q projection, allocated from just-freed ps_acc
buffers) without starving, and the last stripe drains its lag early so
it does not flush as pure tail.
"""

import os
import numpy as np

N_CORES = 8
N = 4096
HIDDEN = 1024
D = 64
HPC = 2  # heads per core
AD = HPC * D  # 128 att-dim rows per core
NT = N // 128  # 32 k-tiles of 128
HT = HIDDEN // 128  # 8 hidden tiles
QCHUNK = 1024
NQC = N // QCHUNK  # 4 q-chunks (stripes)
NSLICE = N // N_CORES  # 512 rows of output per core

_CACHE = {}


def _build(mm_mode: str = "bf16", skip_a2a: bool = False, att_nt: int = NT):
    import concourse.bass as bass
    import concourse.mybir as mybir
    import concourse.tile as tile
    from concourse import bacc

    DT = mybir.dt.float32
    DTM = mybir.dt.bfloat16

    AF = mybir.ActivationFunctionType

    nc = bacc.Bacc("TRN2", debug=False, num_devices=N_CORES)

    xT = nc.dram_tensor("xT", [HIDDEN, N], DTM, kind="ExternalInput").ap()
    wq = nc.dram_tensor("wq", [HIDDEN, AD], DTM, kind="ExternalInput").ap()
    wk = nc.dram_tensor("wk", [HIDDEN, AD], DTM, kind="ExternalInput").ap()
    wv = nc.dram_tensor("wv", [HIDDEN, AD], DTM, kind="ExternalInput").ap()
    bq = nc.dram_tensor("bq", [AD, 1], DT, kind="ExternalInput").ap()
    bk = nc.dram_tensor("bk", [AD, 1], DT, kind="ExternalInput").ap()
    bvT = nc.dram_tensor("bvT", [1, AD], DT, kind="ExternalInput").ap()
    wo = nc.dram_tensor("wo", [HIDDEN, HIDDEN], DTM, kind="ExternalInput").ap()
    bo = nc.dram_tensor("bo", [1, HIDDEN], DT, kind="ExternalInput").ap()
    out = nc.dram_tensor("out", [NSLICE, HIDDEN], DT, kind="ExternalOutput").ap()

    with tile.TileContext(nc) as tc:
        with (
            tc.tile_pool(name="sb", bufs=1) as sb,
            tc.tile_pool(name="ps", bufs=2, space="PSUM") as ps,
            tc.tile_pool(name="dram", bufs=1, space="DRAM") as dram,
        ):
            # Global reordering: the sequence axis n is processed in
            # "stripe" order n' = (m, j, t) <-> n = 512*j + 128*m + t
            # (m: stripe 0..3, j: destination core 0..7, t: 0..127).
            # Attention is permutation-invariant in the key axis as long as
            # k and v use the same order, and the q axis just needs the
            # inverse map applied at output -- which the AllToAll block
            # routing does implicitly. Stripe m's attention output IS the
            # m-th out-row-tile of every core, so each stripe's AllToAll +
            # out-projection pipeline behind the next stripe's attention.

            bvT_sb = sb.tile([1, AD], DT)
            # qkv weights: one DMA each, [1024, 128] folded to [128, 8*128]
            wq_sb = sb.tile([128, HT * AD], DTM)
            wk_sb = sb.tile([128, HT * AD], DTM)
            wv_sb = sb.tile([128, HT * AD], DTM)
            bq_sb = sb.tile([AD, 1], DT)
            bk_sb = sb.tile([AD, 1], DT)
            bv_bc = sb.tile([128, AD], DT)

            def emit_weight_loads():
                for w_sb, wsrc in ((wq_sb, wq), (wk_sb, wk), (wv_sb, wv)):
                    nc.sync.dma_start(
                        w_sb[:].rearrange("p (a c) -> p a c", a=HT),
                        wsrc.rearrange("(a p) c -> p a c", p=128),
                    )
                nc.sync.dma_start(bq_sb[:], bq[:])
                nc.sync.dma_start(bk_sb[:], bk[:])
                nc.sync.dma_start(bvT_sb[:], bvT[:])
                nc.gpsimd.partition_broadcast(bv_bc[:], bvT_sb[:1, :])

            def wslice(w_sb, i):
                return w_sb[:, i * AD : (i + 1) * AD]

            # Host pre-permutes x columns into stripe order n' = (m, j, t),
            # so streaming, qT, kTc, v_nat are all plain contiguous in n'.
            qT = sb.tile([AD, N], DTM)
            kTc = [sb.tile([AD, 512], DTM, name=f"kTc{c}", tag="kTc", bufs=HT) for c in range(HT)]
            att_m = [sb.tile([AD, QCHUNK], DTM, name=f"attm{m}", tag="attm", bufs=NQC) for m in range(NQC)]
            # v in natural [keys, dims] layout: per chunk [128, (j, h, D+1)],
            # ones column at slot D of each head for the softmax denominator.
            v_nat = [
                sb.tile([128, 4 * HPC * (D + 1)], DTM, name=f"vn{c}", tag="vnat", bufs=HT)
                for c in range(HT)
            ]
            wo_sb = [sb.tile([128, HIDDEN], DTM, name=f"wo{i}", tag="wo", bufs=HT) for i in range(HT)]
            bo_bc = sb.tile([128, HIDDEN], DT)

            a2a_in = [
                dram.tile([N_CORES, AD, 128], DTM, name=f"a2ai{m}", tag="a2ai", bufs=NQC)
                for m in range(NQC)
            ]
            a2a_out = [
                dram.tile([N_CORES, AD, 128], DTM, name=f"a2ao{m}", tag="a2ao", bufs=NQC)
                for m in range(NQC)
            ]

            def vn_h(c, j, h):
                """[128 keys, D+1] slice of chunk c's v for k-tile j, head h."""
                base = (j * HPC + h) * (D + 1)
                return v_nat[c][:, base : base + D + 1]

            # Attention runs with AV matmuls lagging LAG units behind their
            # scores+exp (p_sb holds LAG+1 tiles): the ACT engine keeps a
            # LAG-deep backlog of materialized exps, so projection blocks
            # occupying the in-order PE stream no longer starve it.
            LAG = 9

            # ---- emission helpers --------------------------------------
            # All non-attention PE work is emitted as small "pieces" (2-4
            # matmuls, ~0.4-0.9us) with DVE partial accumulation, woven
            # between attention (scores+exp+AV) pairs. A long uninterrupted
            # matmul block would stall the in-order PE stream past the ~2
            # tiles of exp backlog the s_ps double-buffer can hold, idling
            # the ACT engine (the overall bottleneck) by its own duration.
            # Pieces are always injected in PAIRS so the number of ps_big
            # allocations between consecutive s_ps allocations stays even
            # and s_ps keeps alternating between its two buffers.
            xts = []

            def emit_xt(cp, eng=None):
                """x chunk load via the (otherwise idle) Pool queue so the
                SP queue's weight DMAs never delay it; 8 bufs = fully
                prefetched, no reuse dependency between chunks (they also
                stay resident for the deferred q projections)."""
                cs = slice(cp * 512, (cp + 1) * 512)
                xt = sb.tile([128, HT * 512], DTM, name="xt", tag="xt", bufs=HT)
                xts.append(xt)
                (eng or nc.gpsimd).dma_start(
                    xt[:].rearrange("p (a t) -> p a t", a=HT),
                    xT[:, cs].rearrange("(a p) t -> p a t", p=128),
                )

            def qk_pieces(cp, w_sb, b_sb, dst, npiece, tag="ps_big"):
                """q or k projection for chunk cp as npiece thunks."""
                per = HT // npiece
                thunks = []
                for pc in range(npiece):
                    def piece(pc=pc):
                        pp = ps.tile([128, 512], DT, name="pp", tag=tag)
                        for i in range(per * pc, per * (pc + 1)):
                            nc.tensor.matmul(
                                pp[:AD, :], wslice(w_sb, i),
                                xts[cp][:, i * 512 : (i + 1) * 512],
                                start=(i == per * pc), stop=(i == per * (pc + 1) - 1),
                            )
                        if pc == 0:
                            nc.vector.tensor_scalar_add(dst, pp[:AD, :], b_sb[:])
                        else:
                            nc.vector.tensor_add(dst, dst, pp[:AD, :])
                    thunks.append(piece)
                return thunks

            def k_pieces(cp):
                return qk_pieces(cp, wk_sb, bk_sb, kTc[cp][:], 2)

            def q_pieces(cp, npiece=2, tag="ps_big"):
                cs = slice(cp * 512, (cp + 1) * 512)
                return qk_pieces(cp, wq_sb, bq_sb, qT[:, cs], npiece, tag)

            def v_pieces(cp):
                """v directly in [keys, dims] layout: x-chunk tile as the
                stationary operand, wv moving; out partitions are the 128
                keys of k-tile j. Two thunks of two k-tiles each."""
                thunks = []
                for half in (0, 1):
                    def piece(half=half):
                        pv = ps.tile([128, 256], DT, name="pv", tag="ps_big")
                        for jj in (0, 1):
                            j = 2 * half + jj
                            for i in range(HT):
                                nc.tensor.matmul(
                                    pv[:, jj * 128 : (jj + 1) * 128],
                                    xts[cp][:, i * 512 + j * 128 : i * 512 + (j + 1) * 128],
                                    wslice(wv_sb, i),
                                    start=(i == 0), stop=(i == HT - 1),
                                )
                        vn4 = v_nat[cp][:].rearrange("p (j h x) -> p j h x", j=4, x=D + 1)
                        for jj in (0, 1):
                            j = 2 * half + jj
                            nc.vector.tensor_add(
                                vn4[:, j, :, :D],
                                pv[:, jj * 128 : (jj + 1) * 128].rearrange(
                                    "p (h d) -> p h d", h=HPC
                                ),
                                bv_bc[:].rearrange("p (h d) -> p h d", h=HPC),
                            )
                        nc.vector.memset(
                            vn4[:, 2 * half : 2 * half + 2, :, D : D + 1], 1.0
                        )
                    thunks.append(piece)
                return thunks

            def emit_scores_exp(m, kt_i, h):
                hs = slice(h * D, (h + 1) * D)
                s_ps = ps.tile([128, QCHUNK], DT, name="s_ps", tag="ps_big")
                for half in range(2):
                    nc.tensor.matmul(
                        s_ps[:, half * 512 : (half + 1) * 512],
                        kTc[kt_i // 4][hs, (kt_i % 4) * 128 : (kt_i % 4 + 1) * 128],
                        qT[hs, m * QCHUNK + half * 512 : m * QCHUNK + (half + 1) * 512],
                        start=True, stop=True,
                    )
                p_sb = sb.tile([128, QCHUNK], DTM, name="p_sb", tag="p_sb", bufs=LAG + 1)
                nc.scalar.activation(p_sb[:], s_ps[:], AF.Exp, scale=0.125)
                return p_sb

            def emit_av(kt_i, h, accs, p_sb):
                for half in range(2):
                    hsl = slice(half * 512, (half + 1) * 512)
                    nc.tensor.matmul(
                        accs[h][: D + 1, hsl],
                        vn_h(kt_i // 4, kt_i % 4, h),
                        p_sb[:, hsl],
                        start=(kt_i == 0), stop=(kt_i == att_nt - 1),
                    )

            def emit_att_kt(m, kt_i, h, accs):
                emit_av(kt_i, h, accs, emit_scores_exp(m, kt_i, h))

            def emit_finish_stripe(m, accs):
                # copy both accumulators out of PSUM first so their banks
                # free for the next stripe while normalization runs on SBUF
                acc_sbs = []
                for h in range(HPC):
                    acc_sb = sb.tile([D + 1, QCHUNK], DTM, name="acc_sb", tag="acc_sb", bufs=2)
                    nc.vector.tensor_copy(acc_sb[:], accs[h][: D + 1, :])
                    acc_sbs.append(acc_sb)
                for h in range(HPC):
                    hs = slice(h * D, (h + 1) * D)
                    acc_sb = acc_sbs[h]
                    recip = sb.tile([1, QCHUNK], DTM, name="recip", tag="recip", bufs=2)
                    with nc.allow_low_precision(reason="softmax denom in bf16; tol 2e-2"):
                        nc.vector.reciprocal(recip[:], acc_sb[D : D + 1, :])
                    bcast = sb.tile([D, QCHUNK], DTM, name="bcast", tag="bcast", bufs=2)
                    nc.gpsimd.partition_broadcast(bcast[:], recip[:1, :])
                    nc.vector.tensor_mul(att_m[m][hs, :], acc_sb[:D, :], bcast[:])
                nc.sync.dma_start(
                    a2a_in[m][:].rearrange("a p t -> p a t"),
                    att_m[m][:].rearrange("p (a t) -> p a t", a=N_CORES),
                )
                if not skip_a2a:
                    nc.gpsimd.collective_compute(
                        "AllToAll",
                        mybir.AluOpType.bypass,
                        replica_groups=[list(range(N_CORES))],
                        ins=[a2a_in[m].opt()],
                        outs=[a2a_out[m].opt()],
                    )

            def emit_aTm_load(m):
                aTm = sb.tile([128, N_CORES * 128], DTM, name="aTm", tag="aTm", bufs=2)
                nc.sync.dma_start(
                    aTm[:].rearrange("p (a t) -> p a t", a=N_CORES),
                    a2a_out[m][:].rearrange("a p t -> p a t"),
                )
                return aTm

            def outproj_pieces(m, aTm_ref, cc, npiece=4, tag="ps_big"):
                """output projection for stripe m, 512-column half cc, as
                npiece thunks accumulating into an SBUF tile via DVE."""
                os_ = slice(cc * 512, (cc + 1) * 512)
                per = HT // npiece
                holder = []
                thunks = []
                for pc in range(npiece):
                    def piece(pc=pc):
                        po = ps.tile([128, 512], DT, name="po", tag=tag)
                        for i in range(per * pc, per * (pc + 1)):
                            nc.tensor.matmul(
                                po[:], aTm_ref[0][:, i * 128 : (i + 1) * 128],
                                wo_sb[i][:, os_],
                                start=(i == per * pc), stop=(i == per * (pc + 1) - 1),
                            )
                        if pc == 0:
                            out_sb = sb.tile([128, 512], DT, name="out_sb", tag="out_sb", bufs=2)
                            holder.append(out_sb)
                            nc.vector.tensor_add(out_sb[:], po[:], bo_bc[:, os_])
                        else:
                            out_sb = holder[0]
                            nc.vector.tensor_add(out_sb[:], out_sb[:], po[:])
                        if pc == npiece - 1:
                            nc.sync.dma_start(out[m * 128 : (m + 1) * 128, os_], out_sb[:])
                    thunks.append(piece)
                return thunks

            # ---- schedule ----------------------------------------------
            def new_accs():
                return [
                    ps.tile([128, QCHUNK], DT, name=f"acc{h}", tag="ps_acc")
                    for h in range(HPC)
                ]

            # phase 1: stream chunks with stripe-0 attention interleaved at
            # lag 1 (chunk cp delivers k-tiles 4cp..4cp+3; attention trails
            # one chunk behind so exp work reaches ACT as early as possible).
            # Phase 1 is PE/supply-bound, so projection blocks sit between
            # attention groups without extra cost.
            # chunk-1's q runs before chunk-0's v so the first scores+exp
            # fire as early as possible
            emit_weight_loads()
            emit_xt(0)
            emit_xt(1)
            for t in k_pieces(0) + q_pieces(0) + q_pieces(1) + v_pieces(0):
                t()
            # out-proj weights load early on the SP queue (x loads are on
            # Pool, so these only queue behind qkv weights)
            for i in range(HT):
                nc.sync.dma_start(wo_sb[i][:], wo[i * 128 : (i + 1) * 128, :])
            bo_sb = sb.tile([1, HIDDEN], DT)
            nc.sync.dma_start(bo_sb[:], bo[:])
            nc.gpsimd.partition_broadcast(bo_bc[:], bo_sb[:1, :])

            accs = new_accs()
            stash0 = []

            def unit0(kt_i, h):
                stash0.append((kt_i, h, emit_scores_exp(0, kt_i, h)))
                if len(stash0) > LAG:
                    pk, ph, pp = stash0.pop(0)
                    emit_av(pk, ph, accs, pp)

            for cp in range(1, HT):
                if cp > 1:
                    emit_xt(cp)
                    # only stripe 1's q (chunks 2-3) projects in phase 1;
                    # stripes 2-3's q rides the later stripe boundaries where
                    # the LAG exp backlog absorbs it, shrinking the per-chunk
                    # block that starves ACT here
                    pieces = k_pieces(cp) + (q_pieces(cp) if cp < 4 else []) + v_pieces(cp)
                else:
                    pieces = k_pieces(cp) + v_pieces(cp)
                a = 4 * (cp - 1)
                for kk in range(4):
                    for h in range(HPC):
                        unit0(a + kk, h)
                    # one consolidated projection block per chunk: each
                    # injection site costs a fixed pipeline restart, so
                    # fewer sites beat evenly-spread pieces
                    if kk == 1:
                        for t in pieces:
                            t()
            for kt_i in range(4 * (HT - 1), att_nt):
                for h in range(HPC):
                    unit0(kt_i, h)
            for pk, ph, pp in stash0:
                emit_av(pk, ph, accs, pp)
            emit_finish_stripe(0, accs)

            # Stripe k's out-projection runs at the START boundary of stripe
            # k+2: its AllToAll is a full stripe old (no collective wait) and
            # the po blocks allocate from the just-freed ps_acc buffers, so
            # the exp stream's s_ps rotation is never interrupted mid-stripe.
            # The new stripe's first AV matmuls lag behind the po blocks but
            # the p_sb triple-buffer absorbs that.
            aTms = {}
            for m in range(1, NQC):
                units = [(kt, h) for kt in range(att_nt) for h in range(HPC)]
                # boundary block: previous-previous stripe's out-projection
                # plus the q projections for stripe m+1, all allocated from
                # the just-freed ps_acc buffers so the s_ps rotation is
                # untouched
                blocks = []
                if m >= 2:
                    for cc in range(2):
                        blocks += outproj_pieces(m - 2, [aTms[m - 2]], cc, npiece=1, tag="ps_acc")
                if m < NQC - 1:
                    blocks += q_pieces(2 * (m + 1), tag="ps_acc")
                    blocks += q_pieces(2 * (m + 1) + 1, tag="ps_acc")
                # pre-emit LAG scores+exp units so ACT stays fed while the
                # boundary blocks occupy the in-order PE stream; the whole
                # stripe then runs with AV matmuls lagging LAG units behind
                # their scores, so the deferred AVs interleave with new
                # scores instead of bunching up after the blocks
                stash = [(kt, h, emit_scores_exp(m, kt, h)) for kt, h in units[:LAG]]
                for t in blocks:
                    t()
                accs = new_accs()
                for ui, (kt_i, h) in enumerate(units[LAG:]):
                    if kt_i == 24 and h == 0:
                        aTms[m - 1] = emit_aTm_load(m - 1)
                    p_sb = emit_scores_exp(m, kt_i, h)
                    pk, ph, pp = stash.pop(0)
                    emit_av(pk, ph, accs, pp)
                    # last stripe: no more blocks need the backlog, so drain
                    # the lag early (one extra AV every other unit keeps PE
                    # and ACT balanced) instead of flushing it as pure tail
                    if m == NQC - 1 and ui % 2 == 0 and len(stash) > 1:
                        pk, ph, pp = stash.pop(0)
                        emit_av(pk, ph, accs, pp)
                    stash.append((kt_i, h, p_sb))
                for pk, ph, pp in stash:
                    emit_av(pk, ph, accs, pp)
                emit_finish_stripe(m, accs)
            # tail: stripe 2's projection hides under stripe 3's AllToAll
            for cc in range(2):
                for t in outproj_pieces(NQC - 2, [aTms[NQC - 2]], cc, npiece=1, tag="ps_acc"):
                    t()
            aTm3 = emit_aTm_load(NQC - 1)
            for cc in range(2):
                for t in outproj_pieces(NQC - 1, [aTm3], cc, npiece=1, tag="ps_acc"):
                    t()

    nc.compile()
    return nc


def _get_nc(mm_mode: str):
    if mm_mode not in _CACHE:
        _CACHE[mm_mode] = _build(mm_mode)
    return _CACHE[mm_mode]


def make_in_maps(x, w_qkv, b_qkv, w_out, b_out):
    import ml_dtypes

    bf16 = ml_dtypes.bfloat16
    x = np.asarray(x, dtype=np.float32)
    w_qkv = np.asarray(w_qkv, dtype=np.float32)
    b_qkv = np.asarray(b_qkv, dtype=np.float32)
    w_out = np.asarray(w_out, dtype=np.float32)
    b_out = np.asarray(b_out, dtype=np.float32)

    xT = x.reshape(N, HIDDEN).T  # [hidden, n]
    # permute n into stripe order n' = (m, j, t) <-> n = 512*j + 128*m + t
    xT = np.ascontiguousarray(
        xT.reshape(HIDDEN, N_CORES, NQC, 128).transpose(0, 2, 1, 3).reshape(HIDDEN, N)
    ).astype(bf16)
    w_out_bf = np.ascontiguousarray(w_out).astype(bf16)
    bo = np.ascontiguousarray(b_out.reshape(1, HIDDEN))
    in_maps = []
    for c in range(N_CORES):
        cs = slice(c * AD, (c + 1) * AD)
        in_maps.append(
            {
                "xT": xT,
                "wq": np.ascontiguousarray(w_qkv[:, :HIDDEN][:, cs]).astype(bf16),
                "wk": np.ascontiguousarray(w_qkv[:, HIDDEN : 2 * HIDDEN][:, cs]).astype(bf16),
                "wv": np.ascontiguousarray(w_qkv[:, 2 * HIDDEN :][:, cs]).astype(bf16),
                "bq": np.ascontiguousarray(b_qkv[:HIDDEN][cs].reshape(AD, 1)),
                "bk": np.ascontiguousarray(b_qkv[HIDDEN : 2 * HIDDEN][cs].reshape(AD, 1)),
                "bvT": np.ascontiguousarray(b_qkv[2 * HIDDEN :][cs].reshape(1, AD)),
                "wo": w_out_bf,
                "bo": bo,
            }
        )
    return in_maps


def kernel(x, w_qkv, b_qkv, w_out, b_out):
    from concourse.bass_utils import run_bass_kernel_spmd

    mm_mode = os.environ.get("TRN_MM_MODE", "bf16")
    nc = _get_nc(mm_mode)
    in_maps = make_in_maps(x, w_qkv, b_qkv, w_out, b_out)
    res = run_bass_kernel_spmd(nc, in_maps, list(range(N_CORES)))
    full = np.concatenate([res.results[c]["out"] for c in range(N_CORES)], axis=0)
    return full.reshape(1, N, HIDDEN).astype(np.float32)



# revision 7
# speedup vs baseline: 1.9745x; 1.9745x over previous
"""Trainium2 Bass kernel for a 16-head dense attention layer (v2, bf16).

Problem: x[1,4096,1024] @ w_qkv[1024,3072] -> 16-head attention (N=4096,
D=64) -> @ w_out[1024,1024].

Sharding: tensor-parallel over heads across 8 NeuronCores (2 heads/core).
Each core computes q/k/v for its 2 heads (weights column-sliced on host),
attention with a fused, max-free softmax (scores are bounded so exp never
overflows in fp32; denominator comes from an appended ones-column in V),
then an AllToAll converts the head-sharded attention output into a
sequence-sharded layout so every core applies the full output projection
to its own 512 rows. Host concatenates the 8 row slices.

vs the f32r baseline: all matmul operands bf16 (PSUM accumulation stays
fp32; rel-err budget 2e-2), V projected directly in [keys, dims] layout
(no PE transposes), merged x DMAs prefetched on the Pool queue, lag-1
attention interleave in phase 1, one AllToAll per stripe, and attention
software-pipelined with AV matmuls lagging LAG=9 units behind their
scores+exp: the ACT engine's exp backlog rides through the consolidated
stripe-boundary blocks (older stripes' output projection plus the
next-next stripe's q projection, allocated from just-freed ps_acc
buffers) without starving, and the last stripe drains its lag early so
it does not flush as pure tail.
"""

import os
import numpy as np

N_CORES = 8
N = 4096
HIDDEN = 1024
D = 64
HPC = 2  # heads per core
AD = HPC * D  # 128 att-dim rows per core
NT = N // 128  # 32 k-tiles of 128
HT = HIDDEN // 128  # 8 hidden tiles
QCHUNK = 1024
NQC = N // QCHUNK  # 4 q-chunks (stripes)
NSLICE = N // N_CORES  # 512 rows of output per core

_CACHE = {}


def _build(mm_mode: str = "bf16", skip_a2a: bool = False, att_nt: int = NT, repeat: int = 1):
    import concourse.bass as bass
    import concourse.mybir as mybir
    import concourse.tile as tile
    from concourse import bacc

    DT = mybir.dt.float32
    DTM = mybir.dt.bfloat16

    AF = mybir.ActivationFunctionType

    nc = bacc.Bacc("TRN2", debug=False, num_devices=N_CORES)

    xT = nc.dram_tensor("xT", [HIDDEN, N], DTM, kind="ExternalInput").ap()
    wq = nc.dram_tensor("wq", [HIDDEN, AD], DTM, kind="ExternalInput").ap()
    wk = nc.dram_tensor("wk", [HIDDEN, AD], DTM, kind="ExternalInput").ap()
    wv = nc.dram_tensor("wv", [HIDDEN, AD], DTM, kind="ExternalInput").ap()
    bq = nc.dram_tensor("bq", [AD, 1], DT, kind="ExternalInput").ap()
    bk = nc.dram_tensor("bk", [AD, 1], DT, kind="ExternalInput").ap()
    bvT = nc.dram_tensor("bvT", [1, AD], DT, kind="ExternalInput").ap()
    wo = nc.dram_tensor("wo", [HIDDEN, HIDDEN], DTM, kind="ExternalInput").ap()
    bo = nc.dram_tensor("bo", [1, HIDDEN], DT, kind="ExternalInput").ap()
    out = nc.dram_tensor("out", [NSLICE, HIDDEN], DT, kind="ExternalOutput").ap()

    with tile.TileContext(nc) as tc:
        with (
            tc.tile_pool(name="sb", bufs=1) as sb,
            tc.tile_pool(name="ps", bufs=2, space="PSUM") as ps,
            tc.tile_pool(name="dram", bufs=1, space="DRAM") as dram,
        ):
            # Global reordering: the sequence axis n is processed in
            # "stripe" order n' = (m, j, t) <-> n = 512*j + 128*m + t
            # (m: stripe 0..3, j: destination core 0..7, t: 0..127).
            # Attention is permutation-invariant in the key axis as long as
            # k and v use the same order, and the q axis just needs the
            # inverse map applied at output -- which the AllToAll block
            # routing does implicitly. Stripe m's attention output IS the
            # m-th out-row-tile of every core, so each stripe's AllToAll +
            # out-projection pipeline behind the next stripe's attention.

            # repeat>1 replicates the whole body inside one NEFF for
            # dispatch-amortized timing; kernel() always uses repeat=1.
            for _rep in range(repeat):
                bvT_sb = sb.tile([1, AD], DT)
                # qkv weights: one DMA each, [1024, 128] folded to [128, 8*128]
                wq_sb = sb.tile([128, HT * AD], DTM)
                wk_sb = sb.tile([128, HT * AD], DTM)
                wv_sb = sb.tile([128, HT * AD], DTM)
                bq_sb = sb.tile([AD, 1], DT)
                bk_sb = sb.tile([AD, 1], DT)
                bv_bc = sb.tile([128, AD], DT)

                def emit_weight_loads():
                    for w_sb, wsrc in ((wq_sb, wq), (wk_sb, wk), (wv_sb, wv)):
                        nc.sync.dma_start(
                            w_sb[:].rearrange("p (a c) -> p a c", a=HT),
                            wsrc.rearrange("(a p) c -> p a c", p=128),
                        )
                    nc.sync.dma_start(bq_sb[:], bq[:])
                    nc.sync.dma_start(bk_sb[:], bk[:])
                    nc.sync.dma_start(bvT_sb[:], bvT[:])
                    nc.gpsimd.partition_broadcast(bv_bc[:], bvT_sb[:1, :])

                def wslice(w_sb, i):
                    return w_sb[:, i * AD : (i + 1) * AD]

                # Host pre-permutes x columns into stripe order n' = (m, j, t),
                # so streaming, qT, kTc, v_nat are all plain contiguous in n'.
                qT = sb.tile([AD, N], DTM)
                kTc = [sb.tile([AD, 512], DTM, name=f"kTc{c}", tag="kTc", bufs=HT) for c in range(HT)]
                att_m = [sb.tile([AD, QCHUNK], DTM, name=f"attm{m}", tag="attm", bufs=NQC) for m in range(NQC)]
                # v in natural [keys, dims] layout: per chunk [128, (j, h, D+1)],
                # ones column at slot D of each head for the softmax denominator.
                v_nat = [
                    sb.tile([128, 4 * HPC * (D + 1)], DTM, name=f"vn{c}", tag="vnat", bufs=HT)
                    for c in range(HT)
                ]
                wo_sb = [sb.tile([128, HIDDEN], DTM, name=f"wo{i}", tag="wo", bufs=HT) for i in range(HT)]
                bo_bc = sb.tile([128, HIDDEN], DT)

                a2a_in = [
                    dram.tile([N_CORES, AD, 128], DTM, name=f"a2ai{m}", tag="a2ai", bufs=NQC)
                    for m in range(NQC)
                ]
                a2a_out = [
                    dram.tile([N_CORES, AD, 128], DTM, name=f"a2ao{m}", tag="a2ao", bufs=NQC)
                    for m in range(NQC)
                ]

                def vn_h(c, j, h):
                    """[128 keys, D+1] slice of chunk c's v for k-tile j, head h."""
                    base = (j * HPC + h) * (D + 1)
                    return v_nat[c][:, base : base + D + 1]

                # Attention runs with AV matmuls lagging 2*PRE_KT (kt,h) units
                # behind their scores+exp: the ACT engine keeps that deep a
                # backlog of materialized exps, so projection blocks occupying
                # the in-order PE stream no longer starve it.
                PRE_KT = 5

                # ---- emission helpers --------------------------------------
                # All non-attention PE work is emitted as small "pieces" (2-4
                # matmuls, ~0.4-0.9us) with DVE partial accumulation, woven
                # between attention (scores+exp+AV) pairs. A long uninterrupted
                # matmul block would stall the in-order PE stream past the ~2
                # tiles of exp backlog the s_ps double-buffer can hold, idling
                # the ACT engine (the overall bottleneck) by its own duration.
                # Pieces are always injected in PAIRS so the number of ps_big
                # allocations between consecutive s_ps allocations stays even
                # and s_ps keeps alternating between its two buffers.
                xts = []

                def emit_xt(cp, eng=None):
                    """x chunk load via the (otherwise idle) Pool queue so the
                    SP queue's weight DMAs never delay it; 8 bufs = fully
                    prefetched, no reuse dependency between chunks (they also
                    stay resident for the deferred q projections)."""
                    cs = slice(cp * 512, (cp + 1) * 512)
                    xt = sb.tile([128, HT * 512], DTM, name="xt", tag="xt", bufs=HT)
                    xts.append(xt)
                    (eng or nc.gpsimd).dma_start(
                        xt[:].rearrange("p (a t) -> p a t", a=HT),
                        xT[:, cs].rearrange("(a p) t -> p a t", p=128),
                    )

                def qk_pieces(cp, w_sb, b_sb, dst, npiece, tag="ps_big"):
                    """q or k projection for chunk cp as npiece thunks."""
                    per = HT // npiece
                    thunks = []
                    for pc in range(npiece):
                        def piece(pc=pc):
                            pp = ps.tile([128, 512], DT, name="pp", tag=tag)
                            for i in range(per * pc, per * (pc + 1)):
                                nc.tensor.matmul(
                                    pp[:AD, :], wslice(w_sb, i),
                                    xts[cp][:, i * 512 : (i + 1) * 512],
                                    start=(i == per * pc), stop=(i == per * (pc + 1) - 1),
                                )
                            if pc == 0:
                                nc.vector.tensor_scalar_add(dst, pp[:AD, :], b_sb[:])
                            else:
                                nc.vector.tensor_add(dst, dst, pp[:AD, :])
                        thunks.append(piece)
                    return thunks

                def k_pieces(cp):
                    return qk_pieces(cp, wk_sb, bk_sb, kTc[cp][:], 2)

                def q_pieces(cp, npiece=2, tag="ps_big"):
                    cs = slice(cp * 512, (cp + 1) * 512)
                    return qk_pieces(cp, wq_sb, bq_sb, qT[:, cs], npiece, tag)

                def v_pieces(cp):
                    """v directly in [keys, dims] layout: x-chunk tile as the
                    stationary operand, wv moving; out partitions are the 128
                    keys of k-tile j. Two thunks of two k-tiles each."""
                    thunks = []
                    for half in (0, 1):
                        def piece(half=half):
                            pv = ps.tile([128, 256], DT, name="pv", tag="ps_big")
                            for jj in (0, 1):
                                j = 2 * half + jj
                                for i in range(HT):
                                    nc.tensor.matmul(
                                        pv[:, jj * 128 : (jj + 1) * 128],
                                        xts[cp][:, i * 512 + j * 128 : i * 512 + (j + 1) * 128],
                                        wslice(wv_sb, i),
                                        start=(i == 0), stop=(i == HT - 1),
                                    )
                            vn4 = v_nat[cp][:].rearrange("p (j h x) -> p j h x", j=4, x=D + 1)
                            for jj in (0, 1):
                                j = 2 * half + jj
                                nc.vector.tensor_add(
                                    vn4[:, j, :, :D],
                                    pv[:, jj * 128 : (jj + 1) * 128].rearrange(
                                        "p (h d) -> p h d", h=HPC
                                    ),
                                    bv_bc[:].rearrange("p (h d) -> p h d", h=HPC),
                                )
                            nc.vector.memset(
                                vn4[:, 2 * half : 2 * half + 2, :, D : D + 1], 1.0
                            )
                        thunks.append(piece)
                    return thunks

                def emit_scores_exp2(m, kt_i):
                    """Scores + exp for BOTH heads of k-tile kt_i, with the
                    score matmuls interleaved h0/h1: the two heads' K=64
                    contractions sit on row-groups 0-1 (partitions 0-63) and
                    2-3 (64-127), so adjacent MMs on different row groups run
                    CONCURRENTLY on the PE sub-arrays (tile_position is
                    auto-derived from the APs' base partitions) -- ~2x the
                    score throughput vs serial emission. Order h0a,h1a,h1b,h0b
                    keeps h1's stationary loaded for its second half (one
                    fewer LDWEIGHTS) while preserving pairwise overlap."""
                    s_list = [
                        ps.tile([128, QCHUNK], DT, name=f"s_ps{h}", tag="ps_big")
                        for h in range(HPC)
                    ]

                    def mm(h, half):
                        hs = slice(h * D, (h + 1) * D)
                        nc.tensor.matmul(
                            s_list[h][:, half * 512 : (half + 1) * 512],
                            kTc[kt_i // 4][hs, (kt_i % 4) * 128 : (kt_i % 4 + 1) * 128],
                            qT[hs, m * QCHUNK + half * 512 : m * QCHUNK + (half + 1) * 512],
                            start=True, stop=True,
                        )

                    mm(0, 0)
                    mm(1, 0)
                    mm(1, 1)
                    mm(0, 1)
                    p_out = []
                    for h in range(HPC):
                        p_sb = sb.tile([128, QCHUNK], DTM, name="p_sb", tag="p_sb", bufs=2 * PRE_KT + 2)
                        nc.scalar.activation(p_sb[:], s_list[h][:], AF.Exp, scale=0.125)
                        p_out.append(p_sb)
                    return p_out

                def emit_av(kt_i, h, accs, p_sb):
                    for half in range(2):
                        hsl = slice(half * 512, (half + 1) * 512)
                        nc.tensor.matmul(
                            accs[h][: D + 1, hsl],
                            vn_h(kt_i // 4, kt_i % 4, h),
                            p_sb[:, hsl],
                            start=(kt_i == 0), stop=(kt_i == att_nt - 1),
                        )

                def emit_finish_stripe(m, accs):
                    # copy both accumulators out of PSUM first so their banks
                    # free for the next stripe while normalization runs on SBUF
                    acc_sbs = []
                    for h in range(HPC):
                        acc_sb = sb.tile([D + 1, QCHUNK], DTM, name="acc_sb", tag="acc_sb", bufs=2)
                        nc.vector.tensor_copy(acc_sb[:], accs[h][: D + 1, :])
                        acc_sbs.append(acc_sb)
                    for h in range(HPC):
                        hs = slice(h * D, (h + 1) * D)
                        acc_sb = acc_sbs[h]
                        recip = sb.tile([1, QCHUNK], DTM, name="recip", tag="recip", bufs=2)
                        with nc.allow_low_precision(reason="softmax denom in bf16; tol 2e-2"):
                            nc.vector.reciprocal(recip[:], acc_sb[D : D + 1, :])
                        bcast = sb.tile([D, QCHUNK], DTM, name="bcast", tag="bcast", bufs=2)
                        nc.gpsimd.partition_broadcast(bcast[:], recip[:1, :])
                        nc.vector.tensor_mul(att_m[m][hs, :], acc_sb[:D, :], bcast[:])
                    nc.sync.dma_start(
                        a2a_in[m][:].rearrange("a p t -> p a t"),
                        att_m[m][:].rearrange("p (a t) -> p a t", a=N_CORES),
                    )
                    if not skip_a2a:
                        nc.gpsimd.collective_compute(
                            "AllToAll",
                            mybir.AluOpType.bypass,
                            replica_groups=[list(range(N_CORES))],
                            ins=[a2a_in[m].opt()],
                            outs=[a2a_out[m].opt()],
                        )

                def emit_aTm_load(m):
                    aTm = sb.tile([128, N_CORES * 128], DTM, name="aTm", tag="aTm", bufs=2)
                    nc.sync.dma_start(
                        aTm[:].rearrange("p (a t) -> p a t", a=N_CORES),
                        a2a_out[m][:].rearrange("a p t -> p a t"),
                    )
                    return aTm

                def outproj_pieces(m, aTm_ref, cc, npiece=4, tag="ps_big"):
                    """output projection for stripe m, 512-column half cc, as
                    npiece thunks accumulating into an SBUF tile via DVE."""
                    os_ = slice(cc * 512, (cc + 1) * 512)
                    per = HT // npiece
                    holder = []
                    thunks = []
                    for pc in range(npiece):
                        def piece(pc=pc):
                            po = ps.tile([128, 512], DT, name="po", tag=tag)
                            for i in range(per * pc, per * (pc + 1)):
                                nc.tensor.matmul(
                                    po[:], aTm_ref[0][:, i * 128 : (i + 1) * 128],
                                    wo_sb[i][:, os_],
                                    start=(i == per * pc), stop=(i == per * (pc + 1) - 1),
                                )
                            if pc == 0:
                                out_sb = sb.tile([128, 512], DT, name="out_sb", tag="out_sb", bufs=2)
                                holder.append(out_sb)
                                nc.vector.tensor_add(out_sb[:], po[:], bo_bc[:, os_])
                            else:
                                out_sb = holder[0]
                                nc.vector.tensor_add(out_sb[:], out_sb[:], po[:])
                            if pc == npiece - 1:
                                nc.sync.dma_start(out[m * 128 : (m + 1) * 128, os_], out_sb[:])
                        thunks.append(piece)
                    return thunks

                # ---- schedule ----------------------------------------------
                def new_accs():
                    return [
                        ps.tile([128, QCHUNK], DT, name=f"acc{h}", tag="ps_acc")
                        for h in range(HPC)
                    ]

                # phase 1: stream chunks with stripe-0 attention interleaved at
                # lag 1 (chunk cp delivers k-tiles 4cp..4cp+3; attention trails
                # one chunk behind so exp work reaches ACT as early as possible).
                # Phase 1 is PE/supply-bound, so projection blocks sit between
                # attention groups without extra cost.
                # chunk-1's q runs before chunk-0's v so the first scores+exp
                # fire as early as possible
                emit_weight_loads()
                emit_xt(0)
                emit_xt(1)
                for t in k_pieces(0) + q_pieces(0) + q_pieces(1) + v_pieces(0):
                    t()
                # out-proj weights load early on the SP queue (x loads are on
                # Pool, so these only queue behind qkv weights)
                for i in range(HT):
                    nc.sync.dma_start(wo_sb[i][:], wo[i * 128 : (i + 1) * 128, :])
                bo_sb = sb.tile([1, HIDDEN], DT)
                nc.sync.dma_start(bo_sb[:], bo[:])
                nc.gpsimd.partition_broadcast(bo_bc[:], bo_sb[:1, :])

                accs = new_accs()
                stash0 = []

                def unit0kt(kt_i):
                    stash0.extend(zip((kt_i, kt_i), (0, 1), emit_scores_exp2(0, kt_i)))
                    if len(stash0) > 2 * PRE_KT:
                        for _ in range(2):
                            pk, ph, pp = stash0.pop(0)
                            emit_av(pk, ph, accs, pp)

                for cp in range(1, HT):
                    if cp > 1:
                        emit_xt(cp)
                        # only stripe 1's q (chunks 2-3) projects in phase 1;
                        # stripes 2-3's q rides the later stripe boundaries where
                        # the exp backlog absorbs it, shrinking the per-chunk
                        # block that starves ACT here
                        pieces = k_pieces(cp) + (q_pieces(cp) if cp < 4 else []) + v_pieces(cp)
                    else:
                        pieces = k_pieces(cp) + v_pieces(cp)
                    a = 4 * (cp - 1)
                    for kk in range(4):
                        unit0kt(a + kk)
                        # one consolidated projection block per chunk: each
                        # injection site costs a fixed pipeline restart, so
                        # fewer sites beat evenly-spread pieces
                        if kk == 1:
                            for t in pieces:
                                t()
                for kt_i in range(4 * (HT - 1), att_nt):
                    unit0kt(kt_i)
                for pk, ph, pp in stash0:
                    emit_av(pk, ph, accs, pp)
                emit_finish_stripe(0, accs)

                # Stripe k's out-projection runs at the START boundary of stripe
                # k+2: its AllToAll is a full stripe old (no collective wait) and
                # the po blocks allocate from the just-freed ps_acc buffers, so
                # the exp stream's s_ps rotation is never interrupted mid-stripe.
                # The new stripe's first AV matmuls lag behind the po blocks but
                # the p_sb triple-buffer absorbs that.
                aTms = {}
                for m in range(1, NQC):
                    # boundary block: previous-previous stripe's out-projection
                    # plus the q projections for stripe m+1, all allocated from
                    # the just-freed ps_acc buffers so the s_ps rotation is
                    # untouched
                    blocks = []
                    if m >= 2:
                        for cc in range(2):
                            blocks += outproj_pieces(m - 2, [aTms[m - 2]], cc, npiece=1, tag="ps_acc")
                    if m < NQC - 1:
                        blocks += q_pieces(2 * (m + 1), tag="ps_acc")
                        blocks += q_pieces(2 * (m + 1) + 1, tag="ps_acc")
                    # pre-emit PRE_KT k-tiles of scores+exp so ACT stays fed
                    # while the boundary blocks occupy the in-order PE stream;
                    # the whole stripe then runs with AV matmuls lagging
                    # 2*PRE_KT units behind their scores, so the deferred AVs
                    # interleave with new scores instead of bunching up after
                    # the blocks
                    stash = []
                    for kt in range(PRE_KT):
                        stash.extend(zip((kt, kt), (0, 1), emit_scores_exp2(m, kt)))
                    for t in blocks:
                        t()
                    accs = new_accs()
                    for kt_i in range(PRE_KT, att_nt):
                        if kt_i == 24:
                            aTms[m - 1] = emit_aTm_load(m - 1)
                        p01 = emit_scores_exp2(m, kt_i)
                        for _ in range(2):
                            pk, ph, pp = stash.pop(0)
                            emit_av(pk, ph, accs, pp)
                        # last stripe: no more blocks need the backlog, so drain
                        # the lag early (one extra AV per k-tile keeps PE and
                        # ACT balanced) instead of flushing it as pure tail
                        if m == NQC - 1 and len(stash) > 2:
                            pk, ph, pp = stash.pop(0)
                            emit_av(pk, ph, accs, pp)
                        stash.extend(zip((kt_i, kt_i), (0, 1), p01))
                    for pk, ph, pp in stash:
                        emit_av(pk, ph, accs, pp)
                    emit_finish_stripe(m, accs)
                # tail: stripe 2's projection hides under stripe 3's AllToAll
                for cc in range(2):
                    for t in outproj_pieces(NQC - 2, [aTms[NQC - 2]], cc, npiece=1, tag="ps_acc"):
                        t()
                aTm3 = emit_aTm_load(NQC - 1)
                for cc in range(2):
                    for t in outproj_pieces(NQC - 1, [aTm3], cc, npiece=1, tag="ps_acc"):
                        t()

    nc.compile()
    return nc


def _get_nc(mm_mode: str):
    if mm_mode not in _CACHE:
        _CACHE[mm_mode] = _build(mm_mode)
    return _CACHE[mm_mode]


def make_in_maps(x, w_qkv, b_qkv, w_out, b_out):
    import ml_dtypes

    bf16 = ml_dtypes.bfloat16
    x = np.asarray(x, dtype=np.float32)
    w_qkv = np.asarray(w_qkv, dtype=np.float32)
    b_qkv = np.asarray(b_qkv, dtype=np.float32)
    w_out = np.asarray(w_out, dtype=np.float32)
    b_out = np.asarray(b_out, dtype=np.float32)

    xT = x.reshape(N, HIDDEN).T  # [hidden, n]
    # permute n into stripe order n' = (m, j, t) <-> n = 512*j + 128*m + t
    xT = np.ascontiguousarray(
        xT.reshape(HIDDEN, N_CORES, NQC, 128).transpose(0, 2, 1, 3).reshape(HIDDEN, N)
    ).astype(bf16)
    w_out_bf = np.ascontiguousarray(w_out).astype(bf16)
    bo = np.ascontiguousarray(b_out.reshape(1, HIDDEN))
    in_maps = []
    for c in range(N_CORES):
        cs = slice(c * AD, (c + 1) * AD)
        in_maps.append(
            {
                "xT": xT,
                "wq": np.ascontiguousarray(w_qkv[:, :HIDDEN][:, cs]).astype(bf16),
                "wk": np.ascontiguousarray(w_qkv[:, HIDDEN : 2 * HIDDEN][:, cs]).astype(bf16),
                "wv": np.ascontiguousarray(w_qkv[:, 2 * HIDDEN :][:, cs]).astype(bf16),
                "bq": np.ascontiguousarray(b_qkv[:HIDDEN][cs].reshape(AD, 1)),
                "bk": np.ascontiguousarray(b_qkv[HIDDEN : 2 * HIDDEN][cs].reshape(AD, 1)),
                "bvT": np.ascontiguousarray(b_qkv[2 * HIDDEN :][cs].reshape(1, AD)),
                "wo": w_out_bf,
                "bo": bo,
            }
        )
    return in_maps


def kernel(x, w_qkv, b_qkv, w_out, b_out):
    from concourse.bass_utils import run_bass_kernel_spmd

    mm_mode = os.environ.get("TRN_MM_MODE", "bf16")
    nc = _get_nc(mm_mode)
    in_maps = make_in_maps(x, w_qkv, b_qkv, w_out, b_out)
    res = run_bass_kernel_spmd(nc, in_maps, list(range(N_CORES)))
    full = np.concatenate([res.results[c]["out"] for c in range(N_CORES)], axis=0)
    return full.reshape(1, N, HIDDEN).astype(np.float32)



# revision 14
# speedup vs baseline: 2.1033x; 1.0652x over previous
"""Trainium2 Bass kernel for a 16-head dense attention layer (v2, bf16).

Problem: x[1,4096,1024] @ w_qkv[1024,3072] -> 16-head attention (N=4096,
D=64) -> @ w_out[1024,1024].

Sharding: tensor-parallel over heads across 8 NeuronCores (2 heads/core).
Each core computes q/k/v for its 2 heads (weights column-sliced on host),
attention with a fused, max-free softmax (scores are bounded so exp never
overflows in fp32; denominator comes from an appended ones-column in V),
then an AllToAll converts the head-sharded attention output into a
sequence-sharded layout so every core applies the full output projection
to its own 512 rows. Host concatenates the 8 row slices.

vs the f32r baseline: all matmul operands bf16 (PSUM accumulation stays
fp32; rel-err budget 2e-2), V projected directly in [keys, dims] layout
(no PE transposes), merged x DMAs prefetched on the Pool queue, lag-1
attention interleave in phase 1, one AllToAll per stripe, and attention
software-pipelined with AV matmuls lagging LAG=9 units behind their
scores+exp: the ACT engine's exp backlog rides through the consolidated
stripe-boundary blocks (older stripes' output projection plus the
next-next stripe's q projection, allocated from just-freed ps_acc
buffers) without starving, and the last stripe drains its lag early so
it does not flush as pure tail.
"""

import os
import numpy as np

N_CORES = 8
N = 4096
HIDDEN = 1024
D = 64
HPC = 2  # heads per core
AD = HPC * D  # 128 att-dim rows per core
NT = N // 128  # 32 k-tiles of 128
HT = HIDDEN // 128  # 8 hidden tiles
QCHUNK = 1024
NQC = N // QCHUNK  # 4 q-chunks (stripes)
NSLICE = N // N_CORES  # 512 rows of output per core

_CACHE = {}


def _build(
    mm_mode: str = "bf16",
    skip_a2a: bool = False,
    att_nt: int = NT,
    repeat: int = 1,
    score_order: str = "seq",
    pre_kt: int = 5,
):
    import concourse.bass as bass
    import concourse.mybir as mybir
    import concourse.tile as tile
    from concourse import bacc

    DT = mybir.dt.float32
    DTM = mybir.dt.bfloat16

    AF = mybir.ActivationFunctionType

    nc = bacc.Bacc("TRN2", debug=False, num_devices=N_CORES)

    xT = nc.dram_tensor("xT", [HIDDEN, N], DTM, kind="ExternalInput").ap()
    wq = nc.dram_tensor("wq", [HIDDEN, AD], DTM, kind="ExternalInput").ap()
    wk = nc.dram_tensor("wk", [HIDDEN, AD], DTM, kind="ExternalInput").ap()
    wv = nc.dram_tensor("wv", [HIDDEN, AD], DTM, kind="ExternalInput").ap()
    bq = nc.dram_tensor("bq", [AD, 1], DT, kind="ExternalInput").ap()
    bk = nc.dram_tensor("bk", [AD, 1], DT, kind="ExternalInput").ap()
    bvT = nc.dram_tensor("bvT", [1, AD], DT, kind="ExternalInput").ap()
    wo = nc.dram_tensor("wo", [HIDDEN, HIDDEN], DTM, kind="ExternalInput").ap()
    bo = nc.dram_tensor("bo", [1, HIDDEN], DT, kind="ExternalInput").ap()
    out = nc.dram_tensor("out", [NSLICE, HIDDEN], DT, kind="ExternalOutput").ap()

    with tile.TileContext(nc) as tc:
        with (
            tc.tile_pool(name="sb", bufs=1) as sb,
            tc.tile_pool(name="ps", bufs=2, space="PSUM") as ps,
            tc.tile_pool(name="dram", bufs=1, space="DRAM") as dram,
        ):
            # Global reordering: the sequence axis n is processed in
            # "stripe" order n' = (m, j, t) <-> n = 512*j + 128*m + t
            # (m: stripe 0..3, j: destination core 0..7, t: 0..127).
            # Attention is permutation-invariant in the key axis as long as
            # k and v use the same order, and the q axis just needs the
            # inverse map applied at output -- which the AllToAll block
            # routing does implicitly. Stripe m's attention output IS the
            # m-th out-row-tile of every core, so each stripe's AllToAll +
            # out-projection pipeline behind the next stripe's attention.

            # repeat>1 replicates the whole body inside one NEFF for
            # dispatch-amortized timing; kernel() always uses repeat=1.
            for _rep in range(repeat):
                bvT_sb = sb.tile([1, AD], DT)
                # qkv weights: one DMA each, [1024, 128] folded to [128, 8*128]
                wq_sb = sb.tile([128, HT * AD], DTM)
                wk_sb = sb.tile([128, HT * AD], DTM)
                wv_sb = sb.tile([128, HT * AD], DTM)
                bq_sb = sb.tile([AD, 1], DT)
                bk_sb = sb.tile([AD, 1], DT)
                bv_bc = sb.tile([128, AD], DT)

                def emit_weight_loads():
                    for w_sb, wsrc in ((wq_sb, wq), (wk_sb, wk), (wv_sb, wv)):
                        nc.sync.dma_start(
                            w_sb[:].rearrange("p (a c) -> p a c", a=HT),
                            wsrc.rearrange("(a p) c -> p a c", p=128),
                        )
                    nc.sync.dma_start(bq_sb[:], bq[:])
                    nc.sync.dma_start(bk_sb[:], bk[:])
                    nc.sync.dma_start(bvT_sb[:], bvT[:])
                    nc.gpsimd.partition_broadcast(bv_bc[:], bvT_sb[:1, :])

                def wslice(w_sb, i):
                    return w_sb[:, i * AD : (i + 1) * AD]

                # Host pre-permutes x columns into stripe order n' = (m, j, t),
                # so streaming, qT, kTc, v_nat are all plain contiguous in n'.
                qT = sb.tile([AD, N], DTM)
                kTc = [sb.tile([AD, 512], DTM, name=f"kTc{c}", tag="kTc", bufs=HT) for c in range(HT)]
                att_m = [sb.tile([AD, QCHUNK], DTM, name=f"attm{m}", tag="attm", bufs=NQC) for m in range(NQC)]
                # v in natural [keys, dims] layout: per chunk [128, (j, h, D+1)],
                # ones column at slot D of each head for the softmax denominator.
                v_nat = [
                    sb.tile([128, 4 * HPC * (D + 1)], DTM, name=f"vn{c}", tag="vnat", bufs=HT)
                    for c in range(HT)
                ]
                wo_sb = [sb.tile([128, HIDDEN], DTM, name=f"wo{i}", tag="wo", bufs=HT) for i in range(HT)]
                bo_bc = sb.tile([128, HIDDEN], DT)

                a2a_in = [
                    dram.tile([N_CORES, AD, 128], DTM, name=f"a2ai{m}", tag="a2ai", bufs=NQC)
                    for m in range(NQC)
                ]
                a2a_out = [
                    dram.tile([N_CORES, AD, 128], DTM, name=f"a2ao{m}", tag="a2ao", bufs=NQC)
                    for m in range(NQC)
                ]

                def vn_h(c, j, h):
                    """[128 keys, D+1] slice of chunk c's v for k-tile j, head h."""
                    base = (j * HPC + h) * (D + 1)
                    return v_nat[c][:, base : base + D + 1]

                # Attention runs with AV matmuls lagging 2*PRE_KT (kt,h) units
                # behind their scores+exp: the ACT engine keeps that deep a
                # backlog of materialized exps, so projection blocks occupying
                # the in-order PE stream no longer starve it.
                PRE_KT = pre_kt

                # ---- emission helpers --------------------------------------
                # All non-attention PE work is emitted as small "pieces" (2-4
                # matmuls, ~0.4-0.9us) with DVE partial accumulation, woven
                # between attention (scores+exp+AV) pairs. A long uninterrupted
                # matmul block would stall the in-order PE stream past the ~2
                # tiles of exp backlog the s_ps double-buffer can hold, idling
                # the ACT engine (the overall bottleneck) by its own duration.
                # Pieces are always injected in PAIRS so the number of ps_big
                # allocations between consecutive s_ps allocations stays even
                # and s_ps keeps alternating between its two buffers.
                xts = []

                def emit_xt(cp, eng=None):
                    """x chunk load via the (otherwise idle) Pool queue so the
                    SP queue's weight DMAs never delay it; 8 bufs = fully
                    prefetched, no reuse dependency between chunks (they also
                    stay resident for the deferred q projections)."""
                    cs = slice(cp * 512, (cp + 1) * 512)
                    xt = sb.tile([128, HT * 512], DTM, name="xt", tag="xt", bufs=HT)
                    xts.append(xt)
                    (eng or nc.gpsimd).dma_start(
                        xt[:].rearrange("p (a t) -> p a t", a=HT),
                        xT[:, cs].rearrange("(a p) t -> p a t", p=128),
                    )

                def qk_pieces(cp, w_sb, b_sb, dst, npiece, tag="ps_big"):
                    """q or k projection for chunk cp as npiece thunks."""
                    per = HT // npiece
                    thunks = []
                    for pc in range(npiece):
                        def piece(pc=pc):
                            pp = ps.tile([128, 512], DT, name="pp", tag=tag)
                            for i in range(per * pc, per * (pc + 1)):
                                nc.tensor.matmul(
                                    pp[:AD, :], wslice(w_sb, i),
                                    xts[cp][:, i * 512 : (i + 1) * 512],
                                    start=(i == per * pc), stop=(i == per * (pc + 1) - 1),
                                )
                            if pc == 0:
                                nc.vector.tensor_scalar_add(dst, pp[:AD, :], b_sb[:])
                            else:
                                nc.vector.tensor_add(dst, dst, pp[:AD, :])
                        thunks.append(piece)
                    return thunks

                def k_pieces(cp):
                    return qk_pieces(cp, wk_sb, bk_sb, kTc[cp][:], 2)

                def q_pieces(cp, npiece=2, tag="ps_big"):
                    cs = slice(cp * 512, (cp + 1) * 512)
                    return qk_pieces(cp, wq_sb, bq_sb, qT[:, cs], npiece, tag)

                def v_pieces(cp):
                    """v directly in [keys, dims] layout: x-chunk tile as the
                    stationary operand, wv moving; out partitions are the 128
                    keys of k-tile j. Two thunks of two k-tiles each."""
                    thunks = []
                    for half in (0, 1):
                        def piece(half=half):
                            pv = ps.tile([128, 256], DT, name="pv", tag="ps_big")
                            for jj in (0, 1):
                                j = 2 * half + jj
                                for i in range(HT):
                                    nc.tensor.matmul(
                                        pv[:, jj * 128 : (jj + 1) * 128],
                                        xts[cp][:, i * 512 + j * 128 : i * 512 + (j + 1) * 128],
                                        wslice(wv_sb, i),
                                        start=(i == 0), stop=(i == HT - 1),
                                    )
                            vn4 = v_nat[cp][:].rearrange("p (j h x) -> p j h x", j=4, x=D + 1)
                            for jj in (0, 1):
                                j = 2 * half + jj
                                nc.vector.tensor_add(
                                    vn4[:, j, :, :D],
                                    pv[:, jj * 128 : (jj + 1) * 128].rearrange(
                                        "p (h d) -> p h d", h=HPC
                                    ),
                                    bv_bc[:].rearrange("p (h d) -> p h d", h=HPC),
                                )
                            nc.vector.memset(
                                vn4[:, 2 * half : 2 * half + 2, :, D : D + 1], 1.0
                            )
                        thunks.append(piece)
                    return thunks

                def emit_scores_exp2(m, kt_i):
                    """Scores + exp for BOTH heads of k-tile kt_i, with the
                    score matmuls interleaved h0/h1: the two heads' K=64
                    contractions sit on row-groups 0-1 (partitions 0-63) and
                    2-3 (64-127), so adjacent MMs on different row groups run
                    CONCURRENTLY on the PE sub-arrays (tile_position is
                    auto-derived from the APs' base partitions) -- ~2x the
                    score throughput vs serial emission. Order h0a,h1a,h1b,h0b
                    keeps h1's stationary loaded for its second half (one
                    fewer LDWEIGHTS) while preserving pairwise overlap."""
                    s_list = [
                        ps.tile([128, QCHUNK], DT, name=f"s_ps{h}", tag="ps_big")
                        for h in range(HPC)
                    ]

                    def mm(h, half):
                        hs = slice(h * D, (h + 1) * D)
                        nc.tensor.matmul(
                            s_list[h][:, half * 512 : (half + 1) * 512],
                            kTc[kt_i // 4][hs, (kt_i % 4) * 128 : (kt_i % 4 + 1) * 128],
                            qT[hs, m * QCHUNK + half * 512 : m * QCHUNK + (half + 1) * 512],
                            start=True, stop=True,
                        )

                    def exph(h):
                        p_sb = sb.tile([128, QCHUNK], DTM, name="p_sb", tag="p_sb", bufs=2 * PRE_KT + 2)
                        nc.scalar.activation(p_sb[:], s_list[h][:], AF.Exp, scale=0.125)
                        return p_sb

                    if score_order == "il":
                        mm(0, 0)
                        mm(1, 0)
                        mm(1, 1)
                        mm(0, 1)
                        p_out = [exph(0), exph(1)]
                    else:  # "seq": per-head scores immediately followed by exp
                        mm(0, 0)
                        mm(0, 1)
                        p0 = exph(0)
                        mm(1, 0)
                        mm(1, 1)
                        p_out = [p0, exph(1)]
                    return p_out

                def emit_av(kt_i, h, accs, p_sb):
                    for half in range(2):
                        hsl = slice(half * 512, (half + 1) * 512)
                        nc.tensor.matmul(
                            accs[h][: D + 1, hsl],
                            vn_h(kt_i // 4, kt_i % 4, h),
                            p_sb[:, hsl],
                            start=(kt_i == 0), stop=(kt_i == att_nt - 1),
                        )

                def emit_finish_stripe(m, accs):
                    # copy both accumulators out of PSUM first so their banks
                    # free for the next stripe while normalization runs on SBUF
                    acc_sbs = []
                    for h in range(HPC):
                        acc_sb = sb.tile([D + 1, QCHUNK], DTM, name="acc_sb", tag="acc_sb", bufs=2)
                        nc.vector.tensor_copy(acc_sb[:], accs[h][: D + 1, :])
                        acc_sbs.append(acc_sb)
                    for h in range(HPC):
                        hs = slice(h * D, (h + 1) * D)
                        acc_sb = acc_sbs[h]
                        recip = sb.tile([1, QCHUNK], DTM, name="recip", tag="recip", bufs=2)
                        with nc.allow_low_precision(reason="softmax denom in bf16; tol 2e-2"):
                            nc.vector.reciprocal(recip[:], acc_sb[D : D + 1, :])
                        bcast = sb.tile([D, QCHUNK], DTM, name="bcast", tag="bcast", bufs=2)
                        nc.gpsimd.partition_broadcast(bcast[:], recip[:1, :])
                        nc.vector.tensor_mul(att_m[m][hs, :], acc_sb[:D, :], bcast[:])
                    nc.sync.dma_start(
                        a2a_in[m][:].rearrange("a p t -> p a t"),
                        att_m[m][:].rearrange("p (a t) -> p a t", a=N_CORES),
                    )
                    if not skip_a2a:
                        nc.gpsimd.collective_compute(
                            "AllToAll",
                            mybir.AluOpType.bypass,
                            replica_groups=[list(range(N_CORES))],
                            ins=[a2a_in[m].opt()],
                            outs=[a2a_out[m].opt()],
                        )

                def emit_aTm_load(m):
                    aTm = sb.tile([128, N_CORES * 128], DTM, name="aTm", tag="aTm", bufs=2)
                    nc.sync.dma_start(
                        aTm[:].rearrange("p (a t) -> p a t", a=N_CORES),
                        a2a_out[m][:].rearrange("a p t -> p a t"),
                    )
                    return aTm

                def outproj_pieces(m, aTm_ref, cc, npiece=4, tag="ps_big"):
                    """output projection for stripe m, 512-column half cc, as
                    npiece thunks accumulating into an SBUF tile via DVE."""
                    os_ = slice(cc * 512, (cc + 1) * 512)
                    per = HT // npiece
                    holder = []
                    thunks = []
                    for pc in range(npiece):
                        def piece(pc=pc):
                            po = ps.tile([128, 512], DT, name="po", tag=tag)
                            for i in range(per * pc, per * (pc + 1)):
                                nc.tensor.matmul(
                                    po[:], aTm_ref[0][:, i * 128 : (i + 1) * 128],
                                    wo_sb[i][:, os_],
                                    start=(i == per * pc), stop=(i == per * (pc + 1) - 1),
                                )
                            if pc == 0:
                                out_sb = sb.tile([128, 512], DT, name="out_sb", tag="out_sb", bufs=2)
                                holder.append(out_sb)
                                nc.vector.tensor_add(out_sb[:], po[:], bo_bc[:, os_])
                            else:
                                out_sb = holder[0]
                                nc.vector.tensor_add(out_sb[:], out_sb[:], po[:])
                            if pc == npiece - 1:
                                nc.sync.dma_start(out[m * 128 : (m + 1) * 128, os_], out_sb[:])
                        thunks.append(piece)
                    return thunks

                # ---- schedule ----------------------------------------------
                def new_accs():
                    return [
                        ps.tile([128, QCHUNK], DT, name=f"acc{h}", tag="ps_acc")
                        for h in range(HPC)
                    ]

                # phase 1: stream chunks with stripe-0 attention interleaved at
                # lag 1 (chunk cp delivers k-tiles 4cp..4cp+3; attention trails
                # one chunk behind so exp work reaches ACT as early as possible).
                # Phase 1 is PE/supply-bound, so projection blocks sit between
                # attention groups without extra cost.
                # chunk-1's q runs before chunk-0's v so the first scores+exp
                # fire as early as possible
                emit_weight_loads()
                emit_xt(0)
                emit_xt(1)
                for t in k_pieces(0) + q_pieces(0) + q_pieces(1) + v_pieces(0):
                    t()
                # out-proj weights load early on the SP queue (x loads are on
                # Pool, so these only queue behind qkv weights)
                for i in range(HT):
                    nc.sync.dma_start(wo_sb[i][:], wo[i * 128 : (i + 1) * 128, :])
                bo_sb = sb.tile([1, HIDDEN], DT)
                nc.sync.dma_start(bo_sb[:], bo[:])
                nc.gpsimd.partition_broadcast(bo_bc[:], bo_sb[:1, :])

                accs = new_accs()
                stash0 = []

                def unit0kt(kt_i):
                    stash0.extend(zip((kt_i, kt_i), (0, 1), emit_scores_exp2(0, kt_i)))
                    if len(stash0) > 2 * PRE_KT:
                        for _ in range(2):
                            pk, ph, pp = stash0.pop(0)
                            emit_av(pk, ph, accs, pp)

                for cp in range(1, HT):
                    if cp > 1:
                        emit_xt(cp)
                        # only stripe 1's q (chunks 2-3) projects in phase 1;
                        # stripes 2-3's q rides the later stripe boundaries where
                        # the exp backlog absorbs it, shrinking the per-chunk
                        # block that starves ACT here
                        pieces = k_pieces(cp) + (q_pieces(cp) if cp < 4 else []) + v_pieces(cp)
                    else:
                        pieces = k_pieces(cp) + v_pieces(cp)
                    a = 4 * (cp - 1)
                    for kk in range(4):
                        unit0kt(a + kk)
                        # one consolidated projection block per chunk: each
                        # injection site costs a fixed pipeline restart, so
                        # fewer sites beat evenly-spread pieces
                        if kk == 1:
                            for t in pieces:
                                t()
                for kt_i in range(4 * (HT - 1), att_nt):
                    unit0kt(kt_i)
                for pk, ph, pp in stash0:
                    emit_av(pk, ph, accs, pp)
                emit_finish_stripe(0, accs)

                # Stripe k's out-projection runs at the START boundary of stripe
                # k+2: its AllToAll is a full stripe old (no collective wait) and
                # the po blocks allocate from the just-freed ps_acc buffers, so
                # the exp stream's s_ps rotation is never interrupted mid-stripe.
                # The new stripe's first AV matmuls lag behind the po blocks but
                # the p_sb triple-buffer absorbs that.
                aTms = {}
                for m in range(1, NQC):
                    # boundary block: previous-previous stripe's out-projection
                    # plus the q projections for stripe m+1, all allocated from
                    # the just-freed ps_acc buffers so the s_ps rotation is
                    # untouched
                    blocks = []
                    if m >= 2:
                        for cc in range(2):
                            blocks += outproj_pieces(m - 2, [aTms[m - 2]], cc, npiece=1, tag="ps_acc")
                    if m < NQC - 1:
                        blocks += q_pieces(2 * (m + 1), tag="ps_acc")
                        blocks += q_pieces(2 * (m + 1) + 1, tag="ps_acc")
                    # pre-emit PRE_KT k-tiles of scores+exp so ACT stays fed
                    # while the boundary blocks occupy the in-order PE stream;
                    # the whole stripe then runs with AV matmuls lagging
                    # 2*PRE_KT units behind their scores, so the deferred AVs
                    # interleave with new scores instead of bunching up after
                    # the blocks
                    stash = []
                    for kt in range(PRE_KT):
                        stash.extend(zip((kt, kt), (0, 1), emit_scores_exp2(m, kt)))
                    for t in blocks:
                        t()
                    accs = new_accs()
                    for kt_i in range(PRE_KT, att_nt):
                        if kt_i == att_nt - 8:
                            aTms[m - 1] = emit_aTm_load(m - 1)
                        p01 = emit_scores_exp2(m, kt_i)
                        for _ in range(2):
                            pk, ph, pp = stash.pop(0)
                            emit_av(pk, ph, accs, pp)
                        # last stripe: no more blocks need the backlog, so drain
                        # the lag early (one extra AV per k-tile keeps PE and
                        # ACT balanced) instead of flushing it as pure tail
                        if m == NQC - 1 and len(stash) > 2:
                            pk, ph, pp = stash.pop(0)
                            emit_av(pk, ph, accs, pp)
                        stash.extend(zip((kt_i, kt_i), (0, 1), p01))
                    for pk, ph, pp in stash:
                        emit_av(pk, ph, accs, pp)
                    emit_finish_stripe(m, accs)
                # tail: stripe 2's projection hides under stripe 3's AllToAll
                for cc in range(2):
                    for t in outproj_pieces(NQC - 2, [aTms[NQC - 2]], cc, npiece=1, tag="ps_acc"):
                        t()
                aTm3 = emit_aTm_load(NQC - 1)
                for cc in range(2):
                    for t in outproj_pieces(NQC - 1, [aTm3], cc, npiece=1, tag="ps_acc"):
                        t()

    nc.compile()
    return nc


def _get_nc(mm_mode: str):
    if mm_mode not in _CACHE:
        _CACHE[mm_mode] = _build(mm_mode)
    return _CACHE[mm_mode]


def make_in_maps(x, w_qkv, b_qkv, w_out, b_out):
    import ml_dtypes

    bf16 = ml_dtypes.bfloat16
    x = np.asarray(x, dtype=np.float32)
    w_qkv = np.asarray(w_qkv, dtype=np.float32)
    b_qkv = np.asarray(b_qkv, dtype=np.float32)
    w_out = np.asarray(w_out, dtype=np.float32)
    b_out = np.asarray(b_out, dtype=np.float32)

    xT = x.reshape(N, HIDDEN).T  # [hidden, n]
    # permute n into stripe order n' = (m, j, t) <-> n = 512*j + 128*m + t
    xT = np.ascontiguousarray(
        xT.reshape(HIDDEN, N_CORES, NQC, 128).transpose(0, 2, 1, 3).reshape(HIDDEN, N)
    ).astype(bf16)
    w_out_bf = np.ascontiguousarray(w_out).astype(bf16)
    bo = np.ascontiguousarray(b_out.reshape(1, HIDDEN))
    in_maps = []
    for c in range(N_CORES):
        cs = slice(c * AD, (c + 1) * AD)
        in_maps.append(
            {
                "xT": xT,
                "wq": np.ascontiguousarray(w_qkv[:, :HIDDEN][:, cs]).astype(bf16),
                "wk": np.ascontiguousarray(w_qkv[:, HIDDEN : 2 * HIDDEN][:, cs]).astype(bf16),
                "wv": np.ascontiguousarray(w_qkv[:, 2 * HIDDEN :][:, cs]).astype(bf16),
                "bq": np.ascontiguousarray(b_qkv[:HIDDEN][cs].reshape(AD, 1)),
                "bk": np.ascontiguousarray(b_qkv[HIDDEN : 2 * HIDDEN][cs].reshape(AD, 1)),
                "bvT": np.ascontiguousarray(b_qkv[2 * HIDDEN :][cs].reshape(1, AD)),
                "wo": w_out_bf,
                "bo": bo,
            }
        )
    return in_maps


def kernel(x, w_qkv, b_qkv, w_out, b_out):
    from concourse.bass_utils import run_bass_kernel_spmd

    mm_mode = os.environ.get("TRN_MM_MODE", "bf16")
    nc = _get_nc(mm_mode)
    in_maps = make_in_maps(x, w_qkv, b_qkv, w_out, b_out)
    res = run_bass_kernel_spmd(nc, in_maps, list(range(N_CORES)))
    full = np.concatenate([res.results[c]["out"] for c in range(N_CORES)], axis=0)
    return full.reshape(1, N, HIDDEN).astype(np.float32)



# revision 20
# speedup vs baseline: 2.1229x; 1.0093x over previous
"""Trainium2 Bass kernel for a 16-head dense attention layer (v2, bf16).

Problem: x[1,4096,1024] @ w_qkv[1024,3072] -> 16-head attention (N=4096,
D=64) -> @ w_out[1024,1024].

Sharding: tensor-parallel over heads across 8 NeuronCores (2 heads/core).
Each core computes q/k/v for its 2 heads (weights column-sliced on host),
attention with a fused, max-free softmax (scores are bounded so exp never
overflows in fp32; denominator comes from an appended ones-column in V),
then an AllToAll converts the head-sharded attention output into a
sequence-sharded layout so every core applies the full output projection
to its own 512 rows. Host concatenates the 8 row slices.

vs the f32r baseline: all matmul operands bf16 (PSUM accumulation stays
fp32; rel-err budget 2e-2), V projected directly in [keys, dims] layout
(no PE transposes), merged x DMAs prefetched on the Pool queue, lag-1
attention interleave in phase 1, one AllToAll per stripe, and attention
software-pipelined with AV matmuls lagging 2*PRE_KT (kt,h) units behind
their scores+exp: the ACT engine's exp backlog rides through the
consolidated stripe-boundary blocks (older stripes' output projection
plus the next-next stripe's q projection, allocated from just-freed
ps_acc buffers) without starving, and the last stripe drains its lag
early so it does not flush as pure tail.

Hardware notes (measured via micro-benchmarks on this axon/trn2 stack,
see microprobe.py):
- ACT/DVE instructions whose SOURCE is PSUM largely serialize against
  concurrent PE matmul execution (measured near-additive even on
  disjoint PSUM banks), while SBUF-sourced ACT/DVE work overlaps PE
  fine. The exp stream (PSUM->SBUF) therefore sets a serial floor of
  roughly PE-time + exp-time per k-tile; scheduling can recover only a
  partial (~10-35%) overlap. Fusing both heads into one single-buffered
  [128,2048] exp (score_order="f2") measured ~50us WORSE than "seq" --
  the partial overlap double-buffering enables is worth more than the
  saved per-instruction ACT overhead.
- Adjacent matmuls on disjoint PE row-groups run concurrently
  (tile_position auto-derived from base partitions): an interleaved
  h0/h1 score burst measured 415ns vs 1830ns serial in isolation. In
  the full kernel, however, "seq" (per-head scores immediately followed
  by that head's exp) measured best: the interleaved burst's last MM
  gates the next exp and head-of-line blocks the in-order PE queue.
- repeat>1 replicates the whole body in one NEFF for timing: the
  per-dispatch overhead of this axon client is ~0.8ms (trivial-kernel
  floor), which would otherwise dominate the measurement.
"""

import os
import numpy as np

N_CORES = 8
N = 4096
HIDDEN = 1024
D = 64
HPC = 2  # heads per core
AD = HPC * D  # 128 att-dim rows per core
NT = N // 128  # 32 k-tiles of 128
HT = HIDDEN // 128  # 8 hidden tiles
QCHUNK = 1024
NQC = N // QCHUNK  # 4 q-chunks (stripes)
NSLICE = N // N_CORES  # 512 rows of output per core

_CACHE = {}


def _build(
    mm_mode: str = "bf16",
    skip_a2a: bool = False,
    att_nt: int = NT,
    repeat: int = 1,
    score_order: str = "seq",
    pre_kt: int = 5,
    p_bufs: int = 0,
):
    import concourse.bass as bass
    import concourse.mybir as mybir
    import concourse.tile as tile
    from concourse import bacc

    DT = mybir.dt.float32
    DTM = mybir.dt.bfloat16

    AF = mybir.ActivationFunctionType

    nc = bacc.Bacc("TRN2", debug=False, num_devices=N_CORES)

    xT = nc.dram_tensor("xT", [HIDDEN, N], DTM, kind="ExternalInput").ap()
    wq = nc.dram_tensor("wq", [HIDDEN, AD], DTM, kind="ExternalInput").ap()
    wk = nc.dram_tensor("wk", [HIDDEN, AD], DTM, kind="ExternalInput").ap()
    wv = nc.dram_tensor("wv", [HIDDEN, AD], DTM, kind="ExternalInput").ap()
    bq = nc.dram_tensor("bq", [AD, 1], DT, kind="ExternalInput").ap()
    bk = nc.dram_tensor("bk", [AD, 1], DT, kind="ExternalInput").ap()
    bvT = nc.dram_tensor("bvT", [1, AD], DT, kind="ExternalInput").ap()
    wo = nc.dram_tensor("wo", [HIDDEN, HIDDEN], DTM, kind="ExternalInput").ap()
    bo = nc.dram_tensor("bo", [1, HIDDEN], DT, kind="ExternalInput").ap()
    out = nc.dram_tensor("out", [NSLICE, HIDDEN], DT, kind="ExternalOutput").ap()

    with tile.TileContext(nc) as tc:
        with (
            tc.tile_pool(name="sb", bufs=1) as sb,
            tc.tile_pool(name="ps", bufs=2, space="PSUM") as ps,
            tc.tile_pool(name="dram", bufs=1, space="DRAM") as dram,
        ):
            # Global reordering: the sequence axis n is processed in
            # "stripe" order n' = (m, j, t) <-> n = 512*j + 128*m + t
            # (m: stripe 0..3, j: destination core 0..7, t: 0..127).
            # Attention is permutation-invariant in the key axis as long as
            # k and v use the same order, and the q axis just needs the
            # inverse map applied at output -- which the AllToAll block
            # routing does implicitly. Stripe m's attention output IS the
            # m-th out-row-tile of every core, so each stripe's AllToAll +
            # out-projection pipeline behind the next stripe's attention.

            # repeat>1 replicates the whole body inside one NEFF for
            # dispatch-amortized timing; kernel() always uses repeat=1.
            for _rep in range(repeat):
                bvT_sb = sb.tile([1, AD], DT)
                # qkv weights: one DMA each, [1024, 128] folded to [128, 8*128]
                wq_sb = sb.tile([128, HT * AD], DTM)
                wk_sb = sb.tile([128, HT * AD], DTM)
                wv_sb = sb.tile([128, HT * AD], DTM)
                bq_sb = sb.tile([AD, 1], DT)
                bk_sb = sb.tile([AD, 1], DT)
                bv_bc = sb.tile([128, AD], DT)

                def emit_weight_loads():
                    for w_sb, wsrc in ((wq_sb, wq), (wk_sb, wk), (wv_sb, wv)):
                        nc.sync.dma_start(
                            w_sb[:].rearrange("p (a c) -> p a c", a=HT),
                            wsrc.rearrange("(a p) c -> p a c", p=128),
                        )
                    nc.sync.dma_start(bq_sb[:], bq[:])
                    nc.sync.dma_start(bk_sb[:], bk[:])
                    nc.sync.dma_start(bvT_sb[:], bvT[:])
                    nc.gpsimd.partition_broadcast(bv_bc[:], bvT_sb[:1, :])

                def wslice(w_sb, i):
                    return w_sb[:, i * AD : (i + 1) * AD]

                # Host pre-permutes x columns into stripe order n' = (m, j, t),
                # so streaming, qT, kTc, v_nat are all plain contiguous in n'.
                qT = sb.tile([AD, N], DTM)
                kTc = [sb.tile([AD, 512], DTM, name=f"kTc{c}", tag="kTc", bufs=HT) for c in range(HT)]
                att_m = [sb.tile([AD, QCHUNK], DTM, name=f"attm{m}", tag="attm", bufs=NQC) for m in range(NQC)]
                # v in natural [keys, dims] layout: per chunk [128, (j, h, D+1)],
                # ones column at slot D of each head for the softmax denominator.
                v_nat = [
                    sb.tile([128, 4 * HPC * (D + 1)], DTM, name=f"vn{c}", tag="vnat", bufs=HT)
                    for c in range(HT)
                ]
                wo_sb = [sb.tile([128, HIDDEN], DTM, name=f"wo{i}", tag="wo", bufs=HT) for i in range(HT)]
                bo_bc = sb.tile([128, HIDDEN], DT)

                a2a_in = [
                    dram.tile([N_CORES, AD, 128], DTM, name=f"a2ai{m}", tag="a2ai", bufs=NQC)
                    for m in range(NQC)
                ]
                a2a_out = [
                    dram.tile([N_CORES, AD, 128], DTM, name=f"a2ao{m}", tag="a2ao", bufs=NQC)
                    for m in range(NQC)
                ]

                def vn_h(c, j, h):
                    """[128 keys, D+1] slice of chunk c's v for k-tile j, head h."""
                    base = (j * HPC + h) * (D + 1)
                    return v_nat[c][:, base : base + D + 1]

                # Attention runs with AV matmuls lagging 2*PRE_KT (kt,h) units
                # behind their scores+exp: the ACT engine keeps that deep a
                # backlog of materialized exps, so projection blocks occupying
                # the in-order PE stream no longer starve it.
                PRE_KT = pre_kt

                # ---- emission helpers --------------------------------------
                # All non-attention PE work is emitted as small "pieces" (2-4
                # matmuls, ~0.4-0.9us) with DVE partial accumulation, woven
                # between attention (scores+exp+AV) pairs. A long uninterrupted
                # matmul block would stall the in-order PE stream past the ~2
                # tiles of exp backlog the s_ps double-buffer can hold, idling
                # the ACT engine (the overall bottleneck) by its own duration.
                # Pieces are always injected in PAIRS so the number of ps_big
                # allocations between consecutive s_ps allocations stays even
                # and s_ps keeps alternating between its two buffers.
                xts = []

                def emit_xt(cp, eng=None):
                    """x chunk load via the (otherwise idle) Pool queue so the
                    SP queue's weight DMAs never delay it; 8 bufs = fully
                    prefetched, no reuse dependency between chunks (they also
                    stay resident for the deferred q projections)."""
                    cs = slice(cp * 512, (cp + 1) * 512)
                    xt = sb.tile([128, HT * 512], DTM, name="xt", tag="xt", bufs=HT)
                    xts.append(xt)
                    (eng or nc.gpsimd).dma_start(
                        xt[:].rearrange("p (a t) -> p a t", a=HT),
                        xT[:, cs].rearrange("(a p) t -> p a t", p=128),
                    )

                # default PSUM tag for projection pieces: under f2 the score
                # tile is a single-buffered 4-bank [128,2048], and pieces
                # share its slot (PSUM budget: s2 4 + ps_acc 4 = 8 banks)
                PTAG = "s2" if score_order == "f2" else "ps_big"

                def qk_pieces(cp, w_sb, b_sb, dst, npiece, tag=None):
                    """q or k projection for chunk cp as npiece thunks."""
                    tag = tag or PTAG
                    per = HT // npiece
                    thunks = []
                    for pc in range(npiece):
                        def piece(pc=pc):
                            pp = ps.tile([128, 512], DT, name="pp", tag=tag, bufs=1 if tag == "s2" else None)
                            for i in range(per * pc, per * (pc + 1)):
                                nc.tensor.matmul(
                                    pp[:AD, :], wslice(w_sb, i),
                                    xts[cp][:, i * 512 : (i + 1) * 512],
                                    start=(i == per * pc), stop=(i == per * (pc + 1) - 1),
                                )
                            if pc == 0:
                                nc.vector.tensor_scalar_add(dst, pp[:AD, :], b_sb[:])
                            else:
                                nc.vector.tensor_add(dst, dst, pp[:AD, :])
                        thunks.append(piece)
                    return thunks

                def k_pieces(cp):
                    return qk_pieces(cp, wk_sb, bk_sb, kTc[cp][:], 2)

                def q_pieces(cp, npiece=2, tag=None):
                    cs = slice(cp * 512, (cp + 1) * 512)
                    return qk_pieces(cp, wq_sb, bq_sb, qT[:, cs], npiece, tag)

                def v_pieces(cp):
                    """v directly in [keys, dims] layout: x-chunk tile as the
                    stationary operand, wv moving; out partitions are the 128
                    keys of k-tile j. Two thunks of two k-tiles each."""
                    thunks = []
                    for half in (0, 1):
                        def piece(half=half):
                            pv = ps.tile([128, 256], DT, name="pv", tag=PTAG, bufs=1 if PTAG == "s2" else None)
                            for jj in (0, 1):
                                j = 2 * half + jj
                                for i in range(HT):
                                    nc.tensor.matmul(
                                        pv[:, jj * 128 : (jj + 1) * 128],
                                        xts[cp][:, i * 512 + j * 128 : i * 512 + (j + 1) * 128],
                                        wslice(wv_sb, i),
                                        start=(i == 0), stop=(i == HT - 1),
                                    )
                            vn4 = v_nat[cp][:].rearrange("p (j h x) -> p j h x", j=4, x=D + 1)
                            for jj in (0, 1):
                                j = 2 * half + jj
                                nc.vector.tensor_add(
                                    vn4[:, j, :, :D],
                                    pv[:, jj * 128 : (jj + 1) * 128].rearrange(
                                        "p (h d) -> p h d", h=HPC
                                    ),
                                    bv_bc[:].rearrange("p (h d) -> p h d", h=HPC),
                                )
                            nc.vector.memset(
                                vn4[:, 2 * half : 2 * half + 2, :, D : D + 1], 1.0
                            )
                        thunks.append(piece)
                    return thunks

                def emit_scores_exp2(m, kt_i):
                    """Scores + exp for BOTH heads of k-tile kt_i. "f2": one
                    single-buffered [128,2048] PSUM tile holds both heads'
                    scores and ONE exp covers them -- halves the per-
                    instruction ACT overhead; single-buffering costs nothing
                    because ACT PSUM-reads serialize against PE matmul
                    execution on this hardware anyway (measured). The score
                    matmuls interleave h0/h1: the two heads' K=64 contractions
                    sit on row-groups 0-1 (partitions 0-63) and 2-3 (64-127),
                    so adjacent MMs on different row groups run CONCURRENTLY
                    on the PE sub-arrays (tile_position auto-derives from the
                    APs' base partitions)."""

                    def mm_into(dst, h, half):
                        hs = slice(h * D, (h + 1) * D)
                        nc.tensor.matmul(
                            dst,
                            kTc[kt_i // 4][hs, (kt_i % 4) * 128 : (kt_i % 4 + 1) * 128],
                            qT[hs, m * QCHUNK + half * 512 : m * QCHUNK + (half + 1) * 512],
                            start=True, stop=True,
                        )

                    if score_order == "f2":
                        s2 = ps.tile([128, 2 * QCHUNK], DT, name="s2", tag="s2", bufs=1)
                        for h, half in ((0, 0), (1, 0), (1, 1), (0, 1)):
                            base = h * QCHUNK + half * 512
                            mm_into(s2[:, base : base + 512], h, half)
                        p2 = sb.tile(
                            [128, 2 * QCHUNK], DTM, name="p_sb", tag="p_sb", bufs=PRE_KT + 1
                        )
                        nc.scalar.activation(p2[:], s2[:], AF.Exp, scale=0.125)
                        return [p2[:, :QCHUNK], p2[:, QCHUNK:]]

                    s_list = [
                        ps.tile([128, QCHUNK], DT, name=f"s_ps{h}", tag="ps_big")
                        for h in range(HPC)
                    ]

                    def exph(h):
                        p_sb = sb.tile([128, QCHUNK], DTM, name="p_sb", tag="p_sb", bufs=p_bufs or (2 * PRE_KT + 2))
                        nc.scalar.activation(p_sb[:], s_list[h][:], AF.Exp, scale=0.125)
                        return p_sb

                    if score_order == "il":
                        for h, half in ((0, 0), (1, 0), (1, 1), (0, 1)):
                            mm_into(s_list[h][:, half * 512 : (half + 1) * 512], h, half)
                        p_out = [exph(0), exph(1)]
                    else:  # "seq": per-head scores immediately followed by exp
                        mm_into(s_list[0][:, :512], 0, 0)
                        mm_into(s_list[0][:, 512:], 0, 1)
                        p0 = exph(0)
                        mm_into(s_list[1][:, :512], 1, 0)
                        mm_into(s_list[1][:, 512:], 1, 1)
                        p_out = [p0, exph(1)]
                    return p_out

                def emit_av(kt_i, h, accs, p_sb):
                    for half in range(2):
                        hsl = slice(half * 512, (half + 1) * 512)
                        nc.tensor.matmul(
                            accs[h][: D + 1, hsl],
                            vn_h(kt_i // 4, kt_i % 4, h),
                            p_sb[:, hsl],
                            start=(kt_i == 0), stop=(kt_i == att_nt - 1),
                        )

                def emit_finish_stripe(m, accs):
                    # copy both accumulators out of PSUM first so their banks
                    # free for the next stripe while normalization runs on SBUF
                    acc_sbs = []
                    for h in range(HPC):
                        acc_sb = sb.tile([D + 1, QCHUNK], DTM, name="acc_sb", tag="acc_sb", bufs=2)
                        nc.vector.tensor_copy(acc_sb[:], accs[h][: D + 1, :])
                        acc_sbs.append(acc_sb)
                    for h in range(HPC):
                        hs = slice(h * D, (h + 1) * D)
                        acc_sb = acc_sbs[h]
                        recip = sb.tile([1, QCHUNK], DTM, name="recip", tag="recip", bufs=2)
                        with nc.allow_low_precision(reason="softmax denom in bf16; tol 2e-2"):
                            nc.vector.reciprocal(recip[:], acc_sb[D : D + 1, :])
                        bcast = sb.tile([D, QCHUNK], DTM, name="bcast", tag="bcast", bufs=2)
                        nc.gpsimd.partition_broadcast(bcast[:], recip[:1, :])
                        nc.vector.tensor_mul(att_m[m][hs, :], acc_sb[:D, :], bcast[:])
                    nc.sync.dma_start(
                        a2a_in[m][:].rearrange("a p t -> p a t"),
                        att_m[m][:].rearrange("p (a t) -> p a t", a=N_CORES),
                    )
                    if not skip_a2a:
                        nc.gpsimd.collective_compute(
                            "AllToAll",
                            mybir.AluOpType.bypass,
                            replica_groups=[list(range(N_CORES))],
                            ins=[a2a_in[m].opt()],
                            outs=[a2a_out[m].opt()],
                        )

                def emit_aTm_load(m):
                    aTm = sb.tile([128, N_CORES * 128], DTM, name="aTm", tag="aTm", bufs=2)
                    nc.sync.dma_start(
                        aTm[:].rearrange("p (a t) -> p a t", a=N_CORES),
                        a2a_out[m][:].rearrange("a p t -> p a t"),
                    )
                    return aTm

                def outproj_pieces(m, aTm_ref, cc, npiece=4, tag=None):
                    """output projection for stripe m, 512-column half cc, as
                    npiece thunks accumulating into an SBUF tile via DVE."""
                    os_ = slice(cc * 512, (cc + 1) * 512)
                    per = HT // npiece
                    holder = []
                    thunks = []
                    for pc in range(npiece):
                        def piece(pc=pc):
                            po = ps.tile([128, 512], DT, name="po", tag=tag, bufs=1 if tag == "s2" else None)
                            for i in range(per * pc, per * (pc + 1)):
                                nc.tensor.matmul(
                                    po[:], aTm_ref[0][:, i * 128 : (i + 1) * 128],
                                    wo_sb[i][:, os_],
                                    start=(i == per * pc), stop=(i == per * (pc + 1) - 1),
                                )
                            if pc == 0:
                                out_sb = sb.tile([128, 512], DT, name="out_sb", tag="out_sb", bufs=2)
                                holder.append(out_sb)
                                nc.vector.tensor_add(out_sb[:], po[:], bo_bc[:, os_])
                            else:
                                out_sb = holder[0]
                                nc.vector.tensor_add(out_sb[:], out_sb[:], po[:])
                            if pc == npiece - 1:
                                nc.sync.dma_start(out[m * 128 : (m + 1) * 128, os_], out_sb[:])
                        thunks.append(piece)
                    return thunks

                # ---- schedule ----------------------------------------------
                def new_accs():
                    return [
                        ps.tile([128, QCHUNK], DT, name=f"acc{h}", tag="ps_acc")
                        for h in range(HPC)
                    ]

                # phase 1: stream chunks with stripe-0 attention interleaved at
                # lag 1 (chunk cp delivers k-tiles 4cp..4cp+3; attention trails
                # one chunk behind so exp work reaches ACT as early as possible).
                # Phase 1 is PE/supply-bound, so projection blocks sit between
                # attention groups without extra cost.
                # chunk-1's q runs before chunk-0's v so the first scores+exp
                # fire as early as possible
                emit_weight_loads()
                emit_xt(0)
                emit_xt(1)
                for t in k_pieces(0) + q_pieces(0) + q_pieces(1) + v_pieces(0):
                    t()
                # out-proj weights load early on the SP queue (x loads are on
                # Pool, so these only queue behind qkv weights)
                for i in range(HT):
                    nc.sync.dma_start(wo_sb[i][:], wo[i * 128 : (i + 1) * 128, :])
                bo_sb = sb.tile([1, HIDDEN], DT)
                nc.sync.dma_start(bo_sb[:], bo[:])
                nc.gpsimd.partition_broadcast(bo_bc[:], bo_sb[:1, :])

                accs = new_accs()
                stash0 = []

                def unit0kt(kt_i):
                    stash0.extend(zip((kt_i, kt_i), (0, 1), emit_scores_exp2(0, kt_i)))
                    if len(stash0) > 2 * PRE_KT:
                        for _ in range(2):
                            pk, ph, pp = stash0.pop(0)
                            emit_av(pk, ph, accs, pp)

                for cp in range(1, HT):
                    if cp > 1:
                        emit_xt(cp)
                        # only stripe 1's q (chunks 2-3) projects in phase 1;
                        # stripes 2-3's q rides the later stripe boundaries where
                        # the exp backlog absorbs it, shrinking the per-chunk
                        # block that starves ACT here
                        pieces = k_pieces(cp) + (q_pieces(cp) if cp < 4 else []) + v_pieces(cp)
                    else:
                        pieces = k_pieces(cp) + v_pieces(cp)
                    a = 4 * (cp - 1)
                    for kk in range(4):
                        unit0kt(a + kk)
                        # one consolidated projection block per chunk: each
                        # injection site costs a fixed pipeline restart, so
                        # fewer sites beat evenly-spread pieces
                        if kk == 1:
                            for t in pieces:
                                t()
                for kt_i in range(4 * (HT - 1), att_nt):
                    unit0kt(kt_i)
                for pk, ph, pp in stash0:
                    emit_av(pk, ph, accs, pp)
                emit_finish_stripe(0, accs)

                # Stripe k's out-projection runs at the START boundary of stripe
                # k+2: its AllToAll is a full stripe old (no collective wait) and
                # the po blocks allocate from the just-freed ps_acc buffers, so
                # the exp stream's s_ps rotation is never interrupted mid-stripe.
                # The new stripe's first AV matmuls lag behind the po blocks but
                # the p_sb triple-buffer absorbs that.
                aTms = {}
                for m in range(1, NQC):
                    # boundary block: previous-previous stripe's out-projection
                    # plus the q projections for stripe m+1, all allocated from
                    # the just-freed ps_acc buffers so the s_ps rotation is
                    # untouched
                    blocks = []
                    if m >= 2:
                        for cc in range(2):
                            blocks += outproj_pieces(m - 2, [aTms[m - 2]], cc, npiece=1, tag="ps_acc")
                    if m < NQC - 1:
                        blocks += q_pieces(2 * (m + 1), tag="ps_acc")
                        blocks += q_pieces(2 * (m + 1) + 1, tag="ps_acc")
                    # pre-emit PRE_KT k-tiles of scores+exp so ACT stays fed
                    # while the boundary blocks occupy the in-order PE stream;
                    # the whole stripe then runs with AV matmuls lagging
                    # 2*PRE_KT units behind their scores, so the deferred AVs
                    # interleave with new scores instead of bunching up after
                    # the blocks
                    stash = []
                    for kt in range(PRE_KT):
                        stash.extend(zip((kt, kt), (0, 1), emit_scores_exp2(m, kt)))
                    for t in blocks:
                        t()
                    accs = new_accs()
                    for kt_i in range(PRE_KT, att_nt):
                        if kt_i == att_nt - 8:
                            aTms[m - 1] = emit_aTm_load(m - 1)
                        p01 = emit_scores_exp2(m, kt_i)
                        for _ in range(2):
                            pk, ph, pp = stash.pop(0)
                            emit_av(pk, ph, accs, pp)
                        # last stripe: no more blocks need the backlog, so drain
                        # the lag early (one extra AV per k-tile keeps PE and
                        # ACT balanced) instead of flushing it as pure tail
                        if m == NQC - 1 and len(stash) > 2:
                            pk, ph, pp = stash.pop(0)
                            emit_av(pk, ph, accs, pp)
                        stash.extend(zip((kt_i, kt_i), (0, 1), p01))
                    for pk, ph, pp in stash:
                        emit_av(pk, ph, accs, pp)
                    emit_finish_stripe(m, accs)
                # tail: stripe 2's projection hides under stripe 3's AllToAll
                for cc in range(2):
                    for t in outproj_pieces(NQC - 2, [aTms[NQC - 2]], cc, npiece=1, tag="ps_acc"):
                        t()
                aTm3 = emit_aTm_load(NQC - 1)
                for cc in range(2):
                    for t in outproj_pieces(NQC - 1, [aTm3], cc, npiece=1, tag="ps_acc"):
                        t()

    nc.compile()
    return nc


def _get_nc(mm_mode: str):
    if mm_mode not in _CACHE:
        _CACHE[mm_mode] = _build(mm_mode)
    return _CACHE[mm_mode]


def make_in_maps(x, w_qkv, b_qkv, w_out, b_out):
    import ml_dtypes

    bf16 = ml_dtypes.bfloat16
    x = np.asarray(x, dtype=np.float32)
    w_qkv = np.asarray(w_qkv, dtype=np.float32)
    b_qkv = np.asarray(b_qkv, dtype=np.float32)
    w_out = np.asarray(w_out, dtype=np.float32)
    b_out = np.asarray(b_out, dtype=np.float32)

    xT = x.reshape(N, HIDDEN).T  # [hidden, n]
    # permute n into stripe order n' = (m, j, t) <-> n = 512*j + 128*m + t
    xT = np.ascontiguousarray(
        xT.reshape(HIDDEN, N_CORES, NQC, 128).transpose(0, 2, 1, 3).reshape(HIDDEN, N)
    ).astype(bf16)
    w_out_bf = np.ascontiguousarray(w_out).astype(bf16)
    bo = np.ascontiguousarray(b_out.reshape(1, HIDDEN))
    in_maps = []
    for c in range(N_CORES):
        cs = slice(c * AD, (c + 1) * AD)
        in_maps.append(
            {
                "xT": xT,
                "wq": np.ascontiguousarray(w_qkv[:, :HIDDEN][:, cs]).astype(bf16),
                "wk": np.ascontiguousarray(w_qkv[:, HIDDEN : 2 * HIDDEN][:, cs]).astype(bf16),
                "wv": np.ascontiguousarray(w_qkv[:, 2 * HIDDEN :][:, cs]).astype(bf16),
                "bq": np.ascontiguousarray(b_qkv[:HIDDEN][cs].reshape(AD, 1)),
                "bk": np.ascontiguousarray(b_qkv[HIDDEN : 2 * HIDDEN][cs].reshape(AD, 1)),
                "bvT": np.ascontiguousarray(b_qkv[2 * HIDDEN :][cs].reshape(1, AD)),
                "wo": w_out_bf,
                "bo": bo,
            }
        )
    return in_maps


def kernel(x, w_qkv, b_qkv, w_out, b_out):
    from concourse.bass_utils import run_bass_kernel_spmd

    mm_mode = os.environ.get("TRN_MM_MODE", "bf16")
    nc = _get_nc(mm_mode)
    in_maps = make_in_maps(x, w_qkv, b_qkv, w_out, b_out)
    res = run_bass_kernel_spmd(nc, in_maps, list(range(N_CORES)))
    full = np.concatenate([res.results[c]["out"] for c in range(N_CORES)], axis=0)
    return full.reshape(1, N, HIDDEN).astype(np.float32)



# revision 23
# speedup vs baseline: 2.1450x; 1.0104x over previous
"""Trainium2 Bass kernel for a 16-head dense attention layer (v2, bf16).

Problem: x[1,4096,1024] @ w_qkv[1024,3072] -> 16-head attention (N=4096,
D=64) -> @ w_out[1024,1024].

Sharding: tensor-parallel over heads across 8 NeuronCores (2 heads/core).
Each core computes q/k/v for its 2 heads (weights column-sliced on host),
attention with a fused, max-free softmax (scores are bounded so exp never
overflows in fp32; denominator comes from an appended ones-column in V),
then an AllToAll converts the head-sharded attention output into a
sequence-sharded layout so every core applies the full output projection
to its own 512 rows. Host concatenates the 8 row slices.

vs the f32r baseline: all matmul operands bf16 (PSUM accumulation stays
fp32; rel-err budget 2e-2), V projected directly in [keys, dims] layout
(no PE transposes), merged x DMAs prefetched on the Pool queue, lag-1
attention interleave in phase 1, one AllToAll per stripe, and attention
software-pipelined with AV matmuls lagging 2*PRE_KT (kt,h) units behind
their scores+exp: the ACT engine's exp backlog rides through the
consolidated stripe-boundary blocks (older stripes' output projection
plus the next-next stripe's q projection, allocated from just-freed
ps_acc buffers) without starving, and the last stripe drains its lag
early so it does not flush as pure tail.

Hardware notes (measured via micro-benchmarks on this axon/trn2 stack,
see microprobe.py):
- ACT/DVE instructions whose SOURCE is PSUM largely serialize against
  concurrent PE matmul execution (measured near-additive even on
  disjoint PSUM banks), while SBUF-sourced ACT/DVE work overlaps PE
  fine. The exp stream (PSUM->SBUF) therefore sets a serial floor of
  roughly PE-time + exp-time per k-tile; scheduling can recover only a
  partial (~10-35%) overlap. Fusing both heads into one single-buffered
  [128,2048] exp (score_order="f2") measured ~50us WORSE than "seq" --
  the partial overlap double-buffering enables is worth more than the
  saved per-instruction ACT overhead.
- Adjacent matmuls on disjoint PE row-groups run concurrently
  (tile_position auto-derived from base partitions): an interleaved
  h0/h1 score burst measured 415ns vs 1830ns serial in isolation. In
  the full kernel, however, "seq" (per-head scores immediately followed
  by that head's exp) measured best: the interleaved burst's last MM
  gates the next exp and head-of-line blocks the in-order PE queue.
- repeat>1 replicates the whole body in one NEFF for timing: the
  per-dispatch overhead of this axon client is ~0.8ms (trivial-kernel
  floor), which would otherwise dominate the measurement.
"""

import os
import numpy as np

N_CORES = 8
N = 4096
HIDDEN = 1024
D = 64
HPC = 2  # heads per core
AD = HPC * D  # 128 att-dim rows per core
NT = N // 128  # 32 k-tiles of 128
HT = HIDDEN // 128  # 8 hidden tiles
QCHUNK = 1024
NQC = N // QCHUNK  # 4 q-chunks (stripes)
NSLICE = N // N_CORES  # 512 rows of output per core

_CACHE = {}


def _build(
    mm_mode: str = "bf16",
    skip_a2a: bool = False,
    att_nt: int = NT,
    repeat: int = 1,
    score_order: str = "seq",
    pre_kt: int = 5,
    p_bufs: int = 0,
    av_first: bool = True,
):
    import concourse.bass as bass
    import concourse.mybir as mybir
    import concourse.tile as tile
    from concourse import bacc

    DT = mybir.dt.float32
    DTM = mybir.dt.bfloat16

    AF = mybir.ActivationFunctionType

    nc = bacc.Bacc("TRN2", debug=False, num_devices=N_CORES)

    xT = nc.dram_tensor("xT", [HIDDEN, N], DTM, kind="ExternalInput").ap()
    wq = nc.dram_tensor("wq", [HIDDEN, AD], DTM, kind="ExternalInput").ap()
    wk = nc.dram_tensor("wk", [HIDDEN, AD], DTM, kind="ExternalInput").ap()
    wv = nc.dram_tensor("wv", [HIDDEN, AD], DTM, kind="ExternalInput").ap()
    bq = nc.dram_tensor("bq", [AD, 1], DT, kind="ExternalInput").ap()
    bk = nc.dram_tensor("bk", [AD, 1], DT, kind="ExternalInput").ap()
    bvT = nc.dram_tensor("bvT", [1, AD], DT, kind="ExternalInput").ap()
    wo = nc.dram_tensor("wo", [HIDDEN, HIDDEN], DTM, kind="ExternalInput").ap()
    bo = nc.dram_tensor("bo", [1, HIDDEN], DT, kind="ExternalInput").ap()
    out = nc.dram_tensor("out", [NSLICE, HIDDEN], DT, kind="ExternalOutput").ap()

    with tile.TileContext(nc) as tc:
        with (
            tc.tile_pool(name="sb", bufs=1) as sb,
            tc.tile_pool(name="ps", bufs=2, space="PSUM") as ps,
            tc.tile_pool(name="dram", bufs=1, space="DRAM") as dram,
        ):
            # Global reordering: the sequence axis n is processed in
            # "stripe" order n' = (m, j, t) <-> n = 512*j + 128*m + t
            # (m: stripe 0..3, j: destination core 0..7, t: 0..127).
            # Attention is permutation-invariant in the key axis as long as
            # k and v use the same order, and the q axis just needs the
            # inverse map applied at output -- which the AllToAll block
            # routing does implicitly. Stripe m's attention output IS the
            # m-th out-row-tile of every core, so each stripe's AllToAll +
            # out-projection pipeline behind the next stripe's attention.

            # repeat>1 replicates the whole body inside one NEFF for
            # dispatch-amortized timing; kernel() always uses repeat=1.
            for _rep in range(repeat):
                bvT_sb = sb.tile([1, AD], DT)
                # qkv weights: one DMA each, [1024, 128] folded to [128, 8*128]
                wq_sb = sb.tile([128, HT * AD], DTM)
                wk_sb = sb.tile([128, HT * AD], DTM)
                wv_sb = sb.tile([128, HT * AD], DTM)
                bq_sb = sb.tile([AD, 1], DT)
                bk_sb = sb.tile([AD, 1], DT)
                bv_bc = sb.tile([128, AD], DT)

                def emit_weight_loads():
                    for w_sb, wsrc in ((wq_sb, wq), (wk_sb, wk), (wv_sb, wv)):
                        nc.sync.dma_start(
                            w_sb[:].rearrange("p (a c) -> p a c", a=HT),
                            wsrc.rearrange("(a p) c -> p a c", p=128),
                        )
                    nc.sync.dma_start(bq_sb[:], bq[:])
                    nc.sync.dma_start(bk_sb[:], bk[:])
                    nc.sync.dma_start(bvT_sb[:], bvT[:])
                    nc.gpsimd.partition_broadcast(bv_bc[:], bvT_sb[:1, :])

                def wslice(w_sb, i):
                    return w_sb[:, i * AD : (i + 1) * AD]

                # Host pre-permutes x columns into stripe order n' = (m, j, t),
                # so streaming, qT, kTc, v_nat are all plain contiguous in n'.
                qT = sb.tile([AD, N], DTM)
                kTc = [sb.tile([AD, 512], DTM, name=f"kTc{c}", tag="kTc", bufs=HT) for c in range(HT)]
                att_m = [sb.tile([AD, QCHUNK], DTM, name=f"attm{m}", tag="attm", bufs=NQC) for m in range(NQC)]
                # v in natural [keys, dims] layout: per chunk [128, (j, h, D+1)],
                # ones column at slot D of each head for the softmax denominator.
                v_nat = [
                    sb.tile([128, 4 * HPC * (D + 1)], DTM, name=f"vn{c}", tag="vnat", bufs=HT)
                    for c in range(HT)
                ]
                wo_sb = [sb.tile([128, HIDDEN], DTM, name=f"wo{i}", tag="wo", bufs=HT) for i in range(HT)]
                bo_bc = sb.tile([128, HIDDEN], DT)

                a2a_in = [
                    dram.tile([N_CORES, AD, 128], DTM, name=f"a2ai{m}", tag="a2ai", bufs=NQC)
                    for m in range(NQC)
                ]
                a2a_out = [
                    dram.tile([N_CORES, AD, 128], DTM, name=f"a2ao{m}", tag="a2ao", bufs=NQC)
                    for m in range(NQC)
                ]

                def vn_h(c, j, h):
                    """[128 keys, D+1] slice of chunk c's v for k-tile j, head h."""
                    base = (j * HPC + h) * (D + 1)
                    return v_nat[c][:, base : base + D + 1]

                # Attention runs with AV matmuls lagging 2*PRE_KT (kt,h) units
                # behind their scores+exp: the ACT engine keeps that deep a
                # backlog of materialized exps, so projection blocks occupying
                # the in-order PE stream no longer starve it.
                PRE_KT = pre_kt

                # ---- emission helpers --------------------------------------
                # All non-attention PE work is emitted as small "pieces" (2-4
                # matmuls, ~0.4-0.9us) with DVE partial accumulation, woven
                # between attention (scores+exp+AV) pairs. A long uninterrupted
                # matmul block would stall the in-order PE stream past the ~2
                # tiles of exp backlog the s_ps double-buffer can hold, idling
                # the ACT engine (the overall bottleneck) by its own duration.
                # Pieces are always injected in PAIRS so the number of ps_big
                # allocations between consecutive s_ps allocations stays even
                # and s_ps keeps alternating between its two buffers.
                xts = []

                def emit_xt(cp, eng=None):
                    """x chunk load via the (otherwise idle) Pool queue so the
                    SP queue's weight DMAs never delay it; 8 bufs = fully
                    prefetched, no reuse dependency between chunks (they also
                    stay resident for the deferred q projections)."""
                    cs = slice(cp * 512, (cp + 1) * 512)
                    xt = sb.tile([128, HT * 512], DTM, name="xt", tag="xt", bufs=HT)
                    xts.append(xt)
                    (eng or nc.gpsimd).dma_start(
                        xt[:].rearrange("p (a t) -> p a t", a=HT),
                        xT[:, cs].rearrange("(a p) t -> p a t", p=128),
                    )

                # default PSUM tag for projection pieces: under f2 the score
                # tile is a single-buffered 4-bank [128,2048], and pieces
                # share its slot (PSUM budget: s2 4 + ps_acc 4 = 8 banks)
                PTAG = "s2" if score_order == "f2" else "ps_big"

                def qk_pieces(cp, w_sb, b_sb, dst, npiece, tag=None):
                    """q or k projection for chunk cp as npiece thunks."""
                    tag = tag or PTAG
                    per = HT // npiece
                    thunks = []
                    for pc in range(npiece):
                        def piece(pc=pc):
                            pp = ps.tile([128, 512], DT, name="pp", tag=tag, bufs=1 if tag == "s2" else None)
                            for i in range(per * pc, per * (pc + 1)):
                                nc.tensor.matmul(
                                    pp[:AD, :], wslice(w_sb, i),
                                    xts[cp][:, i * 512 : (i + 1) * 512],
                                    start=(i == per * pc), stop=(i == per * (pc + 1) - 1),
                                )
                            if pc == 0:
                                nc.vector.tensor_scalar_add(dst, pp[:AD, :], b_sb[:])
                            else:
                                nc.vector.tensor_add(dst, dst, pp[:AD, :])
                        thunks.append(piece)
                    return thunks

                def k_pieces(cp):
                    return qk_pieces(cp, wk_sb, bk_sb, kTc[cp][:], 2)

                def q_pieces(cp, npiece=2, tag=None):
                    cs = slice(cp * 512, (cp + 1) * 512)
                    return qk_pieces(cp, wq_sb, bq_sb, qT[:, cs], npiece, tag)

                def v_pieces(cp):
                    """v directly in [keys, dims] layout: x-chunk tile as the
                    stationary operand, wv moving; out partitions are the 128
                    keys of k-tile j. Two thunks of two k-tiles each."""
                    thunks = []
                    for half in (0, 1):
                        def piece(half=half):
                            pv = ps.tile([128, 256], DT, name="pv", tag=PTAG, bufs=1 if PTAG == "s2" else None)
                            for jj in (0, 1):
                                j = 2 * half + jj
                                for i in range(HT):
                                    nc.tensor.matmul(
                                        pv[:, jj * 128 : (jj + 1) * 128],
                                        xts[cp][:, i * 512 + j * 128 : i * 512 + (j + 1) * 128],
                                        wslice(wv_sb, i),
                                        start=(i == 0), stop=(i == HT - 1),
                                    )
                            vn4 = v_nat[cp][:].rearrange("p (j h x) -> p j h x", j=4, x=D + 1)
                            for jj in (0, 1):
                                j = 2 * half + jj
                                nc.vector.tensor_add(
                                    vn4[:, j, :, :D],
                                    pv[:, jj * 128 : (jj + 1) * 128].rearrange(
                                        "p (h d) -> p h d", h=HPC
                                    ),
                                    bv_bc[:].rearrange("p (h d) -> p h d", h=HPC),
                                )
                            nc.vector.memset(
                                vn4[:, 2 * half : 2 * half + 2, :, D : D + 1], 1.0
                            )
                        thunks.append(piece)
                    return thunks

                def emit_scores_exp2(m, kt_i):
                    """Scores + exp for BOTH heads of k-tile kt_i. "f2": one
                    single-buffered [128,2048] PSUM tile holds both heads'
                    scores and ONE exp covers them -- halves the per-
                    instruction ACT overhead; single-buffering costs nothing
                    because ACT PSUM-reads serialize against PE matmul
                    execution on this hardware anyway (measured). The score
                    matmuls interleave h0/h1: the two heads' K=64 contractions
                    sit on row-groups 0-1 (partitions 0-63) and 2-3 (64-127),
                    so adjacent MMs on different row groups run CONCURRENTLY
                    on the PE sub-arrays (tile_position auto-derives from the
                    APs' base partitions)."""

                    def mm_into(dst, h, half):
                        hs = slice(h * D, (h + 1) * D)
                        nc.tensor.matmul(
                            dst,
                            kTc[kt_i // 4][hs, (kt_i % 4) * 128 : (kt_i % 4 + 1) * 128],
                            qT[hs, m * QCHUNK + half * 512 : m * QCHUNK + (half + 1) * 512],
                            start=True, stop=True,
                        )

                    if score_order == "f2":
                        s2 = ps.tile([128, 2 * QCHUNK], DT, name="s2", tag="s2", bufs=1)
                        for h, half in ((0, 0), (1, 0), (1, 1), (0, 1)):
                            base = h * QCHUNK + half * 512
                            mm_into(s2[:, base : base + 512], h, half)
                        p2 = sb.tile(
                            [128, 2 * QCHUNK], DTM, name="p_sb", tag="p_sb", bufs=PRE_KT + 1
                        )
                        nc.scalar.activation(p2[:], s2[:], AF.Exp, scale=0.125)
                        return [p2[:, :QCHUNK], p2[:, QCHUNK:]]

                    s_list = [
                        ps.tile([128, QCHUNK], DT, name=f"s_ps{h}", tag="ps_big")
                        for h in range(HPC)
                    ]

                    def exph(h):
                        p_sb = sb.tile([128, QCHUNK], DTM, name="p_sb", tag="p_sb", bufs=p_bufs or (2 * PRE_KT + 2))
                        nc.scalar.activation(p_sb[:], s_list[h][:], AF.Exp, scale=0.125)
                        return p_sb

                    if score_order == "il":
                        for h, half in ((0, 0), (1, 0), (1, 1), (0, 1)):
                            mm_into(s_list[h][:, half * 512 : (half + 1) * 512], h, half)
                        p_out = [exph(0), exph(1)]
                    else:  # "seq": per-head scores immediately followed by exp
                        mm_into(s_list[0][:, :512], 0, 0)
                        mm_into(s_list[0][:, 512:], 0, 1)
                        p0 = exph(0)
                        mm_into(s_list[1][:, :512], 1, 0)
                        mm_into(s_list[1][:, 512:], 1, 1)
                        p_out = [p0, exph(1)]
                    return p_out

                def emit_av(kt_i, h, accs, p_sb):
                    for half in range(2):
                        hsl = slice(half * 512, (half + 1) * 512)
                        nc.tensor.matmul(
                            accs[h][: D + 1, hsl],
                            vn_h(kt_i // 4, kt_i % 4, h),
                            p_sb[:, hsl],
                            start=(kt_i == 0), stop=(kt_i == att_nt - 1),
                        )

                def emit_finish_stripe(m, accs):
                    # copy both accumulators out of PSUM first so their banks
                    # free for the next stripe while normalization runs on SBUF
                    acc_sbs = []
                    for h in range(HPC):
                        acc_sb = sb.tile([D + 1, QCHUNK], DTM, name="acc_sb", tag="acc_sb", bufs=2)
                        nc.vector.tensor_copy(acc_sb[:], accs[h][: D + 1, :])
                        acc_sbs.append(acc_sb)
                    for h in range(HPC):
                        hs = slice(h * D, (h + 1) * D)
                        acc_sb = acc_sbs[h]
                        recip = sb.tile([1, QCHUNK], DTM, name="recip", tag="recip", bufs=2)
                        with nc.allow_low_precision(reason="softmax denom in bf16; tol 2e-2"):
                            nc.vector.reciprocal(recip[:], acc_sb[D : D + 1, :])
                        bcast = sb.tile([D, QCHUNK], DTM, name="bcast", tag="bcast", bufs=2)
                        nc.gpsimd.partition_broadcast(bcast[:], recip[:1, :])
                        nc.vector.tensor_mul(att_m[m][hs, :], acc_sb[:D, :], bcast[:])
                    nc.sync.dma_start(
                        a2a_in[m][:].rearrange("a p t -> p a t"),
                        att_m[m][:].rearrange("p (a t) -> p a t", a=N_CORES),
                    )
                    if not skip_a2a:
                        nc.gpsimd.collective_compute(
                            "AllToAll",
                            mybir.AluOpType.bypass,
                            replica_groups=[list(range(N_CORES))],
                            ins=[a2a_in[m].opt()],
                            outs=[a2a_out[m].opt()],
                        )

                def emit_aTm_load(m):
                    aTm = sb.tile([128, N_CORES * 128], DTM, name="aTm", tag="aTm", bufs=2)
                    nc.sync.dma_start(
                        aTm[:].rearrange("p (a t) -> p a t", a=N_CORES),
                        a2a_out[m][:].rearrange("a p t -> p a t"),
                    )
                    return aTm

                def outproj_pieces(m, aTm_ref, cc, npiece=4, tag=None):
                    """output projection for stripe m, 512-column half cc, as
                    npiece thunks accumulating into an SBUF tile via DVE."""
                    os_ = slice(cc * 512, (cc + 1) * 512)
                    per = HT // npiece
                    holder = []
                    thunks = []
                    for pc in range(npiece):
                        def piece(pc=pc):
                            po = ps.tile([128, 512], DT, name="po", tag=tag, bufs=1 if tag == "s2" else None)
                            for i in range(per * pc, per * (pc + 1)):
                                nc.tensor.matmul(
                                    po[:], aTm_ref[0][:, i * 128 : (i + 1) * 128],
                                    wo_sb[i][:, os_],
                                    start=(i == per * pc), stop=(i == per * (pc + 1) - 1),
                                )
                            if pc == 0:
                                out_sb = sb.tile([128, 512], DT, name="out_sb", tag="out_sb", bufs=2)
                                holder.append(out_sb)
                                nc.vector.tensor_add(out_sb[:], po[:], bo_bc[:, os_])
                            else:
                                out_sb = holder[0]
                                nc.vector.tensor_add(out_sb[:], out_sb[:], po[:])
                            if pc == npiece - 1:
                                nc.sync.dma_start(out[m * 128 : (m + 1) * 128, os_], out_sb[:])
                        thunks.append(piece)
                    return thunks

                # ---- schedule ----------------------------------------------
                def new_accs():
                    return [
                        ps.tile([128, QCHUNK], DT, name=f"acc{h}", tag="ps_acc")
                        for h in range(HPC)
                    ]

                # phase 1: stream chunks with stripe-0 attention interleaved at
                # lag 1 (chunk cp delivers k-tiles 4cp..4cp+3; attention trails
                # one chunk behind so exp work reaches ACT as early as possible).
                # Phase 1 is PE/supply-bound, so projection blocks sit between
                # attention groups without extra cost.
                # chunk-1's q runs before chunk-0's v so the first scores+exp
                # fire as early as possible
                emit_weight_loads()
                emit_xt(0)
                emit_xt(1)
                for t in k_pieces(0) + q_pieces(0) + q_pieces(1) + v_pieces(0):
                    t()
                # out-proj weights load early on the SP queue (x loads are on
                # Pool, so these only queue behind qkv weights)
                for i in range(HT):
                    nc.sync.dma_start(wo_sb[i][:], wo[i * 128 : (i + 1) * 128, :])
                bo_sb = sb.tile([1, HIDDEN], DT)
                nc.sync.dma_start(bo_sb[:], bo[:])
                nc.gpsimd.partition_broadcast(bo_bc[:], bo_sb[:1, :])

                accs = new_accs()
                stash0 = []

                def unit0kt(kt_i):
                    # AVs first: their p_sb inputs are PRE_KT k-tiles old, so
                    # they are never gated -- emitting them before the (gated)
                    # score matmuls keeps the in-order PE queue busy during
                    # the previous k-tile's exp instead of stalling behind it
                    if av_first and len(stash0) >= 2 * PRE_KT:
                        for _ in range(2):
                            pk, ph, pp = stash0.pop(0)
                            emit_av(pk, ph, accs, pp)
                    stash0.extend(zip((kt_i, kt_i), (0, 1), emit_scores_exp2(0, kt_i)))
                    if not av_first and len(stash0) > 2 * PRE_KT:
                        for _ in range(2):
                            pk, ph, pp = stash0.pop(0)
                            emit_av(pk, ph, accs, pp)

                for cp in range(1, HT):
                    if cp > 1:
                        emit_xt(cp)
                        # only stripe 1's q (chunks 2-3) projects in phase 1;
                        # stripes 2-3's q rides the later stripe boundaries where
                        # the exp backlog absorbs it, shrinking the per-chunk
                        # block that starves ACT here
                        pieces = k_pieces(cp) + (q_pieces(cp) if cp < 4 else []) + v_pieces(cp)
                    else:
                        pieces = k_pieces(cp) + v_pieces(cp)
                    a = 4 * (cp - 1)
                    for kk in range(4):
                        unit0kt(a + kk)
                        # one consolidated projection block per chunk: each
                        # injection site costs a fixed pipeline restart, so
                        # fewer sites beat evenly-spread pieces
                        if kk == 1:
                            for t in pieces:
                                t()
                for kt_i in range(4 * (HT - 1), att_nt):
                    unit0kt(kt_i)
                for pk, ph, pp in stash0:
                    emit_av(pk, ph, accs, pp)
                emit_finish_stripe(0, accs)

                # Stripe k's out-projection runs at the START boundary of stripe
                # k+2: its AllToAll is a full stripe old (no collective wait) and
                # the po blocks allocate from the just-freed ps_acc buffers, so
                # the exp stream's s_ps rotation is never interrupted mid-stripe.
                # The new stripe's first AV matmuls lag behind the po blocks but
                # the p_sb triple-buffer absorbs that.
                aTms = {}
                for m in range(1, NQC):
                    # boundary block: previous-previous stripe's out-projection
                    # plus the q projections for stripe m+1, all allocated from
                    # the just-freed ps_acc buffers so the s_ps rotation is
                    # untouched
                    blocks = []
                    if m >= 2:
                        for cc in range(2):
                            blocks += outproj_pieces(m - 2, [aTms[m - 2]], cc, npiece=1, tag="ps_acc")
                    if m < NQC - 1:
                        blocks += q_pieces(2 * (m + 1), tag="ps_acc")
                        blocks += q_pieces(2 * (m + 1) + 1, tag="ps_acc")
                    # pre-emit PRE_KT k-tiles of scores+exp so ACT stays fed
                    # while the boundary blocks occupy the in-order PE stream;
                    # the whole stripe then runs with AV matmuls lagging
                    # 2*PRE_KT units behind their scores, so the deferred AVs
                    # interleave with new scores instead of bunching up after
                    # the blocks
                    stash = []
                    for kt in range(PRE_KT):
                        stash.extend(zip((kt, kt), (0, 1), emit_scores_exp2(m, kt)))
                    for t in blocks:
                        t()
                    accs = new_accs()
                    for kt_i in range(PRE_KT, att_nt):
                        if kt_i == att_nt - 8:
                            aTms[m - 1] = emit_aTm_load(m - 1)
                        # AVs first (inputs PRE_KT k-tiles old, never gated):
                        # the in-order PE queue stays busy during the previous
                        # k-tile's exp instead of stalling behind the gated
                        # score matmuls
                        p01 = None
                        if not av_first:
                            p01 = emit_scores_exp2(m, kt_i)
                        for _ in range(2):
                            pk, ph, pp = stash.pop(0)
                            emit_av(pk, ph, accs, pp)
                        # last stripe: no more blocks need the backlog, so drain
                        # the lag early (one extra AV per k-tile keeps PE and
                        # ACT balanced) instead of flushing it as pure tail
                        if m == NQC - 1 and len(stash) > 2:
                            pk, ph, pp = stash.pop(0)
                            emit_av(pk, ph, accs, pp)
                        if p01 is None:
                            p01 = emit_scores_exp2(m, kt_i)
                        stash.extend(zip((kt_i, kt_i), (0, 1), p01))
                    for pk, ph, pp in stash:
                        emit_av(pk, ph, accs, pp)
                    emit_finish_stripe(m, accs)
                # tail: stripe 2's projection hides under stripe 3's AllToAll
                for cc in range(2):
                    for t in outproj_pieces(NQC - 2, [aTms[NQC - 2]], cc, npiece=1, tag="ps_acc"):
                        t()
                aTm3 = emit_aTm_load(NQC - 1)
                for cc in range(2):
                    for t in outproj_pieces(NQC - 1, [aTm3], cc, npiece=1, tag="ps_acc"):
                        t()

    nc.compile()
    return nc


def _get_nc(mm_mode: str):
    if mm_mode not in _CACHE:
        _CACHE[mm_mode] = _build(mm_mode)
    return _CACHE[mm_mode]


def make_in_maps(x, w_qkv, b_qkv, w_out, b_out):
    import ml_dtypes

    bf16 = ml_dtypes.bfloat16
    x = np.asarray(x, dtype=np.float32)
    w_qkv = np.asarray(w_qkv, dtype=np.float32)
    b_qkv = np.asarray(b_qkv, dtype=np.float32)
    w_out = np.asarray(w_out, dtype=np.float32)
    b_out = np.asarray(b_out, dtype=np.float32)

    xT = x.reshape(N, HIDDEN).T  # [hidden, n]
    # permute n into stripe order n' = (m, j, t) <-> n = 512*j + 128*m + t
    xT = np.ascontiguousarray(
        xT.reshape(HIDDEN, N_CORES, NQC, 128).transpose(0, 2, 1, 3).reshape(HIDDEN, N)
    ).astype(bf16)
    w_out_bf = np.ascontiguousarray(w_out).astype(bf16)
    bo = np.ascontiguousarray(b_out.reshape(1, HIDDEN))
    in_maps = []
    for c in range(N_CORES):
        cs = slice(c * AD, (c + 1) * AD)
        in_maps.append(
            {
                "xT": xT,
                "wq": np.ascontiguousarray(w_qkv[:, :HIDDEN][:, cs]).astype(bf16),
                "wk": np.ascontiguousarray(w_qkv[:, HIDDEN : 2 * HIDDEN][:, cs]).astype(bf16),
                "wv": np.ascontiguousarray(w_qkv[:, 2 * HIDDEN :][:, cs]).astype(bf16),
                "bq": np.ascontiguousarray(b_qkv[:HIDDEN][cs].reshape(AD, 1)),
                "bk": np.ascontiguousarray(b_qkv[HIDDEN : 2 * HIDDEN][cs].reshape(AD, 1)),
                "bvT": np.ascontiguousarray(b_qkv[2 * HIDDEN :][cs].reshape(1, AD)),
                "wo": w_out_bf,
                "bo": bo,
            }
        )
    return in_maps


def kernel(x, w_qkv, b_qkv, w_out, b_out):
    from concourse.bass_utils import run_bass_kernel_spmd

    mm_mode = os.environ.get("TRN_MM_MODE", "bf16")
    nc = _get_nc(mm_mode)
    in_maps = make_in_maps(x, w_qkv, b_qkv, w_out, b_out)
    res = run_bass_kernel_spmd(nc, in_maps, list(range(N_CORES)))
    full = np.concatenate([res.results[c]["out"] for c in range(N_CORES)], axis=0)
    return full.reshape(1, N, HIDDEN).astype(np.float32)



# revision 24
# speedup vs baseline: 2.1931x; 1.0224x over previous
"""Trainium2 Bass kernel for a 16-head dense attention layer (v2, bf16).

Problem: x[1,4096,1024] @ w_qkv[1024,3072] -> 16-head attention (N=4096,
D=64) -> @ w_out[1024,1024].

Sharding: tensor-parallel over heads across 8 NeuronCores (2 heads/core).
Each core computes q/k/v for its 2 heads (weights column-sliced on host),
attention with a fused, max-free softmax (scores are bounded so exp never
overflows in fp32; denominator comes from an appended ones-column in V),
then an AllToAll converts the head-sharded attention output into a
sequence-sharded layout so every core applies the full output projection
to its own 512 rows. Host concatenates the 8 row slices.

vs the f32r baseline: all matmul operands bf16 (PSUM accumulation stays
fp32; rel-err budget 2e-2), V projected directly in [keys, dims] layout
(no PE transposes), merged x DMAs prefetched on the Pool queue, lag-1
attention interleave in phase 1, one AllToAll per stripe, and attention
software-pipelined with AV matmuls lagging 2*PRE_KT (kt,h) units behind
their scores+exp: the ACT engine's exp backlog rides through the
consolidated stripe-boundary blocks (older stripes' output projection
plus the next-next stripe's q projection, allocated from just-freed
ps_acc buffers) without starving, and the last stripe drains its lag
early so it does not flush as pure tail.

Hardware notes (measured via micro-benchmarks on this axon/trn2 stack,
see microprobe.py):
- ACT/DVE instructions whose SOURCE is PSUM largely serialize against
  concurrent PE matmul execution (measured near-additive even on
  disjoint PSUM banks), while SBUF-sourced ACT/DVE work overlaps PE
  fine. The exp stream (PSUM->SBUF) therefore sets a serial floor of
  roughly PE-time + exp-time per k-tile; scheduling can recover only a
  partial (~10-35%) overlap. Fusing both heads into one single-buffered
  [128,2048] exp (score_order="f2") measured ~50us WORSE than "seq" --
  the partial overlap double-buffering enables is worth more than the
  saved per-instruction ACT overhead.
- Adjacent matmuls on disjoint PE row-groups run concurrently
  (tile_position auto-derived from base partitions): an interleaved
  h0/h1 score burst measured 415ns vs 1830ns serial in isolation. In
  the full kernel, however, "seq" (per-head scores immediately followed
  by that head's exp) measured best: the interleaved burst's last MM
  gates the next exp and head-of-line blocks the in-order PE queue.
- repeat>1 replicates the whole body in one NEFF for timing: the
  per-dispatch overhead of this axon client is ~0.8ms (trivial-kernel
  floor), which would otherwise dominate the measurement.
"""

import os
import numpy as np

N_CORES = 8
N = 4096
HIDDEN = 1024
D = 64
HPC = 2  # heads per core
AD = HPC * D  # 128 att-dim rows per core
NT = N // 128  # 32 k-tiles of 128
HT = HIDDEN // 128  # 8 hidden tiles
QCHUNK = 1024
NQC = N // QCHUNK  # 4 q-chunks (stripes)
NSLICE = N // N_CORES  # 512 rows of output per core

_CACHE = {}


def _build(
    mm_mode: str = "bf16",
    skip_a2a: bool = False,
    att_nt: int = NT,
    repeat: int = 1,
    score_order: str = "seq",
    pre_kt: int = 4,
    p_bufs: int = 0,
    av_first: bool = True,
):
    import concourse.bass as bass
    import concourse.mybir as mybir
    import concourse.tile as tile
    from concourse import bacc

    DT = mybir.dt.float32
    DTM = mybir.dt.bfloat16

    AF = mybir.ActivationFunctionType

    nc = bacc.Bacc("TRN2", debug=False, num_devices=N_CORES)

    xT = nc.dram_tensor("xT", [HIDDEN, N], DTM, kind="ExternalInput").ap()
    wq = nc.dram_tensor("wq", [HIDDEN, AD], DTM, kind="ExternalInput").ap()
    wk = nc.dram_tensor("wk", [HIDDEN, AD], DTM, kind="ExternalInput").ap()
    wv = nc.dram_tensor("wv", [HIDDEN, AD], DTM, kind="ExternalInput").ap()
    bq = nc.dram_tensor("bq", [AD, 1], DT, kind="ExternalInput").ap()
    bk = nc.dram_tensor("bk", [AD, 1], DT, kind="ExternalInput").ap()
    bvT = nc.dram_tensor("bvT", [1, AD], DT, kind="ExternalInput").ap()
    wo = nc.dram_tensor("wo", [HIDDEN, HIDDEN], DTM, kind="ExternalInput").ap()
    bo = nc.dram_tensor("bo", [1, HIDDEN], DT, kind="ExternalInput").ap()
    out = nc.dram_tensor("out", [NSLICE, HIDDEN], DT, kind="ExternalOutput").ap()

    with tile.TileContext(nc) as tc:
        with (
            tc.tile_pool(name="sb", bufs=1) as sb,
            tc.tile_pool(name="ps", bufs=2, space="PSUM") as ps,
            tc.tile_pool(name="dram", bufs=1, space="DRAM") as dram,
        ):
            # Global reordering: the sequence axis n is processed in
            # "stripe" order n' = (m, j, t) <-> n = 512*j + 128*m + t
            # (m: stripe 0..3, j: destination core 0..7, t: 0..127).
            # Attention is permutation-invariant in the key axis as long as
            # k and v use the same order, and the q axis just needs the
            # inverse map applied at output -- which the AllToAll block
            # routing does implicitly. Stripe m's attention output IS the
            # m-th out-row-tile of every core, so each stripe's AllToAll +
            # out-projection pipeline behind the next stripe's attention.

            # repeat>1 replicates the whole body inside one NEFF for
            # dispatch-amortized timing; kernel() always uses repeat=1.
            for _rep in range(repeat):
                bvT_sb = sb.tile([1, AD], DT)
                # qkv weights: one DMA each, [1024, 128] folded to [128, 8*128]
                wq_sb = sb.tile([128, HT * AD], DTM)
                wk_sb = sb.tile([128, HT * AD], DTM)
                wv_sb = sb.tile([128, HT * AD], DTM)
                bq_sb = sb.tile([AD, 1], DT)
                bk_sb = sb.tile([AD, 1], DT)
                bv_bc = sb.tile([128, AD], DT)

                def emit_weight_loads():
                    for w_sb, wsrc in ((wq_sb, wq), (wk_sb, wk), (wv_sb, wv)):
                        nc.sync.dma_start(
                            w_sb[:].rearrange("p (a c) -> p a c", a=HT),
                            wsrc.rearrange("(a p) c -> p a c", p=128),
                        )
                    nc.sync.dma_start(bq_sb[:], bq[:])
                    nc.sync.dma_start(bk_sb[:], bk[:])
                    nc.sync.dma_start(bvT_sb[:], bvT[:])
                    nc.gpsimd.partition_broadcast(bv_bc[:], bvT_sb[:1, :])

                def wslice(w_sb, i):
                    return w_sb[:, i * AD : (i + 1) * AD]

                # Host pre-permutes x columns into stripe order n' = (m, j, t),
                # so streaming, qT, kTc, v_nat are all plain contiguous in n'.
                qT = sb.tile([AD, N], DTM)
                kTc = [sb.tile([AD, 512], DTM, name=f"kTc{c}", tag="kTc", bufs=HT) for c in range(HT)]
                att_m = [sb.tile([AD, QCHUNK], DTM, name=f"attm{m}", tag="attm", bufs=NQC) for m in range(NQC)]
                # v in natural [keys, dims] layout: per chunk [128, (j, h, D+1)],
                # ones column at slot D of each head for the softmax denominator.
                v_nat = [
                    sb.tile([128, 4 * HPC * (D + 1)], DTM, name=f"vn{c}", tag="vnat", bufs=HT)
                    for c in range(HT)
                ]
                wo_sb = [sb.tile([128, HIDDEN], DTM, name=f"wo{i}", tag="wo", bufs=HT) for i in range(HT)]
                bo_bc = sb.tile([128, HIDDEN], DT)

                a2a_in = [
                    dram.tile([N_CORES, AD, 128], DTM, name=f"a2ai{m}", tag="a2ai", bufs=NQC)
                    for m in range(NQC)
                ]
                a2a_out = [
                    dram.tile([N_CORES, AD, 128], DTM, name=f"a2ao{m}", tag="a2ao", bufs=NQC)
                    for m in range(NQC)
                ]

                def vn_h(c, j, h):
                    """[128 keys, D+1] slice of chunk c's v for k-tile j, head h."""
                    base = (j * HPC + h) * (D + 1)
                    return v_nat[c][:, base : base + D + 1]

                # Attention runs with AV matmuls lagging 2*PRE_KT (kt,h) units
                # behind their scores+exp: the ACT engine keeps that deep a
                # backlog of materialized exps, so projection blocks occupying
                # the in-order PE stream no longer starve it.
                PRE_KT = pre_kt

                # ---- emission helpers --------------------------------------
                # All non-attention PE work is emitted as small "pieces" (2-4
                # matmuls, ~0.4-0.9us) with DVE partial accumulation, woven
                # between attention (scores+exp+AV) pairs. A long uninterrupted
                # matmul block would stall the in-order PE stream past the ~2
                # tiles of exp backlog the s_ps double-buffer can hold, idling
                # the ACT engine (the overall bottleneck) by its own duration.
                # Pieces are always injected in PAIRS so the number of ps_big
                # allocations between consecutive s_ps allocations stays even
                # and s_ps keeps alternating between its two buffers.
                xts = []

                def emit_xt(cp, eng=None):
                    """x chunk load via the (otherwise idle) Pool queue so the
                    SP queue's weight DMAs never delay it; 8 bufs = fully
                    prefetched, no reuse dependency between chunks (they also
                    stay resident for the deferred q projections)."""
                    cs = slice(cp * 512, (cp + 1) * 512)
                    xt = sb.tile([128, HT * 512], DTM, name="xt", tag="xt", bufs=HT)
                    xts.append(xt)
                    (eng or nc.gpsimd).dma_start(
                        xt[:].rearrange("p (a t) -> p a t", a=HT),
                        xT[:, cs].rearrange("(a p) t -> p a t", p=128),
                    )

                # default PSUM tag for projection pieces: under f2 the score
                # tile is a single-buffered 4-bank [128,2048], and pieces
                # share its slot (PSUM budget: s2 4 + ps_acc 4 = 8 banks)
                PTAG = "s2" if score_order == "f2" else "ps_big"

                def qk_pieces(cp, w_sb, b_sb, dst, npiece, tag=None):
                    """q or k projection for chunk cp as npiece thunks."""
                    tag = tag or PTAG
                    per = HT // npiece
                    thunks = []
                    for pc in range(npiece):
                        def piece(pc=pc):
                            pp = ps.tile([128, 512], DT, name="pp", tag=tag, bufs=1 if tag == "s2" else None)
                            for i in range(per * pc, per * (pc + 1)):
                                nc.tensor.matmul(
                                    pp[:AD, :], wslice(w_sb, i),
                                    xts[cp][:, i * 512 : (i + 1) * 512],
                                    start=(i == per * pc), stop=(i == per * (pc + 1) - 1),
                                )
                            if pc == 0:
                                nc.vector.tensor_scalar_add(dst, pp[:AD, :], b_sb[:])
                            else:
                                nc.vector.tensor_add(dst, dst, pp[:AD, :])
                        thunks.append(piece)
                    return thunks

                def k_pieces(cp):
                    return qk_pieces(cp, wk_sb, bk_sb, kTc[cp][:], 2)

                def q_pieces(cp, npiece=2, tag=None):
                    cs = slice(cp * 512, (cp + 1) * 512)
                    return qk_pieces(cp, wq_sb, bq_sb, qT[:, cs], npiece, tag)

                def v_pieces(cp):
                    """v directly in [keys, dims] layout: x-chunk tile as the
                    stationary operand, wv moving; out partitions are the 128
                    keys of k-tile j. Two thunks of two k-tiles each."""
                    thunks = []
                    for half in (0, 1):
                        def piece(half=half):
                            pv = ps.tile([128, 256], DT, name="pv", tag=PTAG, bufs=1 if PTAG == "s2" else None)
                            for jj in (0, 1):
                                j = 2 * half + jj
                                for i in range(HT):
                                    nc.tensor.matmul(
                                        pv[:, jj * 128 : (jj + 1) * 128],
                                        xts[cp][:, i * 512 + j * 128 : i * 512 + (j + 1) * 128],
                                        wslice(wv_sb, i),
                                        start=(i == 0), stop=(i == HT - 1),
                                    )
                            vn4 = v_nat[cp][:].rearrange("p (j h x) -> p j h x", j=4, x=D + 1)
                            for jj in (0, 1):
                                j = 2 * half + jj
                                nc.vector.tensor_add(
                                    vn4[:, j, :, :D],
                                    pv[:, jj * 128 : (jj + 1) * 128].rearrange(
                                        "p (h d) -> p h d", h=HPC
                                    ),
                                    bv_bc[:].rearrange("p (h d) -> p h d", h=HPC),
                                )
                            nc.vector.memset(
                                vn4[:, 2 * half : 2 * half + 2, :, D : D + 1], 1.0
                            )
                        thunks.append(piece)
                    return thunks

                def emit_scores_exp2(m, kt_i):
                    """Scores + exp for BOTH heads of k-tile kt_i. "f2": one
                    single-buffered [128,2048] PSUM tile holds both heads'
                    scores and ONE exp covers them -- halves the per-
                    instruction ACT overhead; single-buffering costs nothing
                    because ACT PSUM-reads serialize against PE matmul
                    execution on this hardware anyway (measured). The score
                    matmuls interleave h0/h1: the two heads' K=64 contractions
                    sit on row-groups 0-1 (partitions 0-63) and 2-3 (64-127),
                    so adjacent MMs on different row groups run CONCURRENTLY
                    on the PE sub-arrays (tile_position auto-derives from the
                    APs' base partitions)."""

                    def mm_into(dst, h, half):
                        hs = slice(h * D, (h + 1) * D)
                        nc.tensor.matmul(
                            dst,
                            kTc[kt_i // 4][hs, (kt_i % 4) * 128 : (kt_i % 4 + 1) * 128],
                            qT[hs, m * QCHUNK + half * 512 : m * QCHUNK + (half + 1) * 512],
                            start=True, stop=True,
                        )

                    if score_order == "f2":
                        s2 = ps.tile([128, 2 * QCHUNK], DT, name="s2", tag="s2", bufs=1)
                        for h, half in ((0, 0), (1, 0), (1, 1), (0, 1)):
                            base = h * QCHUNK + half * 512
                            mm_into(s2[:, base : base + 512], h, half)
                        p2 = sb.tile(
                            [128, 2 * QCHUNK], DTM, name="p_sb", tag="p_sb", bufs=PRE_KT + 1
                        )
                        nc.scalar.activation(p2[:], s2[:], AF.Exp, scale=0.125)
                        return [p2[:, :QCHUNK], p2[:, QCHUNK:]]

                    s_list = [
                        ps.tile([128, QCHUNK], DT, name=f"s_ps{h}", tag="ps_big")
                        for h in range(HPC)
                    ]

                    def exph(h):
                        p_sb = sb.tile([128, QCHUNK], DTM, name="p_sb", tag="p_sb", bufs=p_bufs or (2 * PRE_KT + 2))
                        nc.scalar.activation(p_sb[:], s_list[h][:], AF.Exp, scale=0.125)
                        return p_sb

                    if score_order == "il":
                        for h, half in ((0, 0), (1, 0), (1, 1), (0, 1)):
                            mm_into(s_list[h][:, half * 512 : (half + 1) * 512], h, half)
                        p_out = [exph(0), exph(1)]
                    else:  # "seq": per-head scores immediately followed by exp
                        mm_into(s_list[0][:, :512], 0, 0)
                        mm_into(s_list[0][:, 512:], 0, 1)
                        p0 = exph(0)
                        mm_into(s_list[1][:, :512], 1, 0)
                        mm_into(s_list[1][:, 512:], 1, 1)
                        p_out = [p0, exph(1)]
                    return p_out

                def emit_av(kt_i, h, accs, p_sb):
                    for half in range(2):
                        hsl = slice(half * 512, (half + 1) * 512)
                        nc.tensor.matmul(
                            accs[h][: D + 1, hsl],
                            vn_h(kt_i // 4, kt_i % 4, h),
                            p_sb[:, hsl],
                            start=(kt_i == 0), stop=(kt_i == att_nt - 1),
                        )

                def emit_finish_stripe(m, accs):
                    # copy both accumulators out of PSUM first so their banks
                    # free for the next stripe while normalization runs on SBUF
                    acc_sbs = []
                    for h in range(HPC):
                        acc_sb = sb.tile([D + 1, QCHUNK], DTM, name="acc_sb", tag="acc_sb", bufs=2)
                        nc.vector.tensor_copy(acc_sb[:], accs[h][: D + 1, :])
                        acc_sbs.append(acc_sb)
                    for h in range(HPC):
                        hs = slice(h * D, (h + 1) * D)
                        acc_sb = acc_sbs[h]
                        recip = sb.tile([1, QCHUNK], DTM, name="recip", tag="recip", bufs=2)
                        with nc.allow_low_precision(reason="softmax denom in bf16; tol 2e-2"):
                            nc.vector.reciprocal(recip[:], acc_sb[D : D + 1, :])
                        bcast = sb.tile([D, QCHUNK], DTM, name="bcast", tag="bcast", bufs=2)
                        nc.gpsimd.partition_broadcast(bcast[:], recip[:1, :])
                        nc.vector.tensor_mul(att_m[m][hs, :], acc_sb[:D, :], bcast[:])
                    nc.sync.dma_start(
                        a2a_in[m][:].rearrange("a p t -> p a t"),
                        att_m[m][:].rearrange("p (a t) -> p a t", a=N_CORES),
                    )
                    if not skip_a2a:
                        nc.gpsimd.collective_compute(
                            "AllToAll",
                            mybir.AluOpType.bypass,
                            replica_groups=[list(range(N_CORES))],
                            ins=[a2a_in[m].opt()],
                            outs=[a2a_out[m].opt()],
                        )

                def emit_aTm_load(m):
                    aTm = sb.tile([128, N_CORES * 128], DTM, name="aTm", tag="aTm", bufs=2)
                    nc.sync.dma_start(
                        aTm[:].rearrange("p (a t) -> p a t", a=N_CORES),
                        a2a_out[m][:].rearrange("a p t -> p a t"),
                    )
                    return aTm

                def outproj_pieces(m, aTm_ref, cc, npiece=4, tag=None):
                    """output projection for stripe m, 512-column half cc, as
                    npiece thunks accumulating into an SBUF tile via DVE."""
                    os_ = slice(cc * 512, (cc + 1) * 512)
                    per = HT // npiece
                    holder = []
                    thunks = []
                    for pc in range(npiece):
                        def piece(pc=pc):
                            po = ps.tile([128, 512], DT, name="po", tag=tag, bufs=1 if tag == "s2" else None)
                            for i in range(per * pc, per * (pc + 1)):
                                nc.tensor.matmul(
                                    po[:], aTm_ref[0][:, i * 128 : (i + 1) * 128],
                                    wo_sb[i][:, os_],
                                    start=(i == per * pc), stop=(i == per * (pc + 1) - 1),
                                )
                            if pc == 0:
                                out_sb = sb.tile([128, 512], DT, name="out_sb", tag="out_sb", bufs=2)
                                holder.append(out_sb)
                                nc.vector.tensor_add(out_sb[:], po[:], bo_bc[:, os_])
                            else:
                                out_sb = holder[0]
                                nc.vector.tensor_add(out_sb[:], out_sb[:], po[:])
                            if pc == npiece - 1:
                                nc.sync.dma_start(out[m * 128 : (m + 1) * 128, os_], out_sb[:])
                        thunks.append(piece)
                    return thunks

                # ---- schedule ----------------------------------------------
                def new_accs():
                    return [
                        ps.tile([128, QCHUNK], DT, name=f"acc{h}", tag="ps_acc")
                        for h in range(HPC)
                    ]

                # phase 1: stream chunks with stripe-0 attention interleaved at
                # lag 1 (chunk cp delivers k-tiles 4cp..4cp+3; attention trails
                # one chunk behind so exp work reaches ACT as early as possible).
                # Phase 1 is PE/supply-bound, so projection blocks sit between
                # attention groups without extra cost.
                # chunk-1's q runs before chunk-0's v so the first scores+exp
                # fire as early as possible
                emit_weight_loads()
                emit_xt(0)
                emit_xt(1)
                for t in k_pieces(0) + q_pieces(0) + q_pieces(1) + v_pieces(0):
                    t()
                # out-proj weights load early on the SP queue (x loads are on
                # Pool, so these only queue behind qkv weights)
                for i in range(HT):
                    nc.sync.dma_start(wo_sb[i][:], wo[i * 128 : (i + 1) * 128, :])
                bo_sb = sb.tile([1, HIDDEN], DT)
                nc.sync.dma_start(bo_sb[:], bo[:])
                nc.gpsimd.partition_broadcast(bo_bc[:], bo_sb[:1, :])

                accs = new_accs()
                stash0 = []

                def unit0kt(kt_i):
                    # AVs first: their p_sb inputs are PRE_KT k-tiles old, so
                    # they are never gated -- emitting them before the (gated)
                    # score matmuls keeps the in-order PE queue busy during
                    # the previous k-tile's exp instead of stalling behind it
                    if av_first and len(stash0) >= 2 * PRE_KT:
                        for _ in range(2):
                            pk, ph, pp = stash0.pop(0)
                            emit_av(pk, ph, accs, pp)
                    stash0.extend(zip((kt_i, kt_i), (0, 1), emit_scores_exp2(0, kt_i)))
                    if not av_first and len(stash0) > 2 * PRE_KT:
                        for _ in range(2):
                            pk, ph, pp = stash0.pop(0)
                            emit_av(pk, ph, accs, pp)

                for cp in range(1, HT):
                    if cp > 1:
                        emit_xt(cp)
                        # only stripe 1's q (chunks 2-3) projects in phase 1;
                        # stripes 2-3's q rides the later stripe boundaries where
                        # the exp backlog absorbs it, shrinking the per-chunk
                        # block that starves ACT here
                        pieces = k_pieces(cp) + (q_pieces(cp) if cp < 4 else []) + v_pieces(cp)
                    else:
                        pieces = k_pieces(cp) + v_pieces(cp)
                    a = 4 * (cp - 1)
                    for kk in range(4):
                        unit0kt(a + kk)
                        # one consolidated projection block per chunk: each
                        # injection site costs a fixed pipeline restart, so
                        # fewer sites beat evenly-spread pieces
                        if kk == 1:
                            for t in pieces:
                                t()
                for kt_i in range(4 * (HT - 1), att_nt):
                    unit0kt(kt_i)
                for pk, ph, pp in stash0:
                    emit_av(pk, ph, accs, pp)
                emit_finish_stripe(0, accs)

                # Stripe k's out-projection runs at the START boundary of stripe
                # k+2: its AllToAll is a full stripe old (no collective wait) and
                # the po blocks allocate from the just-freed ps_acc buffers, so
                # the exp stream's s_ps rotation is never interrupted mid-stripe.
                # The new stripe's first AV matmuls lag behind the po blocks but
                # the p_sb triple-buffer absorbs that.
                aTms = {}
                for m in range(1, NQC):
                    # boundary block: previous-previous stripe's out-projection
                    # plus the q projections for stripe m+1, all allocated from
                    # the just-freed ps_acc buffers so the s_ps rotation is
                    # untouched
                    blocks = []
                    if m >= 2:
                        for cc in range(2):
                            blocks += outproj_pieces(m - 2, [aTms[m - 2]], cc, npiece=1, tag="ps_acc")
                    if m < NQC - 1:
                        blocks += q_pieces(2 * (m + 1), tag="ps_acc")
                        blocks += q_pieces(2 * (m + 1) + 1, tag="ps_acc")
                    # pre-emit PRE_KT k-tiles of scores+exp so ACT stays fed
                    # while the boundary blocks occupy the in-order PE stream;
                    # the whole stripe then runs with AV matmuls lagging
                    # 2*PRE_KT units behind their scores, so the deferred AVs
                    # interleave with new scores instead of bunching up after
                    # the blocks
                    stash = []
                    for kt in range(PRE_KT):
                        stash.extend(zip((kt, kt), (0, 1), emit_scores_exp2(m, kt)))
                    for t in blocks:
                        t()
                    accs = new_accs()
                    for kt_i in range(PRE_KT, att_nt):
                        if kt_i == att_nt - 8:
                            aTms[m - 1] = emit_aTm_load(m - 1)
                        # AVs first (inputs PRE_KT k-tiles old, never gated):
                        # the in-order PE queue stays busy during the previous
                        # k-tile's exp instead of stalling behind the gated
                        # score matmuls
                        p01 = None
                        if not av_first:
                            p01 = emit_scores_exp2(m, kt_i)
                        for _ in range(2):
                            pk, ph, pp = stash.pop(0)
                            emit_av(pk, ph, accs, pp)
                        # last stripe: no more blocks need the backlog, so drain
                        # the lag early (one extra AV per k-tile keeps PE and
                        # ACT balanced) instead of flushing it as pure tail
                        if m == NQC - 1 and len(stash) > 2:
                            pk, ph, pp = stash.pop(0)
                            emit_av(pk, ph, accs, pp)
                        if p01 is None:
                            p01 = emit_scores_exp2(m, kt_i)
                        stash.extend(zip((kt_i, kt_i), (0, 1), p01))
                    for pk, ph, pp in stash:
                        emit_av(pk, ph, accs, pp)
                    emit_finish_stripe(m, accs)
                # tail: stripe 2's projection hides under stripe 3's AllToAll
                for cc in range(2):
                    for t in outproj_pieces(NQC - 2, [aTms[NQC - 2]], cc, npiece=1, tag="ps_acc"):
                        t()
                aTm3 = emit_aTm_load(NQC - 1)
                for cc in range(2):
                    for t in outproj_pieces(NQC - 1, [aTm3], cc, npiece=1, tag="ps_acc"):
                        t()

    nc.compile()
    return nc


def _get_nc(mm_mode: str):
    if mm_mode not in _CACHE:
        _CACHE[mm_mode] = _build(mm_mode)
    return _CACHE[mm_mode]


def make_in_maps(x, w_qkv, b_qkv, w_out, b_out):
    import ml_dtypes

    bf16 = ml_dtypes.bfloat16
    x = np.asarray(x, dtype=np.float32)
    w_qkv = np.asarray(w_qkv, dtype=np.float32)
    b_qkv = np.asarray(b_qkv, dtype=np.float32)
    w_out = np.asarray(w_out, dtype=np.float32)
    b_out = np.asarray(b_out, dtype=np.float32)

    xT = x.reshape(N, HIDDEN).T  # [hidden, n]
    # permute n into stripe order n' = (m, j, t) <-> n = 512*j + 128*m + t
    xT = np.ascontiguousarray(
        xT.reshape(HIDDEN, N_CORES, NQC, 128).transpose(0, 2, 1, 3).reshape(HIDDEN, N)
    ).astype(bf16)
    w_out_bf = np.ascontiguousarray(w_out).astype(bf16)
    bo = np.ascontiguousarray(b_out.reshape(1, HIDDEN))
    in_maps = []
    for c in range(N_CORES):
        cs = slice(c * AD, (c + 1) * AD)
        in_maps.append(
            {
                "xT": xT,
                "wq": np.ascontiguousarray(w_qkv[:, :HIDDEN][:, cs]).astype(bf16),
                "wk": np.ascontiguousarray(w_qkv[:, HIDDEN : 2 * HIDDEN][:, cs]).astype(bf16),
                "wv": np.ascontiguousarray(w_qkv[:, 2 * HIDDEN :][:, cs]).astype(bf16),
                "bq": np.ascontiguousarray(b_qkv[:HIDDEN][cs].reshape(AD, 1)),
                "bk": np.ascontiguousarray(b_qkv[HIDDEN : 2 * HIDDEN][cs].reshape(AD, 1)),
                "bvT": np.ascontiguousarray(b_qkv[2 * HIDDEN :][cs].reshape(1, AD)),
                "wo": w_out_bf,
                "bo": bo,
            }
        )
    return in_maps


def kernel(x, w_qkv, b_qkv, w_out, b_out):
    from concourse.bass_utils import run_bass_kernel_spmd

    mm_mode = os.environ.get("TRN_MM_MODE", "bf16")
    nc = _get_nc(mm_mode)
    in_maps = make_in_maps(x, w_qkv, b_qkv, w_out, b_out)
    res = run_bass_kernel_spmd(nc, in_maps, list(range(N_CORES)))
    full = np.concatenate([res.results[c]["out"] for c in range(N_CORES)], axis=0)
    return full.reshape(1, N, HIDDEN).astype(np.float32)



# revision 26
# speedup vs baseline: 2.2065x; 1.0061x over previous
"""Trainium2 Bass kernel for a 16-head dense attention layer (v2, bf16).

Problem: x[1,4096,1024] @ w_qkv[1024,3072] -> 16-head attention (N=4096,
D=64) -> @ w_out[1024,1024].

Sharding: tensor-parallel over heads across 8 NeuronCores (2 heads/core).
Each core computes q/k/v for its 2 heads (weights column-sliced on host),
attention with a fused, max-free softmax (scores are bounded so exp never
overflows in fp32; denominator comes from an appended ones-column in V),
then an AllToAll converts the head-sharded attention output into a
sequence-sharded layout so every core applies the full output projection
to its own 512 rows. Host concatenates the 8 row slices.

vs the f32r baseline: all matmul operands bf16 (PSUM accumulation stays
fp32; rel-err budget 2e-2), V projected directly in [keys, dims] layout
(no PE transposes), merged x DMAs prefetched on the Pool queue, lag-1
attention interleave in phase 1, one AllToAll per stripe, and attention
software-pipelined with AV matmuls lagging 2*PRE_KT (kt,h) units behind
their scores+exp: the ACT engine's exp backlog rides through the
consolidated stripe-boundary blocks (older stripes' output projection
plus the next-next stripe's q projection, allocated from just-freed
ps_acc buffers) without starving, and the last stripe drains its lag
early so it does not flush as pure tail.

Hardware notes (measured via micro-benchmarks on this axon/trn2 stack,
see microprobe.py):
- ACT/DVE instructions whose SOURCE is PSUM largely serialize against
  concurrent PE matmul execution (measured near-additive even on
  disjoint PSUM banks), while SBUF-sourced ACT/DVE work overlaps PE
  fine. The exp stream (PSUM->SBUF) therefore sets a serial floor of
  roughly PE-time + exp-time per k-tile; scheduling can recover only a
  partial (~10-35%) overlap. Fusing both heads into one single-buffered
  [128,2048] exp (score_order="f2") measured ~50us WORSE than "seq" --
  the partial overlap double-buffering enables is worth more than the
  saved per-instruction ACT overhead.
- Adjacent matmuls on disjoint PE row-groups run concurrently
  (tile_position auto-derived from base partitions): an interleaved
  h0/h1 score burst measured 415ns vs 1830ns serial in isolation. In
  the full kernel, however, "seq" (per-head scores immediately followed
  by that head's exp) measured best: the interleaved burst's last MM
  gates the next exp and head-of-line blocks the in-order PE queue.
- repeat>1 replicates the whole body in one NEFF for timing: the
  per-dispatch overhead of this axon client is ~0.8ms (trivial-kernel
  floor), which would otherwise dominate the measurement.
"""

import os
import numpy as np

N_CORES = 8
N = 4096
HIDDEN = 1024
D = 64
HPC = 2  # heads per core
AD = HPC * D  # 128 att-dim rows per core
NT = N // 128  # 32 k-tiles of 128
HT = HIDDEN // 128  # 8 hidden tiles
QCHUNK = 1024
NQC = N // QCHUNK  # 4 q-chunks (stripes)
NSLICE = N // N_CORES  # 512 rows of output per core

_CACHE = {}


def _build(
    mm_mode: str = "bf16",
    skip_a2a: bool = False,
    att_nt: int = NT,
    repeat: int = 1,
    score_order: str = "seq",
    pre_kt: int = 4,
    p_bufs: int = 0,
    av_first: bool = True,
):
    import concourse.bass as bass
    import concourse.mybir as mybir
    import concourse.tile as tile
    from concourse import bacc

    DT = mybir.dt.float32
    DTM = mybir.dt.bfloat16

    AF = mybir.ActivationFunctionType

    nc = bacc.Bacc("TRN2", debug=False, num_devices=N_CORES)

    xT = nc.dram_tensor("xT", [HIDDEN, N], DTM, kind="ExternalInput").ap()
    wq = nc.dram_tensor("wq", [HIDDEN, AD], DTM, kind="ExternalInput").ap()
    wk = nc.dram_tensor("wk", [HIDDEN, AD], DTM, kind="ExternalInput").ap()
    wv = nc.dram_tensor("wv", [HIDDEN, AD], DTM, kind="ExternalInput").ap()
    bq = nc.dram_tensor("bq", [AD, 1], DT, kind="ExternalInput").ap()
    bk = nc.dram_tensor("bk", [AD, 1], DT, kind="ExternalInput").ap()
    bvT = nc.dram_tensor("bvT", [1, AD], DT, kind="ExternalInput").ap()
    wo = nc.dram_tensor("wo", [HIDDEN, HIDDEN], DTM, kind="ExternalInput").ap()
    bo = nc.dram_tensor("bo", [1, HIDDEN], DT, kind="ExternalInput").ap()
    out = nc.dram_tensor("out", [NSLICE, HIDDEN], DT, kind="ExternalOutput").ap()

    with tile.TileContext(nc) as tc:
        with (
            tc.tile_pool(name="sb", bufs=1) as sb,
            tc.tile_pool(name="ps", bufs=2, space="PSUM") as ps,
            tc.tile_pool(name="dram", bufs=1, space="DRAM") as dram,
        ):
            # Global reordering: the sequence axis n is processed in
            # "stripe" order n' = (m, j, t) <-> n = 512*j + 128*m + t
            # (m: stripe 0..3, j: destination core 0..7, t: 0..127).
            # Attention is permutation-invariant in the key axis as long as
            # k and v use the same order, and the q axis just needs the
            # inverse map applied at output -- which the AllToAll block
            # routing does implicitly. Stripe m's attention output IS the
            # m-th out-row-tile of every core, so each stripe's AllToAll +
            # out-projection pipeline behind the next stripe's attention.

            # repeat>1 replicates the whole body inside one NEFF for
            # dispatch-amortized timing; kernel() always uses repeat=1.
            for _rep in range(repeat):
                bvT_sb = sb.tile([1, AD], DT)
                # qkv weights: one DMA each, [1024, 128] folded to [128, 8*128]
                wq_sb = sb.tile([128, HT * AD], DTM)
                wk_sb = sb.tile([128, HT * AD], DTM)
                wv_sb = sb.tile([128, HT * AD], DTM)
                bq_sb = sb.tile([AD, 1], DT)
                bk_sb = sb.tile([AD, 1], DT)
                bv_bc = sb.tile([128, AD], DT)

                def emit_weight_loads():
                    for w_sb, wsrc in ((wq_sb, wq), (wk_sb, wk), (wv_sb, wv)):
                        nc.sync.dma_start(
                            w_sb[:].rearrange("p (a c) -> p a c", a=HT),
                            wsrc.rearrange("(a p) c -> p a c", p=128),
                        )
                    nc.sync.dma_start(bq_sb[:], bq[:])
                    nc.sync.dma_start(bk_sb[:], bk[:])
                    nc.sync.dma_start(bvT_sb[:], bvT[:])
                    nc.gpsimd.partition_broadcast(bv_bc[:], bvT_sb[:1, :])

                def wslice(w_sb, i):
                    return w_sb[:, i * AD : (i + 1) * AD]

                # Host pre-permutes x columns into stripe order n' = (m, j, t),
                # so streaming, qT, kTc, v_nat are all plain contiguous in n'.
                qT = sb.tile([AD, N], DTM)
                kTc = [sb.tile([AD, 512], DTM, name=f"kTc{c}", tag="kTc", bufs=HT) for c in range(HT)]
                att_m = [sb.tile([AD, QCHUNK], DTM, name=f"attm{m}", tag="attm", bufs=NQC) for m in range(NQC)]
                # v in natural [keys, dims] layout: per chunk [128, (j, h, D+1)],
                # ones column at slot D of each head for the softmax denominator.
                v_nat = [
                    sb.tile([128, 4 * HPC * (D + 1)], DTM, name=f"vn{c}", tag="vnat", bufs=HT)
                    for c in range(HT)
                ]
                wo_sb = [sb.tile([128, HIDDEN], DTM, name=f"wo{i}", tag="wo", bufs=HT) for i in range(HT)]
                bo_bc = sb.tile([128, HIDDEN], DT)

                a2a_in = [
                    dram.tile([N_CORES, AD, 128], DTM, name=f"a2ai{m}", tag="a2ai", bufs=NQC)
                    for m in range(NQC)
                ]
                a2a_out = [
                    dram.tile([N_CORES, AD, 128], DTM, name=f"a2ao{m}", tag="a2ao", bufs=NQC)
                    for m in range(NQC)
                ]

                def vn_h(c, j, h):
                    """[128 keys, D+1] slice of chunk c's v for k-tile j, head h."""
                    base = (j * HPC + h) * (D + 1)
                    return v_nat[c][:, base : base + D + 1]

                # Attention runs with AV matmuls lagging 2*PRE_KT (kt,h) units
                # behind their scores+exp: the ACT engine keeps that deep a
                # backlog of materialized exps, so projection blocks occupying
                # the in-order PE stream no longer starve it.
                PRE_KT = pre_kt

                # ---- emission helpers --------------------------------------
                # All non-attention PE work is emitted as small "pieces" (2-4
                # matmuls, ~0.4-0.9us) with DVE partial accumulation, woven
                # between attention (scores+exp+AV) pairs. A long uninterrupted
                # matmul block would stall the in-order PE stream past the ~2
                # tiles of exp backlog the s_ps double-buffer can hold, idling
                # the ACT engine (the overall bottleneck) by its own duration.
                # Pieces are always injected in PAIRS so the number of ps_big
                # allocations between consecutive s_ps allocations stays even
                # and s_ps keeps alternating between its two buffers.
                xts = []

                def emit_xt(cp, eng=None):
                    """x chunk load via the (otherwise idle) Pool queue so the
                    SP queue's weight DMAs never delay it; 8 bufs = fully
                    prefetched, no reuse dependency between chunks (they also
                    stay resident for the deferred q projections)."""
                    cs = slice(cp * 512, (cp + 1) * 512)
                    xt = sb.tile([128, HT * 512], DTM, name="xt", tag="xt", bufs=HT)
                    xts.append(xt)
                    (eng or nc.gpsimd).dma_start(
                        xt[:].rearrange("p (a t) -> p a t", a=HT),
                        xT[:, cs].rearrange("(a p) t -> p a t", p=128),
                    )

                # default PSUM tag for projection pieces: under f2 the score
                # tile is a single-buffered 4-bank [128,2048], and pieces
                # share its slot (PSUM budget: s2 4 + ps_acc 4 = 8 banks)
                PTAG = "s2" if score_order == "f2" else "ps_big"

                def qk_pieces(cp, w_sb, b_sb, dst, npiece, tag=None):
                    """q or k projection for chunk cp as npiece thunks."""
                    tag = tag or PTAG
                    per = HT // npiece
                    thunks = []
                    for pc in range(npiece):
                        def piece(pc=pc):
                            pp = ps.tile([128, 512], DT, name="pp", tag=tag, bufs=1 if tag == "s2" else None)
                            for i in range(per * pc, per * (pc + 1)):
                                nc.tensor.matmul(
                                    pp[:AD, :], wslice(w_sb, i),
                                    xts[cp][:, i * 512 : (i + 1) * 512],
                                    start=(i == per * pc), stop=(i == per * (pc + 1) - 1),
                                )
                            if pc == 0:
                                nc.vector.tensor_scalar_add(dst, pp[:AD, :], b_sb[:])
                            else:
                                nc.vector.tensor_add(dst, dst, pp[:AD, :])
                        thunks.append(piece)
                    return thunks

                def k_pieces(cp):
                    return qk_pieces(cp, wk_sb, bk_sb, kTc[cp][:], 2)

                def q_pieces(cp, npiece=2, tag=None):
                    cs = slice(cp * 512, (cp + 1) * 512)
                    return qk_pieces(cp, wq_sb, bq_sb, qT[:, cs], npiece, tag)

                def v_pieces(cp):
                    """v directly in [keys, dims] layout: x-chunk tile as the
                    stationary operand, wv moving; out partitions are the 128
                    keys of k-tile j. Two thunks of two k-tiles each."""
                    thunks = []
                    for half in (0, 1):
                        def piece(half=half):
                            pv = ps.tile([128, 256], DT, name="pv", tag=PTAG, bufs=1 if PTAG == "s2" else None)
                            for jj in (0, 1):
                                j = 2 * half + jj
                                for i in range(HT):
                                    nc.tensor.matmul(
                                        pv[:, jj * 128 : (jj + 1) * 128],
                                        xts[cp][:, i * 512 + j * 128 : i * 512 + (j + 1) * 128],
                                        wslice(wv_sb, i),
                                        start=(i == 0), stop=(i == HT - 1),
                                    )
                            vn4 = v_nat[cp][:].rearrange("p (j h x) -> p j h x", j=4, x=D + 1)
                            for jj in (0, 1):
                                j = 2 * half + jj
                                nc.vector.tensor_add(
                                    vn4[:, j, :, :D],
                                    pv[:, jj * 128 : (jj + 1) * 128].rearrange(
                                        "p (h d) -> p h d", h=HPC
                                    ),
                                    bv_bc[:].rearrange("p (h d) -> p h d", h=HPC),
                                )
                            nc.vector.memset(
                                vn4[:, 2 * half : 2 * half + 2, :, D : D + 1], 1.0
                            )
                        thunks.append(piece)
                    return thunks

                def emit_scores_exp2(m, kt_i):
                    """Scores + exp for BOTH heads of k-tile kt_i. "f2": one
                    single-buffered [128,2048] PSUM tile holds both heads'
                    scores and ONE exp covers them -- halves the per-
                    instruction ACT overhead; single-buffering costs nothing
                    because ACT PSUM-reads serialize against PE matmul
                    execution on this hardware anyway (measured). The score
                    matmuls interleave h0/h1: the two heads' K=64 contractions
                    sit on row-groups 0-1 (partitions 0-63) and 2-3 (64-127),
                    so adjacent MMs on different row groups run CONCURRENTLY
                    on the PE sub-arrays (tile_position auto-derives from the
                    APs' base partitions)."""

                    def mm_into(dst, h, half):
                        hs = slice(h * D, (h + 1) * D)
                        nc.tensor.matmul(
                            dst,
                            kTc[kt_i // 4][hs, (kt_i % 4) * 128 : (kt_i % 4 + 1) * 128],
                            qT[hs, m * QCHUNK + half * 512 : m * QCHUNK + (half + 1) * 512],
                            start=True, stop=True,
                        )

                    if score_order == "f2":
                        s2 = ps.tile([128, 2 * QCHUNK], DT, name="s2", tag="s2", bufs=1)
                        for h, half in ((0, 0), (1, 0), (1, 1), (0, 1)):
                            base = h * QCHUNK + half * 512
                            mm_into(s2[:, base : base + 512], h, half)
                        p2 = sb.tile(
                            [128, 2 * QCHUNK], DTM, name="p_sb", tag="p_sb", bufs=PRE_KT + 1
                        )
                        nc.scalar.activation(p2[:], s2[:], AF.Exp, scale=0.125)
                        return [p2[:, :QCHUNK], p2[:, QCHUNK:]]

                    s_list = [
                        ps.tile([128, QCHUNK], DT, name=f"s_ps{h}", tag="ps_big")
                        for h in range(HPC)
                    ]

                    def exph(h):
                        p_sb = sb.tile([128, QCHUNK], DTM, name="p_sb", tag="p_sb", bufs=p_bufs or (2 * PRE_KT + 2))
                        nc.scalar.activation(p_sb[:], s_list[h][:], AF.Exp, scale=0.125)
                        return p_sb

                    if score_order == "il":
                        for h, half in ((0, 0), (1, 0), (1, 1), (0, 1)):
                            mm_into(s_list[h][:, half * 512 : (half + 1) * 512], h, half)
                        p_out = [exph(0), exph(1)]
                    else:  # "seq": per-head scores immediately followed by exp
                        mm_into(s_list[0][:, :512], 0, 0)
                        mm_into(s_list[0][:, 512:], 0, 1)
                        p0 = exph(0)
                        mm_into(s_list[1][:, :512], 1, 0)
                        mm_into(s_list[1][:, 512:], 1, 1)
                        p_out = [p0, exph(1)]
                    return p_out

                def emit_av(kt_i, h, accs, p_sb):
                    for half in range(2):
                        hsl = slice(half * 512, (half + 1) * 512)
                        nc.tensor.matmul(
                            accs[h][: D + 1, hsl],
                            vn_h(kt_i // 4, kt_i % 4, h),
                            p_sb[:, hsl],
                            start=(kt_i == 0), stop=(kt_i == att_nt - 1),
                        )

                def emit_finish_stripe(m, accs):
                    # copy both accumulators out of PSUM first so their banks
                    # free for the next stripe while normalization runs on SBUF
                    acc_sbs = []
                    for h in range(HPC):
                        acc_sb = sb.tile([D + 1, QCHUNK], DTM, name="acc_sb", tag="acc_sb", bufs=2)
                        nc.vector.tensor_copy(acc_sb[:], accs[h][: D + 1, :])
                        acc_sbs.append(acc_sb)
                    for h in range(HPC):
                        hs = slice(h * D, (h + 1) * D)
                        acc_sb = acc_sbs[h]
                        recip = sb.tile([1, QCHUNK], DTM, name="recip", tag="recip", bufs=2)
                        with nc.allow_low_precision(reason="softmax denom in bf16; tol 2e-2"):
                            nc.vector.reciprocal(recip[:], acc_sb[D : D + 1, :])
                        bcast = sb.tile([D, QCHUNK], DTM, name="bcast", tag="bcast", bufs=2)
                        nc.gpsimd.partition_broadcast(bcast[:], recip[:1, :])
                        nc.vector.tensor_mul(att_m[m][hs, :], acc_sb[:D, :], bcast[:])
                    nc.sync.dma_start(
                        a2a_in[m][:].rearrange("a p t -> p a t"),
                        att_m[m][:].rearrange("p (a t) -> p a t", a=N_CORES),
                    )
                    if not skip_a2a:
                        nc.gpsimd.collective_compute(
                            "AllToAll",
                            mybir.AluOpType.bypass,
                            replica_groups=[list(range(N_CORES))],
                            ins=[a2a_in[m].opt()],
                            outs=[a2a_out[m].opt()],
                        )

                def emit_aTm_load(m):
                    aTm = sb.tile([128, N_CORES * 128], DTM, name="aTm", tag="aTm", bufs=2)
                    nc.sync.dma_start(
                        aTm[:].rearrange("p (a t) -> p a t", a=N_CORES),
                        a2a_out[m][:].rearrange("a p t -> p a t"),
                    )
                    return aTm

                def outproj_pieces(m, aTm_ref, cc, npiece=4, tag=None):
                    """output projection for stripe m, 512-column half cc, as
                    npiece thunks accumulating into an SBUF tile via DVE."""
                    os_ = slice(cc * 512, (cc + 1) * 512)
                    per = HT // npiece
                    holder = []
                    thunks = []
                    for pc in range(npiece):
                        def piece(pc=pc):
                            po = ps.tile([128, 512], DT, name="po", tag=tag, bufs=1 if tag == "s2" else None)
                            for i in range(per * pc, per * (pc + 1)):
                                nc.tensor.matmul(
                                    po[:], aTm_ref[0][:, i * 128 : (i + 1) * 128],
                                    wo_sb[i][:, os_],
                                    start=(i == per * pc), stop=(i == per * (pc + 1) - 1),
                                )
                            if pc == 0:
                                out_sb = sb.tile([128, 512], DT, name="out_sb", tag="out_sb", bufs=2)
                                holder.append(out_sb)
                                nc.vector.tensor_add(out_sb[:], po[:], bo_bc[:, os_])
                            else:
                                out_sb = holder[0]
                                nc.vector.tensor_add(out_sb[:], out_sb[:], po[:])
                            if pc == npiece - 1:
                                nc.sync.dma_start(out[m * 128 : (m + 1) * 128, os_], out_sb[:])
                        thunks.append(piece)
                    return thunks

                # ---- schedule ----------------------------------------------
                def new_accs():
                    return [
                        ps.tile([128, QCHUNK], DT, name=f"acc{h}", tag="ps_acc")
                        for h in range(HPC)
                    ]

                # phase 1: stream chunks with stripe-0 attention interleaved at
                # lag 1 (chunk cp delivers k-tiles 4cp..4cp+3; attention trails
                # one chunk behind so exp work reaches ACT as early as possible).
                # Phase 1 is PE/supply-bound, so projection blocks sit between
                # attention groups without extra cost.
                # chunk-1's q runs before chunk-0's v so the first scores+exp
                # fire as early as possible
                emit_weight_loads()
                emit_xt(0)
                emit_xt(1)
                for t in k_pieces(0) + q_pieces(0) + q_pieces(1) + v_pieces(0):
                    t()
                # out-proj weights load early on the SP queue (x loads are on
                # Pool, so these only queue behind qkv weights)
                for i in range(HT):
                    nc.sync.dma_start(wo_sb[i][:], wo[i * 128 : (i + 1) * 128, :])
                bo_sb = sb.tile([1, HIDDEN], DT)
                nc.sync.dma_start(bo_sb[:], bo[:])
                nc.gpsimd.partition_broadcast(bo_bc[:], bo_sb[:1, :])

                accs = new_accs()
                stash0 = []

                def unit0kt(kt_i):
                    # AVs first: their p_sb inputs are PRE_KT k-tiles old, so
                    # they are never gated -- emitting them before the (gated)
                    # score matmuls keeps the in-order PE queue busy during
                    # the previous k-tile's exp instead of stalling behind it
                    if av_first and len(stash0) >= 2 * PRE_KT:
                        for _ in range(2):
                            pk, ph, pp = stash0.pop(0)
                            emit_av(pk, ph, accs, pp)
                    stash0.extend(zip((kt_i, kt_i), (0, 1), emit_scores_exp2(0, kt_i)))
                    if not av_first and len(stash0) > 2 * PRE_KT:
                        for _ in range(2):
                            pk, ph, pp = stash0.pop(0)
                            emit_av(pk, ph, accs, pp)

                for cp in range(1, HT):
                    if cp > 1:
                        emit_xt(cp)
                        # only stripe 1's q (chunks 2-3) projects in phase 1;
                        # stripes 2-3's q rides the later stripe boundaries where
                        # the exp backlog absorbs it, shrinking the per-chunk
                        # block that starves ACT here
                        pieces = k_pieces(cp) + (q_pieces(cp) if cp < 4 else []) + v_pieces(cp)
                    else:
                        pieces = k_pieces(cp) + v_pieces(cp)
                    a = 4 * (cp - 1)
                    for kk in range(4):
                        unit0kt(a + kk)
                        # one consolidated projection block per chunk (after
                        # the unit: phase 1 is supply-bound and the exps must
                        # start as early as possible; tried before-the-unit,
                        # measured slightly worse)
                        if kk == 1:
                            for t in pieces:
                                t()
                for kt_i in range(4 * (HT - 1), att_nt):
                    unit0kt(kt_i)
                for pk, ph, pp in stash0:
                    emit_av(pk, ph, accs, pp)
                emit_finish_stripe(0, accs)

                # Stripe k's out-projection runs at the START boundary of stripe
                # k+2: its AllToAll is a full stripe old (no collective wait) and
                # the po blocks allocate from the just-freed ps_acc buffers, so
                # the exp stream's s_ps rotation is never interrupted mid-stripe.
                # The new stripe's first AV matmuls lag behind the po blocks but
                # the p_sb triple-buffer absorbs that.
                aTms = {}
                for m in range(1, NQC):
                    # boundary block: previous-previous stripe's out-projection
                    # plus the q projections for stripe m+1, all allocated from
                    # the just-freed ps_acc buffers so the s_ps rotation is
                    # untouched
                    blocks = []
                    if m >= 2:
                        for cc in range(2):
                            blocks += outproj_pieces(m - 2, [aTms[m - 2]], cc, npiece=1, tag="ps_acc")
                    if m < NQC - 1:
                        blocks += q_pieces(2 * (m + 1), tag="ps_acc")
                        blocks += q_pieces(2 * (m + 1) + 1, tag="ps_acc")
                    # pre-emit PRE_KT k-tiles of scores+exp so ACT stays fed
                    # while the boundary blocks occupy the in-order PE stream;
                    # the whole stripe then runs with AV matmuls lagging
                    # 2*PRE_KT units behind their scores, so the deferred AVs
                    # interleave with new scores instead of bunching up after
                    # the blocks
                    stash = []
                    for kt in range(PRE_KT):
                        stash.extend(zip((kt, kt), (0, 1), emit_scores_exp2(m, kt)))
                    for t in blocks:
                        t()
                    accs = new_accs()
                    for kt_i in range(PRE_KT, att_nt):
                        if kt_i == att_nt - 8:
                            aTms[m - 1] = emit_aTm_load(m - 1)
                        # AVs first (inputs PRE_KT k-tiles old, never gated):
                        # the in-order PE queue stays busy during the previous
                        # k-tile's exp instead of stalling behind the gated
                        # score matmuls
                        p01 = None
                        if not av_first:
                            p01 = emit_scores_exp2(m, kt_i)
                        for _ in range(2):
                            pk, ph, pp = stash.pop(0)
                            emit_av(pk, ph, accs, pp)
                        # last stripe: no more blocks need the backlog, so drain
                        # the lag early (one extra AV per k-tile keeps PE and
                        # ACT balanced) instead of flushing it as pure tail
                        if m == NQC - 1 and len(stash) > 2:
                            pk, ph, pp = stash.pop(0)
                            emit_av(pk, ph, accs, pp)
                        if p01 is None:
                            p01 = emit_scores_exp2(m, kt_i)
                        stash.extend(zip((kt_i, kt_i), (0, 1), p01))
                    for pk, ph, pp in stash:
                        emit_av(pk, ph, accs, pp)
                    emit_finish_stripe(m, accs)
                # tail: stripe 2's projection hides under stripe 3's AllToAll
                for cc in range(2):
                    for t in outproj_pieces(NQC - 2, [aTms[NQC - 2]], cc, npiece=1, tag="ps_acc"):
                        t()
                aTm3 = emit_aTm_load(NQC - 1)
                for cc in range(2):
                    for t in outproj_pieces(NQC - 1, [aTm3], cc, npiece=1, tag="ps_acc"):
                        t()

    nc.compile()
    return nc


def _get_nc(mm_mode: str):
    if mm_mode not in _CACHE:
        _CACHE[mm_mode] = _build(mm_mode)
    return _CACHE[mm_mode]


def make_in_maps(x, w_qkv, b_qkv, w_out, b_out):
    import ml_dtypes

    bf16 = ml_dtypes.bfloat16
    x = np.asarray(x, dtype=np.float32)
    w_qkv = np.asarray(w_qkv, dtype=np.float32)
    b_qkv = np.asarray(b_qkv, dtype=np.float32)
    w_out = np.asarray(w_out, dtype=np.float32)
    b_out = np.asarray(b_out, dtype=np.float32)

    xT = x.reshape(N, HIDDEN).T  # [hidden, n]
    # permute n into stripe order n' = (m, j, t) <-> n = 512*j + 128*m + t
    xT = np.ascontiguousarray(
        xT.reshape(HIDDEN, N_CORES, NQC, 128).transpose(0, 2, 1, 3).reshape(HIDDEN, N)
    ).astype(bf16)
    w_out_bf = np.ascontiguousarray(w_out).astype(bf16)
    bo = np.ascontiguousarray(b_out.reshape(1, HIDDEN))
    in_maps = []
    for c in range(N_CORES):
        cs = slice(c * AD, (c + 1) * AD)
        in_maps.append(
            {
                "xT": xT,
                "wq": np.ascontiguousarray(w_qkv[:, :HIDDEN][:, cs]).astype(bf16),
                "wk": np.ascontiguousarray(w_qkv[:, HIDDEN : 2 * HIDDEN][:, cs]).astype(bf16),
                "wv": np.ascontiguousarray(w_qkv[:, 2 * HIDDEN :][:, cs]).astype(bf16),
                "bq": np.ascontiguousarray(b_qkv[:HIDDEN][cs].reshape(AD, 1)),
                "bk": np.ascontiguousarray(b_qkv[HIDDEN : 2 * HIDDEN][cs].reshape(AD, 1)),
                "bvT": np.ascontiguousarray(b_qkv[2 * HIDDEN :][cs].reshape(1, AD)),
                "wo": w_out_bf,
                "bo": bo,
            }
        )
    return in_maps


def kernel(x, w_qkv, b_qkv, w_out, b_out):
    from concourse.bass_utils import run_bass_kernel_spmd

    mm_mode = os.environ.get("TRN_MM_MODE", "bf16")
    nc = _get_nc(mm_mode)
    in_maps = make_in_maps(x, w_qkv, b_qkv, w_out, b_out)
    res = run_bass_kernel_spmd(nc, in_maps, list(range(N_CORES)))
    full = np.concatenate([res.results[c]["out"] for c in range(N_CORES)], axis=0)
    return full.reshape(1, N, HIDDEN).astype(np.float32)



# revision 28
# speedup vs baseline: 2.2136x; 1.0032x over previous
"""Trainium2 Bass kernel for a 16-head dense attention layer (v2, bf16).

Problem: x[1,4096,1024] @ w_qkv[1024,3072] -> 16-head attention (N=4096,
D=64) -> @ w_out[1024,1024].

Sharding: tensor-parallel over heads across 8 NeuronCores (2 heads/core).
Each core computes q/k/v for its 2 heads (weights column-sliced on host),
attention with a fused, max-free softmax (scores are bounded so exp never
overflows in fp32; denominator comes from an appended ones-column in V),
then an AllToAll converts the head-sharded attention output into a
sequence-sharded layout so every core applies the full output projection
to its own 512 rows. Host concatenates the 8 row slices.

vs the f32r baseline: all matmul operands bf16 (PSUM accumulation stays
fp32; rel-err budget 2e-2), V projected directly in [keys, dims] layout
(no PE transposes), merged x DMAs prefetched on the Pool queue, lag-1
attention interleave in phase 1, one AllToAll per stripe, and attention
software-pipelined with AV matmuls lagging 2*PRE_KT=8 (kt,h) units
behind their scores+exp: the ACT engine's exp backlog rides through the
consolidated stripe-boundary blocks (older stripes' output projection
plus the next-next stripe's q projection, allocated from just-freed
ps_acc buffers) without starving, and the last stripe drains its lag
early so it does not flush as pure tail. Within each k-tile iteration
the (never-gated) AV matmuls are emitted BEFORE the score matmuls
(av_first): scores gate on the previous k-tile's exp freeing their PSUM
buffer, and anything emitted after them stalls in the in-order PE queue
-- moving the AVs ahead measured ~50us/body faster.

Hardware notes (measured via micro-benchmarks on this axon/trn2 stack,
see microprobe.py):
- ACT/DVE instructions whose SOURCE is PSUM largely serialize against
  concurrent PE matmul execution (measured near-additive even on
  disjoint PSUM banks), while SBUF-sourced ACT/DVE work overlaps PE
  fine. The exp stream (PSUM->SBUF) therefore sets a serial floor of
  roughly PE-time + exp-time per k-tile; scheduling can recover only a
  partial (~10-35%) overlap. Fusing both heads into one single-buffered
  [128,2048] exp (score_order="f2") measured ~50us WORSE than "seq" --
  the partial overlap double-buffering enables is worth more than the
  saved per-instruction ACT overhead.
- Adjacent matmuls on disjoint PE row-groups run concurrently
  (tile_position auto-derived from base partitions): an interleaved
  h0/h1 score burst measured 415ns vs 1830ns serial in isolation. In
  the full kernel, however, "seq" (per-head scores immediately followed
  by that head's exp) measured best: the interleaved burst's last MM
  gates the next exp and head-of-line blocks the in-order PE queue.
- repeat>1 replicates the whole body in one NEFF for timing: the
  per-dispatch overhead of this axon client is ~0.8ms (trivial-kernel
  floor), which would otherwise dominate the measurement.
"""

import os
import numpy as np

N_CORES = 8
N = 4096
HIDDEN = 1024
D = 64
HPC = 2  # heads per core
AD = HPC * D  # 128 att-dim rows per core
NT = N // 128  # 32 k-tiles of 128
HT = HIDDEN // 128  # 8 hidden tiles
QCHUNK = 1024
NQC = N // QCHUNK  # 4 q-chunks (stripes)
NSLICE = N // N_CORES  # 512 rows of output per core

_CACHE = {}


def _build(
    mm_mode: str = "bf16",
    skip_a2a: bool = False,
    att_nt: int = NT,
    repeat: int = 1,
    score_order: str = "seq",
    pre_kt: int = 4,
    p_bufs: int = 0,
    av_first: bool = True,
    blocks_first: bool = False,
):
    import concourse.bass as bass
    import concourse.mybir as mybir
    import concourse.tile as tile
    from concourse import bacc

    DT = mybir.dt.float32
    DTM = mybir.dt.bfloat16

    AF = mybir.ActivationFunctionType

    nc = bacc.Bacc("TRN2", debug=False, num_devices=N_CORES)

    xT = nc.dram_tensor("xT", [HIDDEN, N], DTM, kind="ExternalInput").ap()
    wq = nc.dram_tensor("wq", [HIDDEN, AD], DTM, kind="ExternalInput").ap()
    wk = nc.dram_tensor("wk", [HIDDEN, AD], DTM, kind="ExternalInput").ap()
    wv = nc.dram_tensor("wv", [HIDDEN, AD], DTM, kind="ExternalInput").ap()
    bq = nc.dram_tensor("bq", [AD, 1], DT, kind="ExternalInput").ap()
    bk = nc.dram_tensor("bk", [AD, 1], DT, kind="ExternalInput").ap()
    bvT = nc.dram_tensor("bvT", [1, AD], DT, kind="ExternalInput").ap()
    wo = nc.dram_tensor("wo", [HIDDEN, HIDDEN], DTM, kind="ExternalInput").ap()
    bo = nc.dram_tensor("bo", [1, HIDDEN], DT, kind="ExternalInput").ap()
    out = nc.dram_tensor("out", [NSLICE, HIDDEN], DT, kind="ExternalOutput").ap()

    with tile.TileContext(nc) as tc:
        with (
            tc.tile_pool(name="sb", bufs=1) as sb,
            tc.tile_pool(name="ps", bufs=2, space="PSUM") as ps,
            tc.tile_pool(name="dram", bufs=1, space="DRAM") as dram,
        ):
            # Global reordering: the sequence axis n is processed in
            # "stripe" order n' = (m, j, t) <-> n = 512*j + 128*m + t
            # (m: stripe 0..3, j: destination core 0..7, t: 0..127).
            # Attention is permutation-invariant in the key axis as long as
            # k and v use the same order, and the q axis just needs the
            # inverse map applied at output -- which the AllToAll block
            # routing does implicitly. Stripe m's attention output IS the
            # m-th out-row-tile of every core, so each stripe's AllToAll +
            # out-projection pipeline behind the next stripe's attention.

            # repeat>1 replicates the whole body inside one NEFF for
            # dispatch-amortized timing; kernel() always uses repeat=1.
            for _rep in range(repeat):
                bvT_sb = sb.tile([1, AD], DT)
                # qkv weights: one DMA each, [1024, 128] folded to [128, 8*128]
                wq_sb = sb.tile([128, HT * AD], DTM)
                wk_sb = sb.tile([128, HT * AD], DTM)
                wv_sb = sb.tile([128, HT * AD], DTM)
                bq_sb = sb.tile([AD, 1], DT)
                bk_sb = sb.tile([AD, 1], DT)
                bv_bc = sb.tile([128, AD], DT)

                def emit_weight_loads():
                    for w_sb, wsrc in ((wq_sb, wq), (wk_sb, wk), (wv_sb, wv)):
                        nc.sync.dma_start(
                            w_sb[:].rearrange("p (a c) -> p a c", a=HT),
                            wsrc.rearrange("(a p) c -> p a c", p=128),
                        )
                    nc.sync.dma_start(bq_sb[:], bq[:])
                    nc.sync.dma_start(bk_sb[:], bk[:])
                    nc.sync.dma_start(bvT_sb[:], bvT[:])
                    nc.gpsimd.partition_broadcast(bv_bc[:], bvT_sb[:1, :])

                def wslice(w_sb, i):
                    return w_sb[:, i * AD : (i + 1) * AD]

                # Host pre-permutes x columns into stripe order n' = (m, j, t),
                # so streaming, qT, kTc, v_nat are all plain contiguous in n'.
                qT = sb.tile([AD, N], DTM)
                kTc = [sb.tile([AD, 512], DTM, name=f"kTc{c}", tag="kTc", bufs=HT) for c in range(HT)]
                att_m = [sb.tile([AD, QCHUNK], DTM, name=f"attm{m}", tag="attm", bufs=NQC) for m in range(NQC)]
                # v in natural [keys, dims] layout: per chunk [128, (j, h, D+1)],
                # ones column at slot D of each head for the softmax denominator.
                v_nat = [
                    sb.tile([128, 4 * HPC * (D + 1)], DTM, name=f"vn{c}", tag="vnat", bufs=HT)
                    for c in range(HT)
                ]
                wo_sb = [sb.tile([128, HIDDEN], DTM, name=f"wo{i}", tag="wo", bufs=HT) for i in range(HT)]
                bo_bc = sb.tile([128, HIDDEN], DT)

                a2a_in = [
                    dram.tile([N_CORES, AD, 128], DTM, name=f"a2ai{m}", tag="a2ai", bufs=NQC)
                    for m in range(NQC)
                ]
                a2a_out = [
                    dram.tile([N_CORES, AD, 128], DTM, name=f"a2ao{m}", tag="a2ao", bufs=NQC)
                    for m in range(NQC)
                ]

                def vn_h(c, j, h):
                    """[128 keys, D+1] slice of chunk c's v for k-tile j, head h."""
                    base = (j * HPC + h) * (D + 1)
                    return v_nat[c][:, base : base + D + 1]

                # Attention runs with AV matmuls lagging 2*PRE_KT (kt,h) units
                # behind their scores+exp: the ACT engine keeps that deep a
                # backlog of materialized exps, so projection blocks occupying
                # the in-order PE stream no longer starve it.
                PRE_KT = pre_kt

                # ---- emission helpers --------------------------------------
                # All non-attention PE work is emitted as small "pieces" (2-4
                # matmuls, ~0.4-0.9us) with DVE partial accumulation, woven
                # between attention (scores+exp+AV) pairs. A long uninterrupted
                # matmul block would stall the in-order PE stream past the ~2
                # tiles of exp backlog the s_ps double-buffer can hold, idling
                # the ACT engine (the overall bottleneck) by its own duration.
                # Pieces are always injected in PAIRS so the number of ps_big
                # allocations between consecutive s_ps allocations stays even
                # and s_ps keeps alternating between its two buffers.
                xts = []

                def emit_xt(cp, eng=None):
                    """x chunk load via the (otherwise idle) Pool queue so the
                    SP queue's weight DMAs never delay it; 8 bufs = fully
                    prefetched, no reuse dependency between chunks (they also
                    stay resident for the deferred q projections)."""
                    cs = slice(cp * 512, (cp + 1) * 512)
                    xt = sb.tile([128, HT * 512], DTM, name="xt", tag="xt", bufs=HT)
                    xts.append(xt)
                    (eng or nc.gpsimd).dma_start(
                        xt[:].rearrange("p (a t) -> p a t", a=HT),
                        xT[:, cs].rearrange("(a p) t -> p a t", p=128),
                    )

                # default PSUM tag for projection pieces: under f2 the score
                # tile is a single-buffered 4-bank [128,2048], and pieces
                # share its slot (PSUM budget: s2 4 + ps_acc 4 = 8 banks)
                PTAG = "s2" if score_order == "f2" else "ps_big"

                def qk_pieces(cp, w_sb, b_sb, dst, npiece, tag=None):
                    """q or k projection for chunk cp as npiece thunks."""
                    tag = tag or PTAG
                    per = HT // npiece
                    thunks = []
                    for pc in range(npiece):
                        def piece(pc=pc):
                            pp = ps.tile([128, 512], DT, name="pp", tag=tag, bufs=1 if tag == "s2" else None)
                            for i in range(per * pc, per * (pc + 1)):
                                nc.tensor.matmul(
                                    pp[:AD, :], wslice(w_sb, i),
                                    xts[cp][:, i * 512 : (i + 1) * 512],
                                    start=(i == per * pc), stop=(i == per * (pc + 1) - 1),
                                )
                            if pc == 0:
                                nc.vector.tensor_scalar_add(dst, pp[:AD, :], b_sb[:])
                            else:
                                nc.vector.tensor_add(dst, dst, pp[:AD, :])
                        thunks.append(piece)
                    return thunks

                def k_pieces(cp):
                    return qk_pieces(cp, wk_sb, bk_sb, kTc[cp][:], 2)

                def q_pieces(cp, npiece=2, tag=None):
                    cs = slice(cp * 512, (cp + 1) * 512)
                    return qk_pieces(cp, wq_sb, bq_sb, qT[:, cs], npiece, tag)

                def v_pieces(cp):
                    """v directly in [keys, dims] layout: x-chunk tile as the
                    stationary operand, wv moving; out partitions are the 128
                    keys of k-tile j. Two thunks of two k-tiles each."""
                    thunks = []
                    for half in (0, 1):
                        def piece(half=half):
                            pv = ps.tile([128, 256], DT, name="pv", tag=PTAG, bufs=1 if PTAG == "s2" else None)
                            for jj in (0, 1):
                                j = 2 * half + jj
                                for i in range(HT):
                                    nc.tensor.matmul(
                                        pv[:, jj * 128 : (jj + 1) * 128],
                                        xts[cp][:, i * 512 + j * 128 : i * 512 + (j + 1) * 128],
                                        wslice(wv_sb, i),
                                        start=(i == 0), stop=(i == HT - 1),
                                    )
                            vn4 = v_nat[cp][:].rearrange("p (j h x) -> p j h x", j=4, x=D + 1)
                            for jj in (0, 1):
                                j = 2 * half + jj
                                nc.vector.tensor_add(
                                    vn4[:, j, :, :D],
                                    pv[:, jj * 128 : (jj + 1) * 128].rearrange(
                                        "p (h d) -> p h d", h=HPC
                                    ),
                                    bv_bc[:].rearrange("p (h d) -> p h d", h=HPC),
                                )
                            nc.vector.memset(
                                vn4[:, 2 * half : 2 * half + 2, :, D : D + 1], 1.0
                            )
                        thunks.append(piece)
                    return thunks

                def emit_scores_exp2(m, kt_i):
                    """Scores + exp for BOTH heads of k-tile kt_i. "f2": one
                    single-buffered [128,2048] PSUM tile holds both heads'
                    scores and ONE exp covers them -- halves the per-
                    instruction ACT overhead; single-buffering costs nothing
                    because ACT PSUM-reads serialize against PE matmul
                    execution on this hardware anyway (measured). The score
                    matmuls interleave h0/h1: the two heads' K=64 contractions
                    sit on row-groups 0-1 (partitions 0-63) and 2-3 (64-127),
                    so adjacent MMs on different row groups run CONCURRENTLY
                    on the PE sub-arrays (tile_position auto-derives from the
                    APs' base partitions)."""

                    def mm_into(dst, h, half):
                        hs = slice(h * D, (h + 1) * D)
                        nc.tensor.matmul(
                            dst,
                            kTc[kt_i // 4][hs, (kt_i % 4) * 128 : (kt_i % 4 + 1) * 128],
                            qT[hs, m * QCHUNK + half * 512 : m * QCHUNK + (half + 1) * 512],
                            start=True, stop=True,
                        )

                    if score_order == "f2":
                        s2 = ps.tile([128, 2 * QCHUNK], DT, name="s2", tag="s2", bufs=1)
                        for h, half in ((0, 0), (1, 0), (1, 1), (0, 1)):
                            base = h * QCHUNK + half * 512
                            mm_into(s2[:, base : base + 512], h, half)
                        p2 = sb.tile(
                            [128, 2 * QCHUNK], DTM, name="p_sb", tag="p_sb", bufs=PRE_KT + 1
                        )
                        nc.scalar.activation(p2[:], s2[:], AF.Exp, scale=0.125)
                        return [p2[:, :QCHUNK], p2[:, QCHUNK:]]

                    s_list = [
                        ps.tile([128, QCHUNK], DT, name=f"s_ps{h}", tag="ps_big")
                        for h in range(HPC)
                    ]

                    def exph(h):
                        p_sb = sb.tile([128, QCHUNK], DTM, name="p_sb", tag="p_sb", bufs=p_bufs or (2 * PRE_KT + 2))
                        nc.scalar.activation(p_sb[:], s_list[h][:], AF.Exp, scale=0.125)
                        return p_sb

                    if score_order == "il":
                        for h, half in ((0, 0), (1, 0), (1, 1), (0, 1)):
                            mm_into(s_list[h][:, half * 512 : (half + 1) * 512], h, half)
                        p_out = [exph(0), exph(1)]
                    else:  # "seq": per-head scores immediately followed by exp
                        mm_into(s_list[0][:, :512], 0, 0)
                        mm_into(s_list[0][:, 512:], 0, 1)
                        p0 = exph(0)
                        mm_into(s_list[1][:, :512], 1, 0)
                        mm_into(s_list[1][:, 512:], 1, 1)
                        p_out = [p0, exph(1)]
                    return p_out

                def emit_av(kt_i, h, accs, p_sb):
                    for half in range(2):
                        hsl = slice(half * 512, (half + 1) * 512)
                        nc.tensor.matmul(
                            accs[h][: D + 1, hsl],
                            vn_h(kt_i // 4, kt_i % 4, h),
                            p_sb[:, hsl],
                            start=(kt_i == 0), stop=(kt_i == att_nt - 1),
                        )

                def emit_finish_stripe(m, accs):
                    # copy both accumulators out of PSUM first so their banks
                    # free for the next stripe while normalization runs on SBUF
                    acc_sbs = []
                    for h in range(HPC):
                        acc_sb = sb.tile([D + 1, QCHUNK], DTM, name="acc_sb", tag="acc_sb", bufs=2)
                        nc.vector.tensor_copy(acc_sb[:], accs[h][: D + 1, :])
                        acc_sbs.append(acc_sb)
                    for h in range(HPC):
                        hs = slice(h * D, (h + 1) * D)
                        acc_sb = acc_sbs[h]
                        recip = sb.tile([1, QCHUNK], DTM, name="recip", tag="recip", bufs=2)
                        with nc.allow_low_precision(reason="softmax denom in bf16; tol 2e-2"):
                            nc.vector.reciprocal(recip[:], acc_sb[D : D + 1, :])
                        bcast = sb.tile([D, QCHUNK], DTM, name="bcast", tag="bcast", bufs=2)
                        nc.gpsimd.partition_broadcast(bcast[:], recip[:1, :])
                        nc.vector.tensor_mul(att_m[m][hs, :], acc_sb[:D, :], bcast[:])
                    nc.sync.dma_start(
                        a2a_in[m][:].rearrange("a p t -> p a t"),
                        att_m[m][:].rearrange("p (a t) -> p a t", a=N_CORES),
                    )
                    if not skip_a2a:
                        nc.gpsimd.collective_compute(
                            "AllToAll",
                            mybir.AluOpType.bypass,
                            replica_groups=[list(range(N_CORES))],
                            ins=[a2a_in[m].opt()],
                            outs=[a2a_out[m].opt()],
                        )

                def emit_aTm_load(m):
                    aTm = sb.tile([128, N_CORES * 128], DTM, name="aTm", tag="aTm", bufs=2)
                    nc.sync.dma_start(
                        aTm[:].rearrange("p (a t) -> p a t", a=N_CORES),
                        a2a_out[m][:].rearrange("a p t -> p a t"),
                    )
                    return aTm

                def outproj_pieces(m, aTm_ref, cc, npiece=4, tag=None):
                    """output projection for stripe m, 512-column half cc, as
                    npiece thunks accumulating into an SBUF tile via DVE."""
                    os_ = slice(cc * 512, (cc + 1) * 512)
                    per = HT // npiece
                    holder = []
                    thunks = []
                    for pc in range(npiece):
                        def piece(pc=pc):
                            po = ps.tile([128, 512], DT, name="po", tag=tag, bufs=1 if tag == "s2" else None)
                            for i in range(per * pc, per * (pc + 1)):
                                nc.tensor.matmul(
                                    po[:], aTm_ref[0][:, i * 128 : (i + 1) * 128],
                                    wo_sb[i][:, os_],
                                    start=(i == per * pc), stop=(i == per * (pc + 1) - 1),
                                )
                            if pc == 0:
                                out_sb = sb.tile([128, 512], DT, name="out_sb", tag="out_sb", bufs=2)
                                holder.append(out_sb)
                                nc.vector.tensor_add(out_sb[:], po[:], bo_bc[:, os_])
                            else:
                                out_sb = holder[0]
                                nc.vector.tensor_add(out_sb[:], out_sb[:], po[:])
                            if pc == npiece - 1:
                                nc.sync.dma_start(out[m * 128 : (m + 1) * 128, os_], out_sb[:])
                        thunks.append(piece)
                    return thunks

                # ---- schedule ----------------------------------------------
                def new_accs():
                    return [
                        ps.tile([128, QCHUNK], DT, name=f"acc{h}", tag="ps_acc")
                        for h in range(HPC)
                    ]

                # phase 1: stream chunks with stripe-0 attention interleaved at
                # lag 1 (chunk cp delivers k-tiles 4cp..4cp+3; attention trails
                # one chunk behind so exp work reaches ACT as early as possible).
                # Phase 1 is PE/supply-bound, so projection blocks sit between
                # attention groups without extra cost.
                # chunk-1's q runs before chunk-0's v so the first scores+exp
                # fire as early as possible
                emit_weight_loads()
                emit_xt(0)
                emit_xt(1)
                for t in k_pieces(0) + q_pieces(0) + q_pieces(1) + v_pieces(0):
                    t()
                # out-proj weights load early on the SP queue (x loads are on
                # Pool, so these only queue behind qkv weights)
                for i in range(HT):
                    nc.sync.dma_start(wo_sb[i][:], wo[i * 128 : (i + 1) * 128, :])
                bo_sb = sb.tile([1, HIDDEN], DT)
                nc.sync.dma_start(bo_sb[:], bo[:])
                nc.gpsimd.partition_broadcast(bo_bc[:], bo_sb[:1, :])

                accs = new_accs()
                stash0 = []

                def unit0kt(kt_i):
                    # AVs first: their p_sb inputs are PRE_KT k-tiles old, so
                    # they are never gated -- emitting them before the (gated)
                    # score matmuls keeps the in-order PE queue busy during
                    # the previous k-tile's exp instead of stalling behind it
                    if av_first and len(stash0) >= 2 * PRE_KT:
                        for _ in range(2):
                            pk, ph, pp = stash0.pop(0)
                            emit_av(pk, ph, accs, pp)
                    stash0.extend(zip((kt_i, kt_i), (0, 1), emit_scores_exp2(0, kt_i)))
                    if not av_first and len(stash0) > 2 * PRE_KT:
                        for _ in range(2):
                            pk, ph, pp = stash0.pop(0)
                            emit_av(pk, ph, accs, pp)

                for cp in range(1, HT):
                    if cp > 1:
                        emit_xt(cp)
                        # only stripe 1's q (chunks 2-3) projects in phase 1;
                        # stripes 2-3's q rides the later stripe boundaries where
                        # the exp backlog absorbs it, shrinking the per-chunk
                        # block that starves ACT here
                        pieces = k_pieces(cp) + (q_pieces(cp) if cp < 4 else []) + v_pieces(cp)
                    else:
                        pieces = k_pieces(cp) + v_pieces(cp)
                    a = 4 * (cp - 1)
                    for kk in range(4):
                        unit0kt(a + kk)
                        # one consolidated projection block per chunk (after
                        # the unit: phase 1 is supply-bound and the exps must
                        # start as early as possible; tried before-the-unit,
                        # measured slightly worse)
                        if kk == 1:
                            for t in pieces:
                                t()
                for kt_i in range(4 * (HT - 1), att_nt):
                    unit0kt(kt_i)
                for pk, ph, pp in stash0:
                    emit_av(pk, ph, accs, pp)
                emit_finish_stripe(0, accs)

                # Stripe k's out-projection runs at the START boundary of stripe
                # k+2: its AllToAll is a full stripe old (no collective wait) and
                # the po blocks allocate from the just-freed ps_acc buffers, so
                # the exp stream's s_ps rotation is never interrupted mid-stripe.
                # The new stripe's first AV matmuls lag behind the po blocks but
                # the p_sb triple-buffer absorbs that.
                aTms = {}
                for m in range(1, NQC):
                    # boundary block: previous-previous stripe's out-projection
                    # plus the q projections for stripe m+1, all allocated from
                    # the just-freed ps_acc buffers so the s_ps rotation is
                    # untouched
                    blocks = []
                    if m >= 2:
                        for cc in range(2):
                            blocks += outproj_pieces(m - 2, [aTms[m - 2]], cc, npiece=1, tag="ps_acc")
                    if m < NQC - 1:
                        blocks += q_pieces(2 * (m + 1), tag="ps_acc")
                        blocks += q_pieces(2 * (m + 1) + 1, tag="ps_acc")
                    # pre-emit PRE_KT k-tiles of scores+exp so ACT stays fed
                    # while the boundary blocks occupy the in-order PE stream;
                    # the whole stripe then runs with AV matmuls lagging
                    # 2*PRE_KT units behind their scores, so the deferred AVs
                    # interleave with new scores instead of bunching up after
                    # the blocks
                    stash = []
                    if blocks_first:
                        # boundary blocks are never gated (aTm a full stripe
                        # old, x resident) -- ahead of the pre-emit scores
                        # they keep the in-order PE queue busy
                        for t in blocks:
                            t()
                        for kt in range(PRE_KT):
                            stash.extend(zip((kt, kt), (0, 1), emit_scores_exp2(m, kt)))
                    else:
                        for kt in range(PRE_KT):
                            stash.extend(zip((kt, kt), (0, 1), emit_scores_exp2(m, kt)))
                        for t in blocks:
                            t()
                    accs = new_accs()
                    for kt_i in range(PRE_KT, att_nt):
                        if kt_i == att_nt - 8:
                            aTms[m - 1] = emit_aTm_load(m - 1)
                        # AVs first (inputs PRE_KT k-tiles old, never gated):
                        # the in-order PE queue stays busy during the previous
                        # k-tile's exp instead of stalling behind the gated
                        # score matmuls
                        p01 = None
                        if not av_first:
                            p01 = emit_scores_exp2(m, kt_i)
                        for _ in range(2):
                            pk, ph, pp = stash.pop(0)
                            emit_av(pk, ph, accs, pp)
                        # last stripe: no more blocks need the backlog, so drain
                        # the lag early (one extra AV per k-tile keeps PE and
                        # ACT balanced) instead of flushing it as pure tail
                        if m == NQC - 1 and len(stash) > 2:
                            pk, ph, pp = stash.pop(0)
                            emit_av(pk, ph, accs, pp)
                        if p01 is None:
                            p01 = emit_scores_exp2(m, kt_i)
                        stash.extend(zip((kt_i, kt_i), (0, 1), p01))
                    for pk, ph, pp in stash:
                        emit_av(pk, ph, accs, pp)
                    emit_finish_stripe(m, accs)
                # tail: stripe 2's projection hides under stripe 3's AllToAll
                for cc in range(2):
                    for t in outproj_pieces(NQC - 2, [aTms[NQC - 2]], cc, npiece=1, tag="ps_acc"):
                        t()
                aTm3 = emit_aTm_load(NQC - 1)
                for cc in range(2):
                    for t in outproj_pieces(NQC - 1, [aTm3], cc, npiece=1, tag="ps_acc"):
                        t()

    nc.compile()
    return nc


def _get_nc(mm_mode: str):
    if mm_mode not in _CACHE:
        _CACHE[mm_mode] = _build(mm_mode)
    return _CACHE[mm_mode]


def make_in_maps(x, w_qkv, b_qkv, w_out, b_out):
    import ml_dtypes

    bf16 = ml_dtypes.bfloat16
    x = np.asarray(x, dtype=np.float32)
    w_qkv = np.asarray(w_qkv, dtype=np.float32)
    b_qkv = np.asarray(b_qkv, dtype=np.float32)
    w_out = np.asarray(w_out, dtype=np.float32)
    b_out = np.asarray(b_out, dtype=np.float32)

    xT = x.reshape(N, HIDDEN).T  # [hidden, n]
    # permute n into stripe order n' = (m, j, t) <-> n = 512*j + 128*m + t
    xT = np.ascontiguousarray(
        xT.reshape(HIDDEN, N_CORES, NQC, 128).transpose(0, 2, 1, 3).reshape(HIDDEN, N)
    ).astype(bf16)
    w_out_bf = np.ascontiguousarray(w_out).astype(bf16)
    bo = np.ascontiguousarray(b_out.reshape(1, HIDDEN))
    in_maps = []
    for c in range(N_CORES):
        cs = slice(c * AD, (c + 1) * AD)
        in_maps.append(
            {
                "xT": xT,
                "wq": np.ascontiguousarray(w_qkv[:, :HIDDEN][:, cs]).astype(bf16),
                "wk": np.ascontiguousarray(w_qkv[:, HIDDEN : 2 * HIDDEN][:, cs]).astype(bf16),
                "wv": np.ascontiguousarray(w_qkv[:, 2 * HIDDEN :][:, cs]).astype(bf16),
                "bq": np.ascontiguousarray(b_qkv[:HIDDEN][cs].reshape(AD, 1)),
                "bk": np.ascontiguousarray(b_qkv[HIDDEN : 2 * HIDDEN][cs].reshape(AD, 1)),
                "bvT": np.ascontiguousarray(b_qkv[2 * HIDDEN :][cs].reshape(1, AD)),
                "wo": w_out_bf,
                "bo": bo,
            }
        )
    return in_maps


def kernel(x, w_qkv, b_qkv, w_out, b_out):
    from concourse.bass_utils import run_bass_kernel_spmd

    mm_mode = os.environ.get("TRN_MM_MODE", "bf16")
    nc = _get_nc(mm_mode)
    in_maps = make_in_maps(x, w_qkv, b_qkv, w_out, b_out)
    res = run_bass_kernel_spmd(nc, in_maps, list(range(N_CORES)))
    full = np.concatenate([res.results[c]["out"] for c in range(N_CORES)], axis=0)
    return full.reshape(1, N, HIDDEN).astype(np.float32)



# revision 30
# speedup vs baseline: 2.5774x; 1.1643x over previous
"""Trainium2 Bass kernel for a 16-head dense attention layer (v2, bf16).

Problem: x[1,4096,1024] @ w_qkv[1024,3072] -> 16-head attention (N=4096,
D=64) -> @ w_out[1024,1024].

Sharding: tensor-parallel over heads across 8 NeuronCores (2 heads/core).
Each core computes q/k/v for its 2 heads (weights column-sliced on host),
attention with a fused, max-free softmax (scores are bounded so exp never
overflows in fp32; denominator comes from an appended ones-column in V),
then an AllToAll converts the head-sharded attention output into a
sequence-sharded layout so every core applies the full output projection
to its own 512 rows. Host concatenates the 8 row slices.

vs the f32r baseline: all matmul operands bf16 (PSUM accumulation stays
fp32; rel-err budget 2e-2), V projected directly in [keys, dims] layout
(no PE transposes), merged x DMAs prefetched on the Pool queue, lag-1
attention interleave in phase 1, one AllToAll per stripe, and attention
software-pipelined with AV matmuls lagging 2*PRE_KT=8 (kt,h) units
behind their scores+exp: the ACT engine's exp backlog rides through the
consolidated stripe-boundary blocks (older stripes' output projection
plus the next-next stripe's q projection, allocated from just-freed
ps_acc buffers) without starving, and the last stripe drains its lag
early so it does not flush as pure tail. Within each k-tile iteration
the (never-gated) AV matmuls are emitted BEFORE the score matmuls
(av_first): scores gate on the previous k-tile's exp freeing their PSUM
buffer, and anything emitted after them stalls in the in-order PE queue
-- moving the AVs ahead measured ~50us/body faster.

Hardware notes (measured via micro-benchmarks on this axon/trn2 stack,
see microprobe.py):
- ACT/DVE instructions whose SOURCE is PSUM largely serialize against
  concurrent PE matmul execution (measured near-additive even on
  disjoint PSUM banks), while SBUF-sourced ACT/DVE work overlaps PE
  fine. The exp stream (PSUM->SBUF) therefore sets a serial floor of
  roughly PE-time + exp-time per k-tile; scheduling can recover only a
  partial (~10-35%) overlap. Fusing both heads into one single-buffered
  [128,2048] exp (score_order="f2") measured ~50us WORSE than "seq" --
  the partial overlap double-buffering enables is worth more than the
  saved per-instruction ACT overhead.
- Adjacent matmuls on disjoint PE row-groups run concurrently
  (tile_position auto-derived from base partitions): an interleaved
  h0/h1 score burst measured 415ns vs 1830ns serial in isolation. In
  the full kernel, however, "seq" (per-head scores immediately followed
  by that head's exp) measured best: the interleaved burst's last MM
  gates the next exp and head-of-line blocks the in-order PE queue.
- repeat>1 replicates the whole body in one NEFF for timing: the
  per-dispatch overhead of this axon client is ~0.8ms (trivial-kernel
  floor), which would otherwise dominate the measurement.
"""

import os
import numpy as np

N_CORES = 8
N = 4096
HIDDEN = 1024
D = 64
HPC = 2  # heads per core
AD = HPC * D  # 128 att-dim rows per core
NT = N // 128  # 32 k-tiles of 128
HT = HIDDEN // 128  # 8 hidden tiles
QCHUNK = 1024
NQC = N // QCHUNK  # 4 q-chunks (stripes)
NSLICE = N // N_CORES  # 512 rows of output per core

_CACHE = {}


def _build(
    mm_mode: str = "bf16",
    skip_a2a: bool = False,
    att_nt: int = NT,
    repeat: int = 1,
    score_order: str = "seq",
    pre_kt: int = 4,
    p_bufs: int = 0,
    av_first: bool = True,
    blocks_first: bool = False,
    prefill_early: bool = False,
):
    import concourse.bass as bass
    import concourse.mybir as mybir
    import concourse.tile as tile
    from concourse import bacc

    DT = mybir.dt.float32
    DTM = mybir.dt.bfloat16

    AF = mybir.ActivationFunctionType

    nc = bacc.Bacc("TRN2", debug=False, num_devices=N_CORES)

    xT = nc.dram_tensor("xT", [HIDDEN, N], DTM, kind="ExternalInput").ap()
    wq = nc.dram_tensor("wq", [HIDDEN, AD], DTM, kind="ExternalInput").ap()
    wk = nc.dram_tensor("wk", [HIDDEN, AD], DTM, kind="ExternalInput").ap()
    wv = nc.dram_tensor("wv", [HIDDEN, AD], DTM, kind="ExternalInput").ap()
    bq = nc.dram_tensor("bq", [AD, 1], DT, kind="ExternalInput").ap()
    bk = nc.dram_tensor("bk", [AD, 1], DT, kind="ExternalInput").ap()
    bvT = nc.dram_tensor("bvT", [1, AD], DT, kind="ExternalInput").ap()
    wo = nc.dram_tensor("wo", [HIDDEN, HIDDEN], DTM, kind="ExternalInput").ap()
    bo = nc.dram_tensor("bo", [1, HIDDEN], DT, kind="ExternalInput").ap()
    out = nc.dram_tensor("out", [NSLICE, HIDDEN], DT, kind="ExternalOutput").ap()

    with tile.TileContext(nc) as tc:
        with (
            tc.tile_pool(name="sb", bufs=1) as sb,
            tc.tile_pool(name="ps", bufs=2, space="PSUM") as ps,
            tc.tile_pool(name="dram", bufs=1, space="DRAM") as dram,
        ):
            # Global reordering: the sequence axis n is processed in
            # "stripe" order n' = (m, j, t) <-> n = 512*j + 128*m + t
            # (m: stripe 0..3, j: destination core 0..7, t: 0..127).
            # Attention is permutation-invariant in the key axis as long as
            # k and v use the same order, and the q axis just needs the
            # inverse map applied at output -- which the AllToAll block
            # routing does implicitly. Stripe m's attention output IS the
            # m-th out-row-tile of every core, so each stripe's AllToAll +
            # out-projection pipeline behind the next stripe's attention.

            # repeat>1 replicates the whole body inside one NEFF for
            # dispatch-amortized timing; kernel() always uses repeat=1.
            for _rep in range(repeat):
                bvT_sb = sb.tile([1, AD], DT)
                # qkv weights: one DMA each, [1024, 128] folded to [128, 8*128]
                wq_sb = sb.tile([128, HT * AD], DTM)
                wk_sb = sb.tile([128, HT * AD], DTM)
                wv_sb = sb.tile([128, HT * AD], DTM)
                bq_sb = sb.tile([AD, 1], DT)
                bk_sb = sb.tile([AD, 1], DT)
                bv_bc = sb.tile([128, AD], DT)

                def emit_weight_loads():
                    for w_sb, wsrc in ((wq_sb, wq), (wk_sb, wk), (wv_sb, wv)):
                        nc.sync.dma_start(
                            w_sb[:].rearrange("p (a c) -> p a c", a=HT),
                            wsrc.rearrange("(a p) c -> p a c", p=128),
                        )
                    nc.sync.dma_start(bq_sb[:], bq[:])
                    nc.sync.dma_start(bk_sb[:], bk[:])
                    nc.sync.dma_start(bvT_sb[:], bvT[:])
                    nc.gpsimd.partition_broadcast(bv_bc[:], bvT_sb[:1, :])

                def wslice(w_sb, i):
                    return w_sb[:, i * AD : (i + 1) * AD]

                # Host pre-permutes x columns into stripe order n' = (m, j, t),
                # so streaming, qT, kTc, v_nat are all plain contiguous in n'.
                qT = sb.tile([AD, N], DTM)
                kTc = [sb.tile([AD, 512], DTM, name=f"kTc{c}", tag="kTc", bufs=HT) for c in range(HT)]
                att_m = [sb.tile([AD, QCHUNK], DTM, name=f"attm{m}", tag="attm", bufs=NQC) for m in range(NQC)]
                # v in natural [keys, dims] layout: per chunk [128, (j, h, D+1)],
                # ones column at slot D of each head for the softmax denominator.
                v_nat = [
                    sb.tile([128, 4 * HPC * (D + 1)], DTM, name=f"vn{c}", tag="vnat", bufs=HT)
                    for c in range(HT)
                ]
                wo_sb = [sb.tile([128, HIDDEN], DTM, name=f"wo{i}", tag="wo", bufs=HT) for i in range(HT)]
                bo_bc = sb.tile([128, HIDDEN], DT)

                a2a_in = [
                    dram.tile([N_CORES, AD, 128], DTM, name=f"a2ai{m}", tag="a2ai", bufs=NQC)
                    for m in range(NQC)
                ]
                a2a_out = [
                    dram.tile([N_CORES, AD, 128], DTM, name=f"a2ao{m}", tag="a2ao", bufs=NQC)
                    for m in range(NQC)
                ]

                def vn_h(c, j, h):
                    """[128 keys, D+1] slice of chunk c's v for k-tile j, head h."""
                    base = (j * HPC + h) * (D + 1)
                    return v_nat[c][:, base : base + D + 1]

                # Attention runs with AV matmuls lagging 2*PRE_KT (kt,h) units
                # behind their scores+exp: the ACT engine keeps that deep a
                # backlog of materialized exps, so projection blocks occupying
                # the in-order PE stream no longer starve it.
                PRE_KT = pre_kt

                # ---- emission helpers --------------------------------------
                # All non-attention PE work is emitted as small "pieces" (2-4
                # matmuls, ~0.4-0.9us) with DVE partial accumulation, woven
                # between attention (scores+exp+AV) pairs. A long uninterrupted
                # matmul block would stall the in-order PE stream past the ~2
                # tiles of exp backlog the s_ps double-buffer can hold, idling
                # the ACT engine (the overall bottleneck) by its own duration.
                # Pieces are always injected in PAIRS so the number of ps_big
                # allocations between consecutive s_ps allocations stays even
                # and s_ps keeps alternating between its two buffers.
                xts = []

                def emit_xt(cp, eng=None):
                    """x chunk load via the (otherwise idle) Pool queue so the
                    SP queue's weight DMAs never delay it; 8 bufs = fully
                    prefetched, no reuse dependency between chunks (they also
                    stay resident for the deferred q projections)."""
                    cs = slice(cp * 512, (cp + 1) * 512)
                    xt = sb.tile([128, HT * 512], DTM, name="xt", tag="xt", bufs=HT)
                    xts.append(xt)
                    (eng or nc.gpsimd).dma_start(
                        xt[:].rearrange("p (a t) -> p a t", a=HT),
                        xT[:, cs].rearrange("(a p) t -> p a t", p=128),
                    )

                # default PSUM tag for projection pieces: under f2 the score
                # tile is a single-buffered 4-bank [128,2048], and pieces
                # share its slot (PSUM budget: s2 4 + ps_acc 4 = 8 banks)
                PTAG = "s2" if score_order == "f2" else "ps_big"

                def qk_pieces(cp, w_sb, b_sb, dst, npiece, tag=None):
                    """q or k projection for chunk cp as npiece thunks."""
                    tag = tag or PTAG
                    per = HT // npiece
                    thunks = []
                    for pc in range(npiece):
                        def piece(pc=pc):
                            pp = ps.tile([128, 512], DT, name="pp", tag=tag, bufs=1 if tag == "s2" else None)
                            for i in range(per * pc, per * (pc + 1)):
                                nc.tensor.matmul(
                                    pp[:AD, :], wslice(w_sb, i),
                                    xts[cp][:, i * 512 : (i + 1) * 512],
                                    start=(i == per * pc), stop=(i == per * (pc + 1) - 1),
                                )
                            if pc == 0:
                                nc.vector.tensor_scalar_add(dst, pp[:AD, :], b_sb[:])
                            else:
                                nc.vector.tensor_add(dst, dst, pp[:AD, :])
                        thunks.append(piece)
                    return thunks

                def k_pieces(cp):
                    return qk_pieces(cp, wk_sb, bk_sb, kTc[cp][:], 2)

                def q_pieces(cp, npiece=2, tag=None):
                    cs = slice(cp * 512, (cp + 1) * 512)
                    return qk_pieces(cp, wq_sb, bq_sb, qT[:, cs], npiece, tag)

                def v_pieces(cp):
                    """v directly in [keys, dims] layout: x-chunk tile as the
                    stationary operand, wv moving; out partitions are the 128
                    keys of k-tile j. Two thunks of two k-tiles each."""
                    thunks = []
                    for half in (0, 1):
                        def piece(half=half):
                            pv = ps.tile([128, 256], DT, name="pv", tag=PTAG, bufs=1 if PTAG == "s2" else None)
                            for jj in (0, 1):
                                j = 2 * half + jj
                                for i in range(HT):
                                    nc.tensor.matmul(
                                        pv[:, jj * 128 : (jj + 1) * 128],
                                        xts[cp][:, i * 512 + j * 128 : i * 512 + (j + 1) * 128],
                                        wslice(wv_sb, i),
                                        start=(i == 0), stop=(i == HT - 1),
                                    )
                            vn4 = v_nat[cp][:].rearrange("p (j h x) -> p j h x", j=4, x=D + 1)
                            for jj in (0, 1):
                                j = 2 * half + jj
                                nc.vector.tensor_add(
                                    vn4[:, j, :, :D],
                                    pv[:, jj * 128 : (jj + 1) * 128].rearrange(
                                        "p (h d) -> p h d", h=HPC
                                    ),
                                    bv_bc[:].rearrange("p (h d) -> p h d", h=HPC),
                                )
                            nc.vector.memset(
                                vn4[:, 2 * half : 2 * half + 2, :, D : D + 1], 1.0
                            )
                        thunks.append(piece)
                    return thunks

                def emit_scores_exp2(m, kt_i):
                    """Scores + exp for BOTH heads of k-tile kt_i. "f2": one
                    single-buffered [128,2048] PSUM tile holds both heads'
                    scores and ONE exp covers them -- halves the per-
                    instruction ACT overhead; single-buffering costs nothing
                    because ACT PSUM-reads serialize against PE matmul
                    execution on this hardware anyway (measured). The score
                    matmuls interleave h0/h1: the two heads' K=64 contractions
                    sit on row-groups 0-1 (partitions 0-63) and 2-3 (64-127),
                    so adjacent MMs on different row groups run CONCURRENTLY
                    on the PE sub-arrays (tile_position auto-derives from the
                    APs' base partitions)."""

                    def mm_into(dst, h, half):
                        hs = slice(h * D, (h + 1) * D)
                        nc.tensor.matmul(
                            dst,
                            kTc[kt_i // 4][hs, (kt_i % 4) * 128 : (kt_i % 4 + 1) * 128],
                            qT[hs, m * QCHUNK + half * 512 : m * QCHUNK + (half + 1) * 512],
                            start=True, stop=True,
                        )

                    if score_order == "f2":
                        s2 = ps.tile([128, 2 * QCHUNK], DT, name="s2", tag="s2", bufs=1)
                        for h, half in ((0, 0), (1, 0), (1, 1), (0, 1)):
                            base = h * QCHUNK + half * 512
                            mm_into(s2[:, base : base + 512], h, half)
                        p2 = sb.tile(
                            [128, 2 * QCHUNK], DTM, name="p_sb", tag="p_sb", bufs=PRE_KT + 1
                        )
                        nc.scalar.activation(p2[:], s2[:], AF.Exp, scale=0.125)
                        return [p2[:, :QCHUNK], p2[:, QCHUNK:]]

                    s_list = [
                        ps.tile([128, QCHUNK], DT, name=f"s_ps{h}", tag="ps_big")
                        for h in range(HPC)
                    ]

                    def exph(h):
                        p_sb = sb.tile([128, QCHUNK], DTM, name="p_sb", tag="p_sb", bufs=p_bufs or ((4 * PRE_KT + 2) if prefill_early else (2 * PRE_KT + 2)))
                        nc.scalar.activation(p_sb[:], s_list[h][:], AF.Exp, scale=0.125)
                        return p_sb

                    if score_order == "il":
                        for h, half in ((0, 0), (1, 0), (1, 1), (0, 1)):
                            mm_into(s_list[h][:, half * 512 : (half + 1) * 512], h, half)
                        p_out = [exph(0), exph(1)]
                    else:  # "seq": per-head scores immediately followed by exp
                        mm_into(s_list[0][:, :512], 0, 0)
                        mm_into(s_list[0][:, 512:], 0, 1)
                        p0 = exph(0)
                        mm_into(s_list[1][:, :512], 1, 0)
                        mm_into(s_list[1][:, 512:], 1, 1)
                        p_out = [p0, exph(1)]
                    return p_out

                def emit_av(kt_i, h, accs, p_sb):
                    for half in range(2):
                        hsl = slice(half * 512, (half + 1) * 512)
                        nc.tensor.matmul(
                            accs[h][: D + 1, hsl],
                            vn_h(kt_i // 4, kt_i % 4, h),
                            p_sb[:, hsl],
                            start=(kt_i == 0), stop=(kt_i == att_nt - 1),
                        )

                def emit_finish_stripe(m, accs):
                    # copy both accumulators out of PSUM first so their banks
                    # free for the next stripe while normalization runs on SBUF
                    acc_sbs = []
                    for h in range(HPC):
                        acc_sb = sb.tile([D + 1, QCHUNK], DTM, name="acc_sb", tag="acc_sb", bufs=2)
                        nc.vector.tensor_copy(acc_sb[:], accs[h][: D + 1, :])
                        acc_sbs.append(acc_sb)
                    for h in range(HPC):
                        hs = slice(h * D, (h + 1) * D)
                        acc_sb = acc_sbs[h]
                        recip = sb.tile([1, QCHUNK], DTM, name="recip", tag="recip", bufs=2)
                        with nc.allow_low_precision(reason="softmax denom in bf16; tol 2e-2"):
                            nc.vector.reciprocal(recip[:], acc_sb[D : D + 1, :])
                        bcast = sb.tile([D, QCHUNK], DTM, name="bcast", tag="bcast", bufs=2)
                        nc.gpsimd.partition_broadcast(bcast[:], recip[:1, :])
                        nc.vector.tensor_mul(att_m[m][hs, :], acc_sb[:D, :], bcast[:])
                    nc.sync.dma_start(
                        a2a_in[m][:].rearrange("a p t -> p a t"),
                        att_m[m][:].rearrange("p (a t) -> p a t", a=N_CORES),
                    )
                    if not skip_a2a:
                        nc.gpsimd.collective_compute(
                            "AllToAll",
                            mybir.AluOpType.bypass,
                            replica_groups=[list(range(N_CORES))],
                            ins=[a2a_in[m].opt()],
                            outs=[a2a_out[m].opt()],
                        )

                def emit_aTm_load(m):
                    aTm = sb.tile([128, N_CORES * 128], DTM, name="aTm", tag="aTm", bufs=2)
                    nc.sync.dma_start(
                        aTm[:].rearrange("p (a t) -> p a t", a=N_CORES),
                        a2a_out[m][:].rearrange("a p t -> p a t"),
                    )
                    return aTm

                def outproj_pieces(m, aTm_ref, cc, npiece=4, tag=None):
                    """output projection for stripe m, 512-column half cc, as
                    npiece thunks accumulating into an SBUF tile via DVE."""
                    os_ = slice(cc * 512, (cc + 1) * 512)
                    per = HT // npiece
                    holder = []
                    thunks = []
                    for pc in range(npiece):
                        def piece(pc=pc):
                            po = ps.tile([128, 512], DT, name="po", tag=tag, bufs=1 if tag == "s2" else None)
                            for i in range(per * pc, per * (pc + 1)):
                                nc.tensor.matmul(
                                    po[:], aTm_ref[0][:, i * 128 : (i + 1) * 128],
                                    wo_sb[i][:, os_],
                                    start=(i == per * pc), stop=(i == per * (pc + 1) - 1),
                                )
                            if pc == 0:
                                out_sb = sb.tile([128, 512], DT, name="out_sb", tag="out_sb", bufs=2)
                                holder.append(out_sb)
                                nc.vector.tensor_add(out_sb[:], po[:], bo_bc[:, os_])
                            else:
                                out_sb = holder[0]
                                nc.vector.tensor_add(out_sb[:], out_sb[:], po[:])
                            if pc == npiece - 1:
                                nc.sync.dma_start(out[m * 128 : (m + 1) * 128, os_], out_sb[:])
                        thunks.append(piece)
                    return thunks

                # ---- schedule ----------------------------------------------
                def new_accs():
                    return [
                        ps.tile([128, QCHUNK], DT, name=f"acc{h}", tag="ps_acc")
                        for h in range(HPC)
                    ]

                # phase 1: stream chunks with stripe-0 attention interleaved at
                # lag 1 (chunk cp delivers k-tiles 4cp..4cp+3; attention trails
                # one chunk behind so exp work reaches ACT as early as possible).
                # Phase 1 is PE/supply-bound, so projection blocks sit between
                # attention groups without extra cost.
                # chunk-1's q runs before chunk-0's v so the first scores+exp
                # fire as early as possible
                emit_weight_loads()
                emit_xt(0)
                emit_xt(1)
                for t in k_pieces(0) + q_pieces(0) + q_pieces(1) + v_pieces(0):
                    t()
                # out-proj weights load early on the SP queue (x loads are on
                # Pool, so these only queue behind qkv weights)
                for i in range(HT):
                    nc.sync.dma_start(wo_sb[i][:], wo[i * 128 : (i + 1) * 128, :])
                bo_sb = sb.tile([1, HIDDEN], DT)
                nc.sync.dma_start(bo_sb[:], bo[:])
                nc.gpsimd.partition_broadcast(bo_bc[:], bo_sb[:1, :])

                accs = new_accs()
                stash0 = []

                def unit0kt(kt_i):
                    # AVs first: their p_sb inputs are PRE_KT k-tiles old, so
                    # they are never gated -- emitting them before the (gated)
                    # score matmuls keeps the in-order PE queue busy during
                    # the previous k-tile's exp instead of stalling behind it
                    if av_first and len(stash0) >= 2 * PRE_KT:
                        for _ in range(2):
                            pk, ph, pp = stash0.pop(0)
                            emit_av(pk, ph, accs, pp)
                    stash0.extend(zip((kt_i, kt_i), (0, 1), emit_scores_exp2(0, kt_i)))
                    if not av_first and len(stash0) > 2 * PRE_KT:
                        for _ in range(2):
                            pk, ph, pp = stash0.pop(0)
                            emit_av(pk, ph, accs, pp)

                for cp in range(1, HT):
                    if cp > 1:
                        emit_xt(cp)
                        # only stripe 1's q (chunks 2-3) projects in phase 1;
                        # stripes 2-3's q rides the later stripe boundaries where
                        # the exp backlog absorbs it, shrinking the per-chunk
                        # block that starves ACT here
                        pieces = k_pieces(cp) + (q_pieces(cp) if cp < 4 else []) + v_pieces(cp)
                    else:
                        pieces = k_pieces(cp) + v_pieces(cp)
                    a = 4 * (cp - 1)
                    for kk in range(4):
                        unit0kt(a + kk)
                        # one consolidated projection block per chunk (after
                        # the unit: phase 1 is supply-bound and the exps must
                        # start as early as possible; tried before-the-unit,
                        # measured slightly worse)
                        if kk == 1:
                            for t in pieces:
                                t()
                for kt_i in range(4 * (HT - 1), att_nt):
                    unit0kt(kt_i)

                def prefill(m):
                    st = []
                    for kt in range(PRE_KT):
                        st.extend(zip((kt, kt), (0, 1), emit_scores_exp2(m, kt)))
                    return st

                # emit the next stripe's pre-fill scores+exps BEFORE draining
                # this stripe's AV tail: the drain plus the following boundary
                # blocks then run against a fresh exp backlog instead of an
                # idle ACT engine
                next_stash = prefill(1) if prefill_early else None
                for pk, ph, pp in stash0:
                    emit_av(pk, ph, accs, pp)
                emit_finish_stripe(0, accs)

                # Stripe k's out-projection runs at the START boundary of stripe
                # k+2: its AllToAll is a full stripe old (no collective wait) and
                # the po blocks allocate from the just-freed ps_acc buffers, so
                # the exp stream's s_ps rotation is never interrupted mid-stripe.
                # The new stripe's first AV matmuls lag behind the po blocks but
                # the p_sb triple-buffer absorbs that.
                aTms = {}
                for m in range(1, NQC):
                    # boundary block: previous-previous stripe's out-projection
                    # plus the q projections for stripe m+1, all allocated from
                    # the just-freed ps_acc buffers so the s_ps rotation is
                    # untouched
                    blocks = []
                    if m >= 2:
                        for cc in range(2):
                            blocks += outproj_pieces(m - 2, [aTms[m - 2]], cc, npiece=1, tag="ps_acc")
                    if m < NQC - 1:
                        blocks += q_pieces(2 * (m + 1), tag="ps_acc")
                        blocks += q_pieces(2 * (m + 1) + 1, tag="ps_acc")
                    # pre-emit PRE_KT k-tiles of scores+exp so ACT stays fed
                    # while the boundary blocks occupy the in-order PE stream;
                    # the whole stripe then runs with AV matmuls lagging
                    # 2*PRE_KT units behind their scores, so the deferred AVs
                    # interleave with new scores instead of bunching up after
                    # the blocks
                    if prefill_early:
                        stash = next_stash
                        for t in blocks:
                            t()
                    elif blocks_first:
                        # boundary blocks are never gated (aTm a full stripe
                        # old, x resident) -- ahead of the pre-emit scores
                        # they keep the in-order PE queue busy
                        stash = []
                        for t in blocks:
                            t()
                        for kt in range(PRE_KT):
                            stash.extend(zip((kt, kt), (0, 1), emit_scores_exp2(m, kt)))
                    else:
                        stash = []
                        for kt in range(PRE_KT):
                            stash.extend(zip((kt, kt), (0, 1), emit_scores_exp2(m, kt)))
                        for t in blocks:
                            t()
                    accs = new_accs()
                    for kt_i in range(PRE_KT, att_nt):
                        if kt_i == att_nt - 8:
                            aTms[m - 1] = emit_aTm_load(m - 1)
                        # AVs first (inputs PRE_KT k-tiles old, never gated):
                        # the in-order PE queue stays busy during the previous
                        # k-tile's exp instead of stalling behind the gated
                        # score matmuls
                        p01 = None
                        if not av_first:
                            p01 = emit_scores_exp2(m, kt_i)
                        for _ in range(2):
                            pk, ph, pp = stash.pop(0)
                            emit_av(pk, ph, accs, pp)
                        # last stripe: no more blocks need the backlog, so drain
                        # the lag early (one extra AV per k-tile keeps PE and
                        # ACT balanced) instead of flushing it as pure tail
                        if m == NQC - 1 and len(stash) > 2:
                            pk, ph, pp = stash.pop(0)
                            emit_av(pk, ph, accs, pp)
                        if p01 is None:
                            p01 = emit_scores_exp2(m, kt_i)
                        stash.extend(zip((kt_i, kt_i), (0, 1), p01))
                    if prefill_early and m < NQC - 1:
                        next_stash = prefill(m + 1)
                    for pk, ph, pp in stash:
                        emit_av(pk, ph, accs, pp)
                    emit_finish_stripe(m, accs)
                # tail: stripe 2's projection hides under stripe 3's AllToAll
                for cc in range(2):
                    for t in outproj_pieces(NQC - 2, [aTms[NQC - 2]], cc, npiece=1, tag="ps_acc"):
                        t()
                aTm3 = emit_aTm_load(NQC - 1)
                for cc in range(2):
                    for t in outproj_pieces(NQC - 1, [aTm3], cc, npiece=1, tag="ps_acc"):
                        t()

    nc.compile()
    return nc


def _get_nc(mm_mode: str):
    if mm_mode not in _CACHE:
        _CACHE[mm_mode] = _build(mm_mode)
    return _CACHE[mm_mode]


def make_in_maps(x, w_qkv, b_qkv, w_out, b_out):
    import ml_dtypes

    bf16 = ml_dtypes.bfloat16
    x = np.asarray(x, dtype=np.float32)
    w_qkv = np.asarray(w_qkv, dtype=np.float32)
    b_qkv = np.asarray(b_qkv, dtype=np.float32)
    w_out = np.asarray(w_out, dtype=np.float32)
    b_out = np.asarray(b_out, dtype=np.float32)

    xT = x.reshape(N, HIDDEN).T  # [hidden, n]
    # permute n into stripe order n' = (m, j, t) <-> n = 512*j + 128*m + t
    xT = np.ascontiguousarray(
        xT.reshape(HIDDEN, N_CORES, NQC, 128).transpose(0, 2, 1, 3).reshape(HIDDEN, N)
    ).astype(bf16)
    w_out_bf = np.ascontiguousarray(w_out).astype(bf16)
    bo = np.ascontiguousarray(b_out.reshape(1, HIDDEN))
    in_maps = []
    for c in range(N_CORES):
        cs = slice(c * AD, (c + 1) * AD)
        in_maps.append(
            {
                "xT": xT,
                "wq": np.ascontiguousarray(w_qkv[:, :HIDDEN][:, cs]).astype(bf16),
                "wk": np.ascontiguousarray(w_qkv[:, HIDDEN : 2 * HIDDEN][:, cs]).astype(bf16),
                "wv": np.ascontiguousarray(w_qkv[:, 2 * HIDDEN :][:, cs]).astype(bf16),
                "bq": np.ascontiguousarray(b_qkv[:HIDDEN][cs].reshape(AD, 1)),
                "bk": np.ascontiguousarray(b_qkv[HIDDEN : 2 * HIDDEN][cs].reshape(AD, 1)),
                "bvT": np.ascontiguousarray(b_qkv[2 * HIDDEN :][cs].reshape(1, AD)),
                "wo": w_out_bf,
                "bo": bo,
            }
        )
    return in_maps


def kernel(x, w_qkv, b_qkv, w_out, b_out):
    from concourse.bass_utils import run_bass_kernel_spmd

    mm_mode = os.environ.get("TRN_MM_MODE", "bf16")
    nc = _get_nc(mm_mode)
    in_maps = make_in_maps(x, w_qkv, b_qkv, w_out, b_out)
    res = run_bass_kernel_spmd(nc, in_maps, list(range(N_CORES)))
    full = np.concatenate([res.results[c]["out"] for c in range(N_CORES)], axis=0)
    return full.reshape(1, N, HIDDEN).astype(np.float32)

